# revision 9
# baseline (speedup 1.0000x reference)
"""Trainium2 Bass kernel for a single attention layer (Baichuan-style W_pack
attention with rotary embeddings), sharded over 8 NeuronCores:
tensor-parallel over 4 head groups x data-parallel over 2 batches.

v3: scheduling overhaul toward the bf16 PE stream floor (~1.01 ms):
 - V-proj runs before QK-proj in each half so the x DMA stream is consumed
   at arrival rate (kills the ~20 us x-wait stall at each half start).
 - softmax denominator: single all-ones [128,128] stationary matmul whose
   PSUM output is den replicated across all partitions (no [1,512] recip
   chain, no separate broadcast matmul), fed by an incremental DVE tree +
   fold so there is one den matmul per (head, q-chunk).
 - exp batched per 2 score blocks (one ACTIVATE over [128,1024]) off the
   diagonal; av matmuls trail their exp by two groups so the PE never
   waits on the ACT exp.
 - O-proj work for seq groups 0,1 is interleaved into the qc2/qc3
   attention emission to fill residual exp-chain bubbles.

Contract: kernel(**inputs) takes the FULL unsharded inputs and returns the
FULL output [2, 2048, 4096] float32. All sharding / gathering happens here.
"""

import math
import sys

import numpy as np

for _p in ("/opt/trn_rl_repo", "/root/.axon_site/_ro/trn_rl_repo"):
    if _p not in sys.path:
        sys.path.insert(0, _p)

HIDDEN = 4096
N_HEADS = 32
HEAD_DIM = 128
BASE = 10000.0
B = 2
S = 2048
HEADS_PER_CORE = 8          # 32 heads / 4 groups
HG = 1024                   # head-group width = 8 heads * 128
NEG_BIG = -1.0e9

# RoPE partner permutation: quadrant q holds [lo_d 16q..16q+15, hi_d 64+16q..]
# so the rotate-half partner of new-row i is i+-16 inside its 32-row quadrant,
# reachable by DVE stream_shuffle.
PERM = np.zeros(128, dtype=np.int64)
for _q in range(4):
    PERM[32 * _q: 32 * _q + 16] = np.arange(16 * _q, 16 * _q + 16)
    PERM[32 * _q + 16: 32 * _q + 32] = 64 + np.arange(16 * _q, 16 * _q + 16)
SHUF_MASK = [(i + 16) % 32 for i in range(32)]
# sign of the sin term per (new) row: -1 where original d < 64
SIGN = np.where(PERM < 64, -1.0, 1.0).astype(np.float32)


def _rope_tables(max_pos):
    inv_freq = 1.0 / (BASE ** (np.arange(0, HEAD_DIM, 2, dtype=np.float32) / HEAD_DIM))
    t = np.arange(max_pos, dtype=np.float32)
    freqs = np.outer(t, inv_freq)                      # [P, 64]
    emb = np.concatenate((freqs, freqs), axis=-1)      # [P, 128]
    return np.cos(emb).astype(np.float32), np.sin(emb).astype(np.float32)


def _build_program(mask_mode):
    """mask_mode: 'causal' (block-skip + shared triangle mask),
    'none' (dense, no mask), 'full' (dense, stream mask tiles)."""
    import concourse.bacc as bacc
    import concourse.mybir as mybir
    import concourse.tile as tile
    from contextlib import ExitStack

    F32 = mybir.dt.float32
    BF16 = mybir.dt.bfloat16
    ALU = mybir.AluOpType
    ACTF = mybir.ActivationFunctionType

    nc = bacc.Bacc("TRN2", target_bir_lowering=False, debug=False)

    # pre-tiled inputs (host side does all layout work)
    x_t = nc.declare_dram_parameter("x_t", [128, 32, S], BF16, isOutput=False)
    wqk_t = nc.declare_dram_parameter("wqk_t", [16, 128, 32, 128], BF16,
                                      isOutput=False)
    wv_t = nc.declare_dram_parameter("wv_t", [2, 32, 128, 512], BF16,
                                     isOutput=False)
    wo_t = nc.declare_dram_parameter("wo_t", [128, 8, HIDDEN], BF16,
                                     isOutput=False)
    cosT = nc.declare_dram_parameter("cosT", [128, S], BF16, isOutput=False)
    sinT = nc.declare_dram_parameter("sinT", [128, S], BF16, isOutput=False)
    if mask_mode == "causal":
        tri = nc.declare_dram_parameter("tri", [128, 128], F32, isOutput=False)
    elif mask_mode == "full":
        maskT = nc.declare_dram_parameter("maskT", [S, S], F32, isOutput=False)
    out_p = nc.declare_dram_parameter("out_p", [S, HIDDEN], BF16, isOutput=True)

    at_s = nc.dram_tensor("at_scratch", [HG, S], BF16)

    inv_sqrt_d = 1.0 / math.sqrt(HEAD_DIM)

    with tile.TileContext(nc, pool_alloc_mode="queue") as tc, ExitStack() as top:
        const_pool = top.enter_context(tc.tile_pool(name="consts", bufs=1))
        ones_f32 = const_pool.tile([128, 128], F32)
        nc.vector.memset(ones_f32, 1.0)
        ones_sq = const_pool.tile([128, 128], BF16)
        nc.vector.tensor_copy(ones_sq, ones_f32)
        if mask_mode == "causal":
            tri_sb = const_pool.tile([128, 128], F32)
            nc.sync.dma_start(out=tri_sb, in_=tri.ap())

        # SBUF-resident q/k/v, one tile per seq half to keep deps precise
        res_pool = top.enter_context(tc.tile_pool(name="resident", bufs=1))
        qT_sb = [res_pool.tile([128, HEADS_PER_CORE, 1024], BF16,
                               name=f"qT{h}") for h in range(2)]
        kT_sb = [res_pool.tile([128, HEADS_PER_CORE, 1024], BF16,
                               name=f"kT{h}") for h in range(2)]
        v_sb = [res_pool.tile([128, 8, HG], BF16, name=f"v{h}")
                for h in range(2)]

        # ---------------- Phase A: V then QK projection per half -------------
        def emit_half(hs, pha):
            s0 = hs * 1024
            # h-chunk sweep order: B-block (16..31) first (its DMA is issued
            # first), A-block (0..15) last.
            H_ORDER = list(range(16, 32)) + list(range(16))
            xpoolA = pha.enter_context(tc.tile_pool(name="xhalfA", bufs=1))
            xpoolB = pha.enter_context(tc.tile_pool(name="xhalfB", bufs=1))
            xtA = xpoolA.tile([128, 16, 1024], BF16, name=f"xtA{hs}")
            xtB = xpoolB.tile([128, 16, 1024], BF16, name=f"xtB{hs}")
            xin = x_t.ap()[:, :, s0:s0 + 1024]
            for c in range(16, 32):
                if hs == 0 and c == 16:
                    with tc.high_priority():
                        nc.sync.dma_start(out=xtB[:, c - 16, :], in_=xin[:, c, :])
                else:
                    nc.sync.dma_start(out=xtB[:, c - 16, :], in_=xin[:, c, :])
            for c in range(16):
                nc.sync.dma_start(out=xtA[:, c, :], in_=xin[:, c, :])

            def xt_slice(c, sl):
                return xtB[:, c - 16, sl] if c >= 16 else xtA[:, c, sl]

            # --- v projection first, x-stationary, out = v [s, o] ------------
            # consumes x chunks at DMA arrival rate (no x-wait stall)
            with ExitStack() as vv:
                wvp = vv.enter_context(tc.tile_pool(name="wvt", bufs=4))
                pv = vv.enter_context(
                    tc.tile_pool(name="pv", bufs=8, space="PSUM"))
                for ov in range(2):          # v-dim chunks of 512
                    vb = [pv.tile([128, 512], F32, tag="vb", name=f"vb{i}")
                          for i in range(8)]
                    for hi, h in enumerate(H_ORDER):
                        wv_tile = wvp.tile([128, 512], BF16, tag="wv_tile")
                        if hs == 0 and ov == 0 and hi == 0:
                            with tc.high_priority():
                                nc.sync.dma_start(out=wv_tile,
                                                  in_=wv_t.ap()[ov, h])
                        else:
                            nc.sync.dma_start(out=wv_tile, in_=wv_t.ap()[ov, h])
                        for sc in range(8):
                            nc.tensor.matmul(
                                vb[sc],
                                xt_slice(h, slice(sc * 128, (sc + 1) * 128)),
                                wv_tile,
                                start=(hi == 0), stop=(hi == 31))
                    for sc in range(8):
                        dst = v_sb[hs][:, sc, ov * 512:(ov + 1) * 512]
                        if sc % 2 == 0:
                            nc.scalar.activation(dst, vb[sc], ACTF.Copy)
                        else:
                            nc.vector.tensor_copy(dst, vb[sc])

            # --- q,k projection, weight-stationary, out = projT [o, s] -------
            with ExitStack() as qk:
                cspool = qk.enter_context(tc.tile_pool(name="cossin", bufs=1))
                cos_sb = cspool.tile([128, 1024], BF16)
                nc.sync.dma_start(out=cos_sb, in_=cosT.ap()[:, s0:s0 + 1024])
                sin_sb = cspool.tile([128, 1024], BF16)
                nc.sync.dma_start(out=sin_sb, in_=sinT.ap()[:, s0:s0 + 1024])

                wpool = qk.enter_context(tc.tile_pool(name="wqk", bufs=2))
                pqk = qk.enter_context(
                    tc.tile_pool(name="pqk", bufs=2, space="PSUM"))
                rpool = qk.enter_context(tc.tile_pool(name="rope", bufs=2))
                for oc in range(16):         # o chunks of 128 (head tiles)
                    w_oc = wpool.tile([128, 32, 128], BF16, tag="w_oc")
                    nc.sync.dma_start(out=w_oc, in_=wqk_t.ap()[oc])
                    pk = pqk.tile([128, 2, 512], F32, tag="pk")
                    for hi, h in enumerate(H_ORDER):
                        for sc in range(2):
                            nc.tensor.matmul(
                                pk[:, sc, :], w_oc[:, h, :],
                                xt_slice(h, slice(sc * 512, (sc + 1) * 512)),
                                start=(hi == 0), stop=(hi == 31))
                    # RoPE: q' = q*cos + shuffle16(q)*sin_signed, -> bf16
                    dst = qT_sb[hs] if oc < 8 else kT_sb[hs]
                    hh = oc % 8
                    for sc in range(2):
                        pks = pk[:, sc, :]
                        cs = cos_sb[:, sc * 512:(sc + 1) * 512]
                        sn = sin_sb[:, sc * 512:(sc + 1) * 512]
                        qrot = rpool.tile([128, 512], F32, tag="qrot")
                        nc.vector.stream_shuffle(qrot, pks, SHUF_MASK)
                        t1 = rpool.tile([128, 512], F32, tag="t1")
                        nc.vector.tensor_tensor(t1, pks, cs, ALU.mult)
                        t2 = rpool.tile([128, 512], F32, tag="t2")
                        nc.gpsimd.tensor_tensor(t2, qrot, sn, ALU.mult)
                        nc.vector.tensor_tensor(
                            dst[:, hh, sc * 512:(sc + 1) * 512],
                            t1, t2, ALU.add)

        # ---------------- Phase B: attention, scores kept as S^T [k, q] ------
        # Scores go into [128, 2, 512] PSUM tiles (2 k-blocks per group),
        # exp'd in one ACTIVATE per off-diagonal group, accumulated into a
        # running DVE sum, folded to [128, 512], and reduced+broadcast by a
        # single all-ones stationary matmul (den lands replicated on every
        # PSUM partition). av matmuls trail their exp by two groups; the
        # finalize (recip + av*recip + DMA out) is deferred by one unit.
        def make_attn_emitter(phb):
            qp_es = phb.enter_context(tc.tile_pool(name="es", bufs=4))
            esump = phb.enter_context(tc.tile_pool(name="esum", bufs=2))
            smallp = phb.enter_context(tc.tile_pool(name="small", bufs=3))
            ps = phb.enter_context(
                tc.tile_pool(name="ps", bufs=2, space="PSUM"))
            pav = phb.enter_context(
                tc.tile_pool(name="pav", bufs=2, space="PSUM"))
            pmisc = phb.enter_context(
                tc.tile_pool(name="pmisc", bufs=2, space="PSUM"))
            mp = None
            if mask_mode == "full":
                mp = phb.enter_context(tc.tile_pool(name="msk", bufs=3))

            state = {"pending": None}

            def finalize(av, den, hh, qc):
                recip = smallp.tile([128, 512], F32, tag="recip")
                nc.vector.reciprocal_approx_fast(recip, den)
                at_t = smallp.tile([128, 512], BF16, tag="at_t")
                nc.vector.tensor_tensor(at_t, av, recip, ALU.mult)
                nc.sync.dma_start(
                    out=at_s.ap()[hh * 128:(hh + 1) * 128,
                                  qc * 512:(qc + 1) * 512],
                    in_=at_t)

            def emit_unit(qc, hh):
                nblk = 4 * qc + 4 if mask_mode == "causal" else 16
                ng = nblk // 2
                av = pav.tile([128, 512], F32, tag="av")
                acc = None
                pend_av = []          # (es_tile, j2, kb, q_lo) awaiting av mm

                def emit_avs(upto):
                    while pend_av and len(pend_av) > upto:
                        es, j2, kb, q_lo = pend_av.pop(0)
                        qs = slice(q_lo, 512)
                        khalf, kloc = kb // 8, kb % 8
                        nc.tensor.matmul(
                            av[:, qs],
                            v_sb[khalf][:, kloc, hh * 128:(hh + 1) * 128],
                            es[:, j2, qs],
                            start=(kb == 0), stop=(kb == nblk - 1))

                for g in range(ng):
                    es = qp_es.tile([128, 2, 512], BF16, tag="es")
                    sps = ps.tile([128, 2, 512], F32, tag="sps")
                    diag = False
                    blk = []
                    for j2 in range(2):
                        kb = 2 * g + j2
                        vd = kb - 4 * qc   # diagonal block index
                        q_lo = (128 * vd
                                if (mask_mode == "causal" and vd > 0)
                                else 0)
                        qs = slice(q_lo, 512)
                        khalf, kloc = kb // 8, kb % 8
                        nc.tensor.matmul(
                            sps[:, j2, qs],
                            kT_sb[khalf][:, hh,
                                         kloc * 128:(kloc + 1) * 128],
                            qT_sb[qc // 2][:, hh,
                                           (qc % 2) * 512 + q_lo:
                                           (qc % 2) * 512 + 512],
                            start=True, stop=True)
                        if mask_mode == "causal" and vd >= 0:
                            diag = True
                            mq = slice(128 * vd, 128 * vd + 128)
                            nc.vector.tensor_tensor(
                                sps[:, j2, mq], sps[:, j2, mq], tri_sb,
                                ALU.add)
                        elif mask_mode == "full":
                            mt = mp.tile([128, 512], F32, tag="mt")
                            nc.sync.dma_start(
                                out=mt,
                                in_=maskT.ap()[kb * 128:(kb + 1) * 128,
                                               qc * 512:(qc + 1) * 512])
                            nc.vector.tensor_tensor(sps[:, j2, :],
                                                    sps[:, j2, :], mt,
                                                    ALU.add)
                        blk.append((kb, q_lo))
                    # exp: one ACTIVATE per clean group; per-region on the
                    # diagonal (unwritten PSUM slivers stay out of the AP)
                    if not diag:
                        nc.scalar.activation(es, sps, ACTF.Exp,
                                             scale=inv_sqrt_d)
                    else:
                        for j2, (kb, q_lo) in enumerate(blk):
                            if q_lo > 0:
                                nc.gpsimd.memset(es[:, j2, 0:q_lo], 0.0)
                            qs = slice(q_lo, 512)
                            nc.scalar.activation(es[:, j2, qs],
                                                 sps[:, j2, qs],
                                                 ACTF.Exp, scale=inv_sqrt_d)
                    # incremental tree: acc += es (DVE), frees es early
                    if g == 0:
                        acc = es
                    else:
                        if g == 1:
                            t = esump.tile([128, 2, 512], BF16, tag="e2")
                            nc.vector.tensor_tensor(t, acc, es, ALU.add)
                            acc = t
                        else:
                            nc.vector.tensor_tensor(acc, acc, es, ALU.add)
                    for j2, (kb, q_lo) in enumerate(blk):
                        pend_av.append((es, j2, kb, q_lo))
                    # av matmuls trail their exp by two groups
                    emit_avs(4)
                    if state["pending"] is not None and g == min(1, ng - 1):
                        finalize(*state["pending"])
                        state["pending"] = None
                emit_avs(0)
                # fold the two k-block columns -> [128, 512]
                fold = smallp.tile([128, 512], BF16, tag="fold")
                nc.vector.tensor_tensor(fold, acc[:, 0, :], acc[:, 1, :],
                                        ALU.add)
                # den replicated across all 128 partitions via all-ones lhsT
                den = pmisc.tile([128, 512], F32, tag="misc", name="den")
                nc.tensor.matmul(den, ones_sq, fold, start=True, stop=True)
                if state["pending"] is not None:
                    finalize(*state["pending"])
                state["pending"] = (av, den, hh, qc)

            def flush():
                if state["pending"] is not None:
                    finalize(*state["pending"])
                    state["pending"] = None

            return emit_unit, flush, pmisc

        # ---------------- Phase C: output projection -------------------------
        def make_c_emitter(phc, pop):
            atp = phc.enter_context(tc.tile_pool(name="atl", bufs=2))
            otp = phc.enter_context(tc.tile_pool(name="ot", bufs=4))
            wop = phc.enter_context(tc.tile_pool(name="wo", bufs=2))
            state = {"at_g": {}, "ot": {}, "wo_sl": None}

            def load_stg(stg):
                at_g = atp.tile([128, 8, 512], BF16, tag="at_g",
                                name=f"at_g{stg}")
                nc.sync.dma_start(
                    out=at_g,
                    in_=at_s.ap()[:, stg * 512:(stg + 1) * 512].rearrange(
                        "(hc p) s -> p hc s", p=128))
                state["at_g"][stg] = at_g

            def emit_unit(stg, st_l, o8):
                # one PSUM bank: out[st, o8] = sum_hc at^T wo
                # o8-major within a stage: wo slice loads once per (stg, o8),
                # the four ot tiles of the stage stay live until its end.
                at_g = state["at_g"][stg]
                st = stg * 4 + st_l
                sl = slice(st_l * 128, (st_l + 1) * 128)
                if st_l == 0:
                    wo_sl = wop.tile([128, 8, 512], BF16, tag="wo_sl",
                                     name=f"wo{stg}_{o8}")
                    nc.sync.dma_start(
                        out=wo_sl,
                        in_=wo_t.ap()[:, :, o8 * 512:(o8 + 1) * 512])
                    state["wo_sl"] = wo_sl
                wo_sl = state["wo_sl"]
                if o8 == 0:
                    state["ot"][st] = otp.tile([128, 8, 512], BF16, tag="ot",
                                               name=f"ot{st}")
                ot = state["ot"][st]
                po = pop.tile([128, 512], F32, tag="misc", name="po")
                for hc in range(8):
                    nc.tensor.matmul(
                        po,
                        at_g[:, hc, sl],
                        wo_sl[:, hc, :],
                        start=(hc == 0), stop=(hc == 7))
                nc.scalar.activation(ot[:, o8, :], po, ACTF.Copy)
                if o8 == 7:
                    nc.sync.dma_start(
                        out=out_p.ap()[st * 128:(st + 1) * 128, :],
                        in_=ot.rearrange("p a b -> p (a b)"))
                    del state["ot"][st]

            return emit_unit, load_stg

        # ================= schedule =================
        with ExitStack() as ph0:
            emit_half(0, ph0)

        if mask_mode == "causal":
            # B01: heads x (qc0, qc1); x0 freed, at_s rows for stg 0,1 written
            with ExitStack() as phb:
                emit_unit_b, flush_b, _ = make_attn_emitter(phb)
                for qc in (0, 1):
                    for hh in range(HEADS_PER_CORE):
                        emit_unit_b(qc, hh)
                flush_b()

            with ExitStack() as ph1:
                emit_half(1, ph1)

            # B23 with C(stg0, stg1) interleaved
            with ExitStack() as phbc:
                emit_unit_b, flush_b, pmisc = make_attn_emitter(phbc)
                emit_unit_c, load_stg = make_c_emitter(phbc, pmisc)
                load_stg(0)
                load_stg(1)
                c_units = [(stg, st_l, o8)
                           for stg in (0, 1)
                           for o8 in range(8)
                           for st_l in range(4)]
                for qc in (2, 3):
                    for hh in range(HEADS_PER_CORE):
                        emit_unit_b(qc, hh)
                        # ~4 C units per B unit balances the two streams
                        for _ in range(4):
                            if c_units:
                                emit_unit_c(*c_units.pop(0))
                flush_b()
                while c_units:
                    emit_unit_c(*c_units.pop(0))

                # C(stg2, stg3) dense
                load_stg(2)
                load_stg(3)
                for stg in (2, 3):
                    for o8 in range(8):
                        for st_l in range(4):
                            emit_unit_c(stg, st_l, o8)
        else:
            with ExitStack() as ph1:
                emit_half(1, ph1)
            with ExitStack() as phbc:
                emit_unit_b, flush_b, pmisc = make_attn_emitter(phbc)
                emit_unit_c, load_stg = make_c_emitter(phbc, pmisc)
                for qc in range(4):
                    for hh in range(HEADS_PER_CORE):
                        emit_unit_b(qc, hh)
                # all units emitted; flush and drain C for every stg in order
                flush_b()
                for stg in range(4):
                    load_stg(stg)
                    for o8 in range(8):
                        for st_l in range(4):
                            emit_unit_c(stg, st_l, o8)

    nc.compile()
    return nc


_PROGRAM_CACHE = {}


def _get_program(mask_mode):
    if mask_mode not in _PROGRAM_CACHE:
        _PROGRAM_CACHE[mask_mode] = _build_program(mask_mode)
    return _PROGRAM_CACHE[mask_mode]


def _classify_mask(attention_mask):
    m = np.asarray(attention_mask)
    if not np.any(m):
        return "none"
    neg = np.float32(np.finfo(np.float32).min)
    causal = np.triu(np.full((S, S), neg, dtype=np.float32), k=1)
    for b in range(m.shape[0]):
        if not np.array_equal(m[b, 0], causal):
            return "full"
    return "causal"


def _prep_core_inputs(hidden_states, attention_mask, position_ids, W_pack, W_o,
                      mask_mode):
    from ml_dtypes import bfloat16

    hidden_states = np.asarray(hidden_states, dtype=np.float32)
    W_pack = np.asarray(W_pack, dtype=np.float32)
    W_o = np.asarray(W_o, dtype=np.float32)
    pos = np.asarray(position_ids).astype(np.int64)

    cos_t, sin_t = _rope_tables(int(pos.max()) + 1)
    # per-batch gathered + transposed + row-permuted (+ sign folded into sin)
    cosT_b, sinT_b = [], []
    for b in range(B):
        c = cos_t[pos[b]][:, PERM].T
        s = (sin_t[pos[b]][:, PERM] * SIGN[None, :]).T
        cosT_b.append(np.ascontiguousarray(c.astype(bfloat16)))
        sinT_b.append(np.ascontiguousarray(s.astype(bfloat16)))

    # x_t[p, c, s] = hidden[b, s, c*128+p]
    x_b = [np.ascontiguousarray(
        hidden_states[b].T.reshape(32, 128, S).transpose(1, 0, 2)
        .astype(bfloat16)) for b in range(B)]

    tri_m = None
    maskT_b = None
    if mask_mode == "causal":
        kk = np.arange(128)[:, None]
        qq = np.arange(128)[None, :]
        tri_m = np.where(kk <= qq, 0.0, NEG_BIG).astype(np.float32)
    elif mask_mode == "full":
        m = np.asarray(attention_mask, dtype=np.float32)
        maskT_b = [np.ascontiguousarray(m[b, 0].T) for b in range(B)]

    in_maps = []
    for cidx in range(8):
        b, g = cidx // 4, cidx % 4
        # per-head d-permuted q/k weight rows, head-major columns in wqk
        qrows = np.concatenate(
            [g * HG + hh * 128 + PERM for hh in range(HEADS_PER_CORE)])
        krows = HIDDEN + qrows
        vrows = 2 * HIDDEN + g * HG + np.arange(HG)
        wqk = np.concatenate([W_pack[qrows], W_pack[krows]], axis=0)  # [2048,4096]
        # wqk_t[oc, p, c, o] = wqk[oc*128+o, c*128+p]
        wqk_t = np.ascontiguousarray(
            wqk.reshape(16, 128, 32, 128).transpose(0, 3, 2, 1)
            .astype(bfloat16))
        wv = W_pack[vrows]                                            # [1024,4096]
        # wv_t[ov, c, p, o] = wv[ov*512+o, c*128+p]
        wv_t = np.ascontiguousarray(
            wv.reshape(2, 512, 32, 128).transpose(0, 2, 3, 1).astype(bfloat16))
        # wo_t[p, hc, o] = W_o[o, g*HG + hc*128 + p]
        wo_t = np.ascontiguousarray(
            W_o[:, g * HG:(g + 1) * HG].reshape(HIDDEN, 8, 128)
            .transpose(2, 1, 0).astype(bfloat16))
        im = {"x_t": x_b[b], "wqk_t": wqk_t, "wv_t": wv_t, "wo_t": wo_t,
              "cosT": cosT_b[b], "sinT": sinT_b[b]}
        if mask_mode == "causal":
            im["tri"] = tri_m
        elif mask_mode == "full":
            im["maskT"] = maskT_b[b]
        in_maps.append(im)
    return in_maps


def _run(hidden_states, attention_mask, position_ids, W_pack, W_o,
         trace=False, trace_kwargs=None):
    from concourse.bass_utils import run_bass_kernel_spmd

    mask_mode = _classify_mask(attention_mask)
    nc = _get_program(mask_mode)
    in_maps = _prep_core_inputs(hidden_states, attention_mask, position_ids,
                                W_pack, W_o, mask_mode)
    try:
        res = run_bass_kernel_spmd(nc, in_maps, list(range(8)), trace=trace,
                                   **(trace_kwargs or {}))
    except Exception:
        # transient NRT_EXEC_UNIT_UNRECOVERABLE wedges recover on retry
        import time as _time
        _time.sleep(15)
        res = run_bass_kernel_spmd(nc, in_maps, list(range(8)), trace=trace,
                                   **(trace_kwargs or {}))
    out = np.zeros((B, S, HIDDEN), dtype=np.float32)
    for c in range(8):
        out[c // 4] += np.asarray(res.results[c]["out_p"], dtype=np.float32)
    return out, res


def kernel(hidden_states, attention_mask, position_ids, W_pack, W_o):
    out, _ = _run(hidden_states, attention_mask, position_ids, W_pack, W_o)
    return out


# revision 14
# speedup vs baseline: 1.0452x; 1.0452x over previous
"""Trainium2 Bass kernel for a single attention layer (Baichuan-style W_pack
attention with rotary embeddings), sharded over 8 NeuronCores:
tensor-parallel over 4 head groups x data-parallel over 2 batches.

v3: scheduling overhaul toward the bf16 PE stream floor (~1.01 ms):
 - V-proj runs before QK-proj in each half so the x DMA stream is consumed
   at arrival rate (kills the ~20 us x-wait stall at each half start).
 - softmax denominator: single all-ones [128,128] stationary matmul whose
   PSUM output is den replicated across all partitions (no [1,512] recip
   chain, no separate broadcast matmul), fed by an incremental DVE tree +
   fold so there is one den matmul per (head, q-chunk).
 - exp batched per 2 score blocks (one ACTIVATE over [128,1024]) off the
   diagonal; av matmuls trail their exp by two groups so the PE never
   waits on the ACT exp.
 - O-proj work for seq groups 0,1 is interleaved into the qc2/qc3
   attention emission to fill residual exp-chain bubbles.

Contract: kernel(**inputs) takes the FULL unsharded inputs and returns the
FULL output [2, 2048, 4096] float32. All sharding / gathering happens here.
"""

import math
import sys

import numpy as np

for _p in ("/opt/trn_rl_repo", "/root/.axon_site/_ro/trn_rl_repo"):
    if _p not in sys.path:
        sys.path.insert(0, _p)

HIDDEN = 4096
N_HEADS = 32
HEAD_DIM = 128
BASE = 10000.0
B = 2
S = 2048
HEADS_PER_CORE = 8          # 32 heads / 4 groups
HG = 1024                   # head-group width = 8 heads * 128
NEG_BIG = -1.0e9

# RoPE partner permutation: quadrant q holds [lo_d 16q..16q+15, hi_d 64+16q..]
# so the rotate-half partner of new-row i is i+-16 inside its 32-row quadrant,
# reachable by DVE stream_shuffle.
PERM = np.zeros(128, dtype=np.int64)
for _q in range(4):
    PERM[32 * _q: 32 * _q + 16] = np.arange(16 * _q, 16 * _q + 16)
    PERM[32 * _q + 16: 32 * _q + 32] = 64 + np.arange(16 * _q, 16 * _q + 16)
SHUF_MASK = [(i + 16) % 32 for i in range(32)]
# sign of the sin term per (new) row: -1 where original d < 64
SIGN = np.where(PERM < 64, -1.0, 1.0).astype(np.float32)


def _rope_tables(max_pos):
    inv_freq = 1.0 / (BASE ** (np.arange(0, HEAD_DIM, 2, dtype=np.float32) / HEAD_DIM))
    t = np.arange(max_pos, dtype=np.float32)
    freqs = np.outer(t, inv_freq)                      # [P, 64]
    emb = np.concatenate((freqs, freqs), axis=-1)      # [P, 128]
    return np.cos(emb).astype(np.float32), np.sin(emb).astype(np.float32)


def _build_program(mask_mode):
    """mask_mode: 'causal' (block-skip + shared triangle mask),
    'none' (dense, no mask), 'full' (dense, stream mask tiles)."""
    import concourse.bacc as bacc
    import concourse.mybir as mybir
    import concourse.tile as tile
    from contextlib import ExitStack

    F32 = mybir.dt.float32
    BF16 = mybir.dt.bfloat16
    ALU = mybir.AluOpType
    ACTF = mybir.ActivationFunctionType

    nc = bacc.Bacc("TRN2", target_bir_lowering=False, debug=False)

    # pre-tiled inputs (host side does all layout work)
    x_t = nc.declare_dram_parameter("x_t", [128, 32, S], BF16, isOutput=False)
    wqk_t = nc.declare_dram_parameter("wqk_t", [16, 128, 32, 128], BF16,
                                      isOutput=False)
    wv_t = nc.declare_dram_parameter("wv_t", [2, 32, 128, 512], BF16,
                                     isOutput=False)
    wo_t = nc.declare_dram_parameter("wo_t", [128, 8, HIDDEN], BF16,
                                     isOutput=False)
    cosT = nc.declare_dram_parameter("cosT", [128, S], BF16, isOutput=False)
    sinT = nc.declare_dram_parameter("sinT", [128, S], BF16, isOutput=False)
    if mask_mode == "causal":
        tri = nc.declare_dram_parameter("tri", [128, 128], F32, isOutput=False)
    elif mask_mode == "full":
        maskT = nc.declare_dram_parameter("maskT", [S, S], F32, isOutput=False)
    out_p = nc.declare_dram_parameter("out_p", [S, HIDDEN], BF16, isOutput=True)

    at_s = nc.dram_tensor("at_scratch", [HG, S], BF16)

    inv_sqrt_d = 1.0 / math.sqrt(HEAD_DIM)

    with tile.TileContext(nc, pool_alloc_mode="queue") as tc, ExitStack() as top:
        const_pool = top.enter_context(tc.tile_pool(name="consts", bufs=1))
        ones_f32 = const_pool.tile([128, 128], F32)
        nc.vector.memset(ones_f32, 1.0)
        ones_sq = const_pool.tile([128, 128], BF16)
        nc.vector.tensor_copy(ones_sq, ones_f32)
        if mask_mode == "causal":
            tri_sb = const_pool.tile([128, 128], F32)
            nc.sync.dma_start(out=tri_sb, in_=tri.ap())

        # SBUF-resident q/k/v, one tile per seq half to keep deps precise
        res_pool = top.enter_context(tc.tile_pool(name="resident", bufs=1))
        qT_sb = [res_pool.tile([128, HEADS_PER_CORE, 1024], BF16,
                               name=f"qT{h}") for h in range(2)]
        kT_sb = [res_pool.tile([128, HEADS_PER_CORE, 1024], BF16,
                               name=f"kT{h}") for h in range(2)]
        v_sb = [res_pool.tile([128, 8, HG], BF16, name=f"v{h}")
                for h in range(2)]

        # ---------------- Phase A: V then QK projection per half -------------
        def emit_half(hs, pha):
            s0 = hs * 1024
            # h-chunk sweep order: B-block (16..31) first (its DMA is issued
            # first), A-block (0..15) last.
            H_ORDER = list(range(16, 32)) + list(range(16))
            xpoolA = pha.enter_context(tc.tile_pool(name="xhalfA", bufs=1))
            xpoolB = pha.enter_context(tc.tile_pool(name="xhalfB", bufs=1))
            xtA = xpoolA.tile([128, 16, 1024], BF16, name=f"xtA{hs}")
            xtB = xpoolB.tile([128, 16, 1024], BF16, name=f"xtB{hs}")
            xin = x_t.ap()[:, :, s0:s0 + 1024]

            def emit_x_dma(c):
                dst = xtB[:, c - 16, :] if c >= 16 else xtA[:, c, :]
                nc.sync.dma_start(out=dst, in_=xin[:, c, :])

            def xt_slice(c, sl):
                return xtB[:, c - 16, sl] if c >= 16 else xtA[:, c, sl]

            # --- v projection first, x-stationary, out = v [s, o] ------------
            # x-chunk DMAs are interleaved with the wv tile loads in
            # consumption order so the first pass streams at DMA rate
            with ExitStack() as vv:
                wvp = vv.enter_context(tc.tile_pool(name="wvt", bufs=4))
                pv = vv.enter_context(
                    tc.tile_pool(name="pv", bufs=8, space="PSUM"))
                for ov in range(2):          # v-dim chunks of 512
                    vb = [pv.tile([128, 512], F32, tag="vb", name=f"vb{i}")
                          for i in range(8)]
                    for hi, h in enumerate(H_ORDER):
                        wv_tile = wvp.tile([128, 512], BF16, tag="wv_tile")
                        nc.sync.dma_start(out=wv_tile, in_=wv_t.ap()[ov, h])
                        if ov == 0:
                            emit_x_dma(h)
                        for sc in range(8):
                            nc.tensor.matmul(
                                vb[sc],
                                xt_slice(h, slice(sc * 128, (sc + 1) * 128)),
                                wv_tile,
                                start=(hi == 0), stop=(hi == 31))
                    for sc in range(8):
                        dst = v_sb[hs][:, sc, ov * 512:(ov + 1) * 512]
                        if sc % 2 == 0:
                            nc.scalar.activation(dst, vb[sc], ACTF.Copy)
                        else:
                            nc.vector.tensor_copy(dst, vb[sc])

            # --- q,k projection, weight-stationary, out = projT [o, s] -------
            with ExitStack() as qk:
                cspool = qk.enter_context(tc.tile_pool(name="cossin", bufs=1))
                cos_sb = cspool.tile([128, 1024], BF16)
                nc.sync.dma_start(out=cos_sb, in_=cosT.ap()[:, s0:s0 + 1024])
                sin_sb = cspool.tile([128, 1024], BF16)
                nc.sync.dma_start(out=sin_sb, in_=sinT.ap()[:, s0:s0 + 1024])

                wpool = qk.enter_context(tc.tile_pool(name="wqk", bufs=2))
                pqk = qk.enter_context(
                    tc.tile_pool(name="pqk", bufs=2, space="PSUM"))
                rpool = qk.enter_context(tc.tile_pool(name="rope", bufs=2))
                for oc in range(16):         # o chunks of 128 (head tiles)
                    w_oc = wpool.tile([128, 32, 128], BF16, tag="w_oc")
                    nc.sync.dma_start(out=w_oc, in_=wqk_t.ap()[oc])
                    pk = pqk.tile([128, 2, 512], F32, tag="pk")
                    for hi, h in enumerate(H_ORDER):
                        for sc in range(2):
                            nc.tensor.matmul(
                                pk[:, sc, :], w_oc[:, h, :],
                                xt_slice(h, slice(sc * 512, (sc + 1) * 512)),
                                start=(hi == 0), stop=(hi == 31))
                    # RoPE: q' = q*cos + shuffle16(q)*sin_signed, -> bf16
                    dst = qT_sb[hs] if oc < 8 else kT_sb[hs]
                    hh = oc % 8
                    for sc in range(2):
                        pks = pk[:, sc, :]
                        cs = cos_sb[:, sc * 512:(sc + 1) * 512]
                        sn = sin_sb[:, sc * 512:(sc + 1) * 512]
                        qrot = rpool.tile([128, 512], F32, tag="qrot")
                        nc.vector.stream_shuffle(qrot, pks, SHUF_MASK)
                        t1 = rpool.tile([128, 512], F32, tag="t1")
                        nc.vector.tensor_tensor(t1, pks, cs, ALU.mult)
                        t2 = rpool.tile([128, 512], F32, tag="t2")
                        nc.gpsimd.tensor_tensor(t2, qrot, sn, ALU.mult)
                        nc.vector.tensor_tensor(
                            dst[:, hh, sc * 512:(sc + 1) * 512],
                            t1, t2, ALU.add)

        # ---------------- Phase B: attention, scores kept as S^T [k, q] ------
        # Scores go into [128, 2, 512] PSUM tiles (2 k-blocks per group),
        # exp'd in one ACTIVATE per off-diagonal group, accumulated into a
        # running DVE sum, folded to [128, 512], and reduced+broadcast by a
        # single all-ones stationary matmul (den lands replicated on every
        # PSUM partition). av matmuls trail their exp by two groups; the
        # finalize (recip + av*recip + DMA out) is deferred by one unit.
        def make_attn_emitter(phb):
            qp_es = phb.enter_context(tc.tile_pool(name="es", bufs=4))
            esump = phb.enter_context(tc.tile_pool(name="esum", bufs=2))
            smallp = phb.enter_context(tc.tile_pool(name="small", bufs=3))
            ps = phb.enter_context(
                tc.tile_pool(name="ps", bufs=2, space="PSUM"))
            pav = phb.enter_context(
                tc.tile_pool(name="pav", bufs=2, space="PSUM"))
            pmisc = phb.enter_context(
                tc.tile_pool(name="pmisc", bufs=2, space="PSUM"))
            mp = None
            if mask_mode == "full":
                mp = phb.enter_context(tc.tile_pool(name="msk", bufs=3))

            state = {"pending": None, "pend_den": None}

            def emit_den():
                # den matmul for the previous unit, deferred so the next
                # unit's score matmuls cover the exp->tree->fold latency
                if state["pend_den"] is None:
                    return
                fold, den = state["pend_den"]
                nc.tensor.matmul(den, ones_sq, fold, start=True, stop=True)
                state["pend_den"] = None

            def finalize(av, den, hh, qc):
                recip = smallp.tile([128, 512], F32, tag="recip")
                nc.vector.reciprocal_approx_fast(recip, den)
                at_t = smallp.tile([128, 512], BF16, tag="at_t")
                nc.vector.tensor_tensor(at_t, av, recip, ALU.mult)
                nc.sync.dma_start(
                    out=at_s.ap()[hh * 128:(hh + 1) * 128,
                                  qc * 512:(qc + 1) * 512],
                    in_=at_t)

            def emit_unit(qc, hh):
                nblk = 4 * qc + 4 if mask_mode == "causal" else 16
                ng = nblk // 2
                av = pav.tile([128, 512], F32, tag="av")
                acc = None
                navs = [0]
                pend_av = []          # (es_tile, j2, kb, q_lo) awaiting av mm

                def emit_avs(upto):
                    while pend_av and len(pend_av) > upto:
                        es, j2, kb, q_lo = pend_av.pop(0)
                        qs = slice(q_lo, 512)
                        khalf, kloc = kb // 8, kb % 8
                        nc.tensor.matmul(
                            av[:, qs],
                            v_sb[khalf][:, kloc, hh * 128:(hh + 1) * 128],
                            es[:, j2, qs],
                            start=(navs[0] == 0), stop=(navs[0] == nblk - 1))
                        navs[0] += 1

                # diagonal groups first: their serial per-region exps overlap
                # the off-diagonal work that follows. The vd=0 block is the
                # first av emitted and covers the full [0:512] PSUM range, so
                # the accumulation start flag is sound.
                if mask_mode == "causal":
                    order = [2 * qc, 2 * qc + 1] + list(range(2 * qc))
                else:
                    order = list(range(ng))
                for gi, g in enumerate(order):
                    es = qp_es.tile([128, 2, 512], BF16, tag="es")
                    sps = ps.tile([128, 2, 512], F32, tag="sps")
                    diag = False
                    blk = []
                    for j2 in range(2):
                        kb = 2 * g + j2
                        vd = kb - 4 * qc   # diagonal block index
                        q_lo = (128 * vd
                                if (mask_mode == "causal" and vd > 0)
                                else 0)
                        qs = slice(q_lo, 512)
                        khalf, kloc = kb // 8, kb % 8
                        nc.tensor.matmul(
                            sps[:, j2, qs],
                            kT_sb[khalf][:, hh,
                                         kloc * 128:(kloc + 1) * 128],
                            qT_sb[qc // 2][:, hh,
                                           (qc % 2) * 512 + q_lo:
                                           (qc % 2) * 512 + 512],
                            start=True, stop=True)
                        if mask_mode == "causal" and vd >= 0:
                            diag = True
                            mq = slice(128 * vd, 128 * vd + 128)
                            nc.vector.tensor_tensor(
                                sps[:, j2, mq], sps[:, j2, mq], tri_sb,
                                ALU.add)
                        elif mask_mode == "full":
                            mt = mp.tile([128, 512], F32, tag="mt")
                            nc.sync.dma_start(
                                out=mt,
                                in_=maskT.ap()[kb * 128:(kb + 1) * 128,
                                               qc * 512:(qc + 1) * 512])
                            nc.vector.tensor_tensor(sps[:, j2, :],
                                                    sps[:, j2, :], mt,
                                                    ALU.add)
                        blk.append((kb, q_lo))
                    # exp: one ACTIVATE per clean group; per-region on the
                    # diagonal (unwritten PSUM slivers stay out of the AP)
                    if not diag:
                        nc.scalar.activation(es, sps, ACTF.Exp,
                                             scale=inv_sqrt_d)
                    else:
                        for j2, (kb, q_lo) in enumerate(blk):
                            if q_lo > 0:
                                nc.gpsimd.memset(es[:, j2, 0:q_lo], 0.0)
                            qs = slice(q_lo, 512)
                            nc.scalar.activation(es[:, j2, qs],
                                                 sps[:, j2, qs],
                                                 ACTF.Exp, scale=inv_sqrt_d)
                    # incremental tree: acc += es (DVE), frees es early
                    if gi == 0:
                        acc = es
                    else:
                        if gi == 1:
                            t = esump.tile([128, 2, 512], BF16, tag="e2")
                            nc.vector.tensor_tensor(t, acc, es, ALU.add)
                            acc = t
                        else:
                            nc.vector.tensor_tensor(acc, acc, es, ALU.add)
                    for j2, (kb, q_lo) in enumerate(blk):
                        pend_av.append((es, j2, kb, q_lo))
                    # av matmuls trail their exp by two groups
                    emit_avs(4)
                    if gi == 0:
                        emit_den()
                    if state["pending"] is not None and gi == min(1, ng - 1):
                        finalize(*state["pending"])
                        state["pending"] = None
                emit_avs(0)
                # fold the two k-block columns -> [128, 512]
                fold = smallp.tile([128, 512], BF16, tag="fold")
                nc.vector.tensor_tensor(fold, acc[:, 0, :], acc[:, 1, :],
                                        ALU.add)
                # den replicated across all 128 partitions via all-ones lhsT;
                # the matmul itself is deferred into the next unit
                den = pmisc.tile([128, 512], F32, tag="misc", name="den")
                state["pend_den"] = (fold, den)
                state["pending"] = (av, den, hh, qc)

            def flush():
                emit_den()
                if state["pending"] is not None:
                    finalize(*state["pending"])
                    state["pending"] = None

            return emit_unit, flush, pmisc

        # ---------------- Phase C: output projection -------------------------
        def make_c_emitter(phc, pop):
            atp = phc.enter_context(tc.tile_pool(name="atl", bufs=2))
            otp = phc.enter_context(tc.tile_pool(name="ot", bufs=4))
            wop = phc.enter_context(tc.tile_pool(name="wo", bufs=2))
            state = {"at_g": {}, "ot": {}, "wo_sl": None}

            def load_stg(stg):
                at_g = atp.tile([128, 8, 512], BF16, tag="at_g",
                                name=f"at_g{stg}")
                nc.sync.dma_start(
                    out=at_g,
                    in_=at_s.ap()[:, stg * 512:(stg + 1) * 512].rearrange(
                        "(hc p) s -> p hc s", p=128))
                state["at_g"][stg] = at_g

            def emit_unit(stg, st_l, o8):
                # one PSUM bank: out[st, o8] = sum_hc at^T wo
                # o8-major within a stage: wo slice loads once per (stg, o8),
                # the four ot tiles of the stage stay live until its end.
                at_g = state["at_g"][stg]
                st = stg * 4 + st_l
                sl = slice(st_l * 128, (st_l + 1) * 128)
                if st_l == 0:
                    wo_sl = wop.tile([128, 8, 512], BF16, tag="wo_sl",
                                     name=f"wo{stg}_{o8}")
                    nc.sync.dma_start(
                        out=wo_sl,
                        in_=wo_t.ap()[:, :, o8 * 512:(o8 + 1) * 512])
                    state["wo_sl"] = wo_sl
                wo_sl = state["wo_sl"]
                if o8 == 0:
                    state["ot"][st] = otp.tile([128, 8, 512], BF16, tag="ot",
                                               name=f"ot{st}")
                ot = state["ot"][st]
                po = pop.tile([128, 512], F32, tag="misc", name="po")
                for hc in range(8):
                    nc.tensor.matmul(
                        po,
                        at_g[:, hc, sl],
                        wo_sl[:, hc, :],
                        start=(hc == 0), stop=(hc == 7))
                nc.scalar.activation(ot[:, o8, :], po, ACTF.Copy)
                if o8 == 7:
                    nc.sync.dma_start(
                        out=out_p.ap()[st * 128:(st + 1) * 128, :],
                        in_=ot.rearrange("p a b -> p (a b)"))
                    del state["ot"][st]

            return emit_unit, load_stg

        # ================= schedule =================
        with ExitStack() as ph0:
            emit_half(0, ph0)

        if mask_mode == "causal":
            # B01: heads x (qc0, qc1); x0 freed, at_s rows for stg 0,1 written
            with ExitStack() as phb:
                emit_unit_b, flush_b, _ = make_attn_emitter(phb)
                for qc in (0, 1):
                    for hh in range(HEADS_PER_CORE):
                        emit_unit_b(qc, hh)
                flush_b()

            with ExitStack() as ph1:
                emit_half(1, ph1)

            # B23 with C(stg0, stg1) interleaved
            with ExitStack() as phbc:
                emit_unit_b, flush_b, pmisc = make_attn_emitter(phbc)
                emit_unit_c, load_stg = make_c_emitter(phbc, pmisc)
                load_stg(0)
                load_stg(1)
                c_units = [(stg, st_l, o8)
                           for stg in (0, 1)
                           for o8 in range(8)
                           for st_l in range(4)]
                for qc in (2, 3):
                    for hh in range(HEADS_PER_CORE):
                        emit_unit_b(qc, hh)
                        if qc == 3 and hh == 0:
                            # at_s rows for stg2 are complete once the
                            # (qc2, hh7) finalize ran inside this unit
                            load_stg(2)
                        # ~4 C units per B unit balances the two streams
                        for _ in range(4):
                            if c_units:
                                emit_unit_c(*c_units.pop(0))
                flush_b()
                load_stg(3)
                while c_units:
                    emit_unit_c(*c_units.pop(0))

                # C(stg2, stg3) dense
                for stg in (2, 3):
                    for o8 in range(8):
                        for st_l in range(4):
                            emit_unit_c(stg, st_l, o8)
        else:
            with ExitStack() as ph1:
                emit_half(1, ph1)
            with ExitStack() as phbc:
                emit_unit_b, flush_b, pmisc = make_attn_emitter(phbc)
                emit_unit_c, load_stg = make_c_emitter(phbc, pmisc)
                for qc in range(4):
                    for hh in range(HEADS_PER_CORE):
                        emit_unit_b(qc, hh)
                # all units emitted; flush and drain C for every stg in order
                flush_b()
                for stg in range(4):
                    load_stg(stg)
                    for o8 in range(8):
                        for st_l in range(4):
                            emit_unit_c(stg, st_l, o8)

    nc.compile()
    return nc


_PROGRAM_CACHE = {}


def _get_program(mask_mode):
    if mask_mode not in _PROGRAM_CACHE:
        _PROGRAM_CACHE[mask_mode] = _build_program(mask_mode)
    return _PROGRAM_CACHE[mask_mode]


def _classify_mask(attention_mask):
    m = np.asarray(attention_mask)
    if not np.any(m):
        return "none"
    neg = np.float32(np.finfo(np.float32).min)
    causal = np.triu(np.full((S, S), neg, dtype=np.float32), k=1)
    for b in range(m.shape[0]):
        if not np.array_equal(m[b, 0], causal):
            return "full"
    return "causal"


def _prep_core_inputs(hidden_states, attention_mask, position_ids, W_pack, W_o,
                      mask_mode):
    from ml_dtypes import bfloat16

    hidden_states = np.asarray(hidden_states, dtype=np.float32)
    W_pack = np.asarray(W_pack, dtype=np.float32)
    W_o = np.asarray(W_o, dtype=np.float32)
    pos = np.asarray(position_ids).astype(np.int64)

    cos_t, sin_t = _rope_tables(int(pos.max()) + 1)
    # per-batch gathered + transposed + row-permuted (+ sign folded into sin)
    cosT_b, sinT_b = [], []
    for b in range(B):
        c = cos_t[pos[b]][:, PERM].T
        s = (sin_t[pos[b]][:, PERM] * SIGN[None, :]).T
        cosT_b.append(np.ascontiguousarray(c.astype(bfloat16)))
        sinT_b.append(np.ascontiguousarray(s.astype(bfloat16)))

    # x_t[p, c, s] = hidden[b, s, c*128+p]
    x_b = [np.ascontiguousarray(
        hidden_states[b].T.reshape(32, 128, S).transpose(1, 0, 2)
        .astype(bfloat16)) for b in range(B)]

    tri_m = None
    maskT_b = None
    if mask_mode == "causal":
        kk = np.arange(128)[:, None]
        qq = np.arange(128)[None, :]
        tri_m = np.where(kk <= qq, 0.0, NEG_BIG).astype(np.float32)
    elif mask_mode == "full":
        m = np.asarray(attention_mask, dtype=np.float32)
        maskT_b = [np.ascontiguousarray(m[b, 0].T) for b in range(B)]

    in_maps = []
    for cidx in range(8):
        b, g = cidx // 4, cidx % 4
        # per-head d-permuted q/k weight rows, head-major columns in wqk
        qrows = np.concatenate(
            [g * HG + hh * 128 + PERM for hh in range(HEADS_PER_CORE)])
        krows = HIDDEN + qrows
        vrows = 2 * HIDDEN + g * HG + np.arange(HG)
        wqk = np.concatenate([W_pack[qrows], W_pack[krows]], axis=0)  # [2048,4096]
        # wqk_t[oc, p, c, o] = wqk[oc*128+o, c*128+p]
        wqk_t = np.ascontiguousarray(
            wqk.reshape(16, 128, 32, 128).transpose(0, 3, 2, 1)
            .astype(bfloat16))
        wv = W_pack[vrows]                                            # [1024,4096]
        # wv_t[ov, c, p, o] = wv[ov*512+o, c*128+p]
        wv_t = np.ascontiguousarray(
            wv.reshape(2, 512, 32, 128).transpose(0, 2, 3, 1).astype(bfloat16))
        # wo_t[p, hc, o] = W_o[o, g*HG + hc*128 + p]
        wo_t = np.ascontiguousarray(
            W_o[:, g * HG:(g + 1) * HG].reshape(HIDDEN, 8, 128)
            .transpose(2, 1, 0).astype(bfloat16))
        im = {"x_t": x_b[b], "wqk_t": wqk_t, "wv_t": wv_t, "wo_t": wo_t,
              "cosT": cosT_b[b], "sinT": sinT_b[b]}
        if mask_mode == "causal":
            im["tri"] = tri_m
        elif mask_mode == "full":
            im["maskT"] = maskT_b[b]
        in_maps.append(im)
    return in_maps


def _run(hidden_states, attention_mask, position_ids, W_pack, W_o,
         trace=False, trace_kwargs=None):
    from concourse.bass_utils import run_bass_kernel_spmd

    mask_mode = _classify_mask(attention_mask)
    nc = _get_program(mask_mode)
    in_maps = _prep_core_inputs(hidden_states, attention_mask, position_ids,
                                W_pack, W_o, mask_mode)
    try:
        res = run_bass_kernel_spmd(nc, in_maps, list(range(8)), trace=trace,
                                   **(trace_kwargs or {}))
    except Exception:
        # transient NRT_EXEC_UNIT_UNRECOVERABLE wedges recover on retry
        import time as _time
        _time.sleep(15)
        res = run_bass_kernel_spmd(nc, in_maps, list(range(8)), trace=trace,
                                   **(trace_kwargs or {}))
    out = np.zeros((B, S, HIDDEN), dtype=np.float32)
    for c in range(8):
        out[c // 4] += np.asarray(res.results[c]["out_p"], dtype=np.float32)
    return out, res


def kernel(hidden_states, attention_mask, position_ids, W_pack, W_o):
    out, _ = _run(hidden_states, attention_mask, position_ids, W_pack, W_o)
    return out


# revision 24
# speedup vs baseline: 1.0611x; 1.0152x over previous
"""Trainium2 Bass kernel for a single attention layer (Baichuan-style W_pack
attention with rotary embeddings), sharded over 8 NeuronCores:
tensor-parallel over 4 head groups x data-parallel over 2 batches.

v3: scheduling overhaul toward the bf16 PE stream floor (~1.01 ms):
 - V-proj runs before QK-proj in each half so the x DMA stream is consumed
   at arrival rate (kills the ~20 us x-wait stall at each half start).
 - softmax denominator: single all-ones [128,128] stationary matmul whose
   PSUM output is den replicated across all partitions (no [1,512] recip
   chain, no separate broadcast matmul), fed by an incremental DVE tree +
   fold so there is one den matmul per (head, q-chunk).
 - exp batched per 2 score blocks (one ACTIVATE over [128,1024]) off the
   diagonal; av matmuls trail their exp by two groups so the PE never
   waits on the ACT exp.
 - O-proj work for seq groups 0,1 is interleaved into the qc2/qc3
   attention emission to fill residual exp-chain bubbles.

Contract: kernel(**inputs) takes the FULL unsharded inputs and returns the
FULL output [2, 2048, 4096] float32. All sharding / gathering happens here.
"""

import math
import sys

import numpy as np

for _p in ("/opt/trn_rl_repo", "/root/.axon_site/_ro/trn_rl_repo"):
    if _p not in sys.path:
        sys.path.insert(0, _p)

HIDDEN = 4096
N_HEADS = 32
HEAD_DIM = 128
BASE = 10000.0
B = 2
S = 2048
HEADS_PER_CORE = 8          # 32 heads / 4 groups
HG = 1024                   # head-group width = 8 heads * 128
NEG_BIG = -1.0e9

# RoPE partner permutation: quadrant q holds [lo_d 16q..16q+15, hi_d 64+16q..]
# so the rotate-half partner of new-row i is i+-16 inside its 32-row quadrant,
# reachable by DVE stream_shuffle.
PERM = np.zeros(128, dtype=np.int64)
for _q in range(4):
    PERM[32 * _q: 32 * _q + 16] = np.arange(16 * _q, 16 * _q + 16)
    PERM[32 * _q + 16: 32 * _q + 32] = 64 + np.arange(16 * _q, 16 * _q + 16)
SHUF_MASK = [(i + 16) % 32 for i in range(32)]
# sign of the sin term per (new) row: -1 where original d < 64
SIGN = np.where(PERM < 64, -1.0, 1.0).astype(np.float32)


def _rope_tables(max_pos):
    inv_freq = 1.0 / (BASE ** (np.arange(0, HEAD_DIM, 2, dtype=np.float32) / HEAD_DIM))
    t = np.arange(max_pos, dtype=np.float32)
    freqs = np.outer(t, inv_freq)                      # [P, 64]
    emb = np.concatenate((freqs, freqs), axis=-1)      # [P, 128]
    return np.cos(emb).astype(np.float32), np.sin(emb).astype(np.float32)


def _build_program(mask_mode):
    """mask_mode: 'causal' (block-skip + shared triangle mask),
    'none' (dense, no mask), 'full' (dense, stream mask tiles)."""
    import concourse.bacc as bacc
    import concourse.mybir as mybir
    import concourse.tile as tile
    from contextlib import ExitStack

    F32 = mybir.dt.float32
    BF16 = mybir.dt.bfloat16
    ALU = mybir.AluOpType
    ACTF = mybir.ActivationFunctionType

    nc = bacc.Bacc("TRN2", target_bir_lowering=False, debug=False)

    # pre-tiled inputs (host side does all layout work)
    x_t = nc.declare_dram_parameter("x_t", [128, 32, S], BF16, isOutput=False)
    wqk_t = nc.declare_dram_parameter("wqk_t", [16, 128, 32, 128], BF16,
                                      isOutput=False)
    wv_t = nc.declare_dram_parameter("wv_t", [2, 32, 128, 512], BF16,
                                     isOutput=False)
    wo_t = nc.declare_dram_parameter("wo_t", [128, 8, HIDDEN], BF16,
                                     isOutput=False)
    cosT = nc.declare_dram_parameter("cosT", [128, S], BF16, isOutput=False)
    sinT = nc.declare_dram_parameter("sinT", [128, S], BF16, isOutput=False)
    if mask_mode == "causal":
        tri = nc.declare_dram_parameter("tri", [128, 128], F32, isOutput=False)
    elif mask_mode == "full":
        maskT = nc.declare_dram_parameter("maskT", [S, S], F32, isOutput=False)
    out_p = nc.declare_dram_parameter("out_p", [S, HIDDEN], BF16, isOutput=True)

    at_s = nc.dram_tensor("at_scratch", [HG, S], BF16)

    inv_sqrt_d = 1.0 / math.sqrt(HEAD_DIM)

    with tile.TileContext(nc, pool_alloc_mode="queue") as tc, ExitStack() as top:
        const_pool = top.enter_context(tc.tile_pool(name="consts", bufs=1))
        ones_f32 = const_pool.tile([128, 128], F32)
        nc.vector.memset(ones_f32, 1.0)
        ones_sq = const_pool.tile([128, 128], BF16)
        nc.vector.tensor_copy(ones_sq, ones_f32)
        if mask_mode == "causal":
            tri_sb = const_pool.tile([128, 128], F32)
            nc.sync.dma_start(out=tri_sb, in_=tri.ap())

        # SBUF-resident q/k/v; q/k split per head so attention units only
        # wait on the head they read (dep tracking is per-tile)
        res_pool = top.enter_context(tc.tile_pool(name="resident", bufs=1))
        qT_sb = [[res_pool.tile([128, 1024], BF16, name=f"qT{h}_{hh}")
                  for hh in range(HEADS_PER_CORE)] for h in range(2)]
        kT_sb = [[res_pool.tile([128, 1024], BF16, name=f"kT{h}_{hh}")
                  for hh in range(HEADS_PER_CORE)] for h in range(2)]
        v_sb = [res_pool.tile([128, 8, HG], BF16, name=f"v{h}")
                for h in range(2)]

        # ---------------- Phase A: V then QK projection per half -------------
        def emit_half(hs, pha):
            s0 = hs * 1024
            # h-chunk sweep order: B-block (16..31) first (its DMA is issued
            # first), A-block (0..15) last.
            H_ORDER = list(range(16, 32)) + list(range(16))
            xpoolA = pha.enter_context(tc.tile_pool(name="xhalfA", bufs=1))
            xpoolB = pha.enter_context(tc.tile_pool(name="xhalfB", bufs=1))
            xtA = xpoolA.tile([128, 16, 1024], BF16, name=f"xtA{hs}")
            xtB = xpoolB.tile([128, 16, 1024], BF16, name=f"xtB{hs}")
            xin = x_t.ap()[:, :, s0:s0 + 1024]

            def emit_x_dma(c):
                dst = xtB[:, c - 16, :] if c >= 16 else xtA[:, c, :]
                nc.sync.dma_start(out=dst, in_=xin[:, c, :])

            def xt_slice(c, sl):
                return xtB[:, c - 16, sl] if c >= 16 else xtA[:, c, sl]

            # --- v projection first, x-stationary, out = v [s, o] ------------
            # x-chunk DMAs are interleaved with the wv tile loads in
            # consumption order so the first pass streams at DMA rate
            with ExitStack() as vv:
                wvp = vv.enter_context(tc.tile_pool(name="wvt", bufs=4))
                pv = vv.enter_context(
                    tc.tile_pool(name="pv", bufs=8, space="PSUM"))
                for ov in range(2):          # v-dim chunks of 512
                    vb = [pv.tile([128, 512], F32, tag="vb", name=f"vb{i}")
                          for i in range(8)]
                    for hi, h in enumerate(H_ORDER):
                        wv_tile = wvp.tile([128, 512], BF16, tag="wv_tile")
                        nc.sync.dma_start(out=wv_tile, in_=wv_t.ap()[ov, h])
                        if ov == 0:
                            emit_x_dma(h)
                        for sc in range(8):
                            nc.tensor.matmul(
                                vb[sc],
                                xt_slice(h, slice(sc * 128, (sc + 1) * 128)),
                                wv_tile,
                                start=(hi == 0), stop=(hi == 31))
                    for sc in range(8):
                        dst = v_sb[hs][:, sc, ov * 512:(ov + 1) * 512]
                        if sc % 2 == 0:
                            nc.scalar.activation(dst, vb[sc], ACTF.Copy)
                        else:
                            nc.vector.tensor_copy(dst, vb[sc])

            # --- q,k projection, weight-stationary, out = projT [o, s] -------
            with ExitStack() as qk:
                cspool = qk.enter_context(tc.tile_pool(name="cossin", bufs=1))
                cos_sb = cspool.tile([128, 1024], BF16)
                nc.sync.dma_start(out=cos_sb, in_=cosT.ap()[:, s0:s0 + 1024])
                sin_sb = cspool.tile([128, 1024], BF16)
                nc.sync.dma_start(out=sin_sb, in_=sinT.ap()[:, s0:s0 + 1024])

                wpool = qk.enter_context(tc.tile_pool(name="wqk", bufs=2))
                pqk = qk.enter_context(
                    tc.tile_pool(name="pqk", bufs=2, space="PSUM"))
                rpool = qk.enter_context(tc.tile_pool(name="rope", bufs=2))
                for oc in range(16):         # o chunks of 128 (head tiles)
                    w_oc = wpool.tile([128, 32, 128], BF16, tag="w_oc")
                    nc.sync.dma_start(out=w_oc, in_=wqk_t.ap()[oc])
                    pk = pqk.tile([128, 2, 512], F32, tag="pk")
                    for hi, h in enumerate(H_ORDER):
                        for sc in range(2):
                            nc.tensor.matmul(
                                pk[:, sc, :], w_oc[:, h, :],
                                xt_slice(h, slice(sc * 512, (sc + 1) * 512)),
                                start=(hi == 0), stop=(hi == 31))
                    # RoPE: q' = q*cos + shuffle16(q)*sin_signed, -> bf16
                    dst = (qT_sb[hs] if oc < 8 else kT_sb[hs])[oc % 8]
                    for sc in range(2):
                        pks = pk[:, sc, :]
                        cs = cos_sb[:, sc * 512:(sc + 1) * 512]
                        sn = sin_sb[:, sc * 512:(sc + 1) * 512]
                        qrot = rpool.tile([128, 512], F32, tag="qrot")
                        nc.vector.stream_shuffle(qrot, pks, SHUF_MASK)
                        t1 = rpool.tile([128, 512], F32, tag="t1")
                        nc.vector.tensor_tensor(t1, pks, cs, ALU.mult)
                        t2 = rpool.tile([128, 512], F32, tag="t2")
                        nc.gpsimd.tensor_tensor(t2, qrot, sn, ALU.mult)
                        nc.vector.tensor_tensor(
                            dst[:, sc * 512:(sc + 1) * 512],
                            t1, t2, ALU.add)

        # ---------------- Phase B: attention, scores kept as S^T [k, q] ------
        # Scores go into [128, 2, 512] PSUM tiles (2 k-blocks per group),
        # exp'd in one ACTIVATE per off-diagonal group, accumulated into a
        # running DVE sum, folded to [128, 512], and reduced+broadcast by a
        # single all-ones stationary matmul (den lands replicated on every
        # PSUM partition). av matmuls trail their exp by two groups; the
        # finalize (recip + av*recip + DMA out) is deferred by one unit.
        def make_attn_emitter(phb):
            qp_es = phb.enter_context(tc.tile_pool(name="es", bufs=4))
            # diagonal-group es tiles cycle a dedicated 4-slot pool; the
            # slot<->sliver-pattern mapping is deterministic (2 allocs/unit),
            # so the zero slivers only need writing on the first two units
            dpool = phb.enter_context(tc.tile_pool(name="esd", bufs=4))
            esump = phb.enter_context(tc.tile_pool(name="esum", bufs=2))
            smallp = phb.enter_context(tc.tile_pool(name="small", bufs=3))
            ps = phb.enter_context(
                tc.tile_pool(name="ps", bufs=2, space="PSUM"))
            pav = phb.enter_context(
                tc.tile_pool(name="pav", bufs=2, space="PSUM"))
            pmisc = phb.enter_context(
                tc.tile_pool(name="pmisc", bufs=2, space="PSUM"))
            mp = None
            if mask_mode == "full":
                mp = phb.enter_context(tc.tile_pool(name="msk", bufs=3))

            state = {"pending": None, "pend_den": None, "dinit": 0}

            def emit_den():
                # den matmul for the previous unit, deferred so the next
                # unit's score matmuls cover the exp->tree->fold latency
                if state["pend_den"] is None:
                    return
                fold, den = state["pend_den"]
                nc.tensor.matmul(den, ones_sq, fold, start=True, stop=True)
                state["pend_den"] = None

            def finalize(av, den, hh, qc):
                recip = smallp.tile([128, 512], F32, tag="recip")
                nc.vector.reciprocal_approx_fast(recip, den)
                at_t = smallp.tile([128, 512], BF16, tag="at_t")
                nc.vector.tensor_tensor(at_t, av, recip, ALU.mult)
                nc.sync.dma_start(
                    out=at_s.ap()[hh * 128:(hh + 1) * 128,
                                  qc * 512:(qc + 1) * 512],
                    in_=at_t)

            def emit_unit(qc, hh):
                nblk = 4 * qc + 4 if mask_mode == "causal" else 16
                ng = nblk // 2
                av = pav.tile([128, 512], F32, tag="av")
                acc = None
                navs = [0]
                pend_av = []          # (es_tile, j2, kb, q_lo) awaiting av mm

                def emit_avs(upto):
                    while pend_av and len(pend_av) > upto:
                        es, j2, kb, q_lo = pend_av.pop(0)
                        qs = slice(q_lo, 512)
                        khalf, kloc = kb // 8, kb % 8
                        nc.tensor.matmul(
                            av[:, qs],
                            v_sb[khalf][:, kloc, hh * 128:(hh + 1) * 128],
                            es[:, j2, qs],
                            start=(navs[0] == 0), stop=(navs[0] == nblk - 1))
                        navs[0] += 1

                # diagonal groups first: their serial per-region exps overlap
                # the off-diagonal work that follows. The vd=0 block is the
                # first av emitted and covers the full [0:512] PSUM range, so
                # the accumulation start flag is sound.
                if mask_mode == "causal":
                    order = [2 * qc, 2 * qc + 1] + list(range(2 * qc))
                else:
                    order = list(range(ng))
                for gi, g in enumerate(order):
                    es = qp_es.tile([128, 2, 512], BF16, tag="es", name="es")
                    sps = ps.tile([128, 2, 512], F32, tag="sps")
                    diag = False
                    blk = []
                    for j2 in range(2):
                        kb = 2 * g + j2
                        vd = kb - 4 * qc   # diagonal block index
                        q_lo = (128 * vd
                                if (mask_mode == "causal" and vd > 0)
                                else 0)
                        qs = slice(q_lo, 512)
                        khalf, kloc = kb // 8, kb % 8
                        nc.tensor.matmul(
                            sps[:, j2, qs],
                            kT_sb[khalf][hh][:, kloc * 128:(kloc + 1) * 128],
                            qT_sb[qc // 2][hh][:, (qc % 2) * 512 + q_lo:
                                               (qc % 2) * 512 + 512],
                            start=True, stop=True)
                        if mask_mode == "causal" and vd >= 0:
                            diag = True
                            mq = slice(128 * vd, 128 * vd + 128)
                            nc.vector.tensor_tensor(
                                sps[:, j2, mq], sps[:, j2, mq], tri_sb,
                                ALU.add)
                        elif mask_mode == "full":
                            mt = mp.tile([128, 512], F32, tag="mt")
                            nc.sync.dma_start(
                                out=mt,
                                in_=maskT.ap()[kb * 128:(kb + 1) * 128,
                                               qc * 512:(qc + 1) * 512])
                            nc.vector.tensor_tensor(sps[:, j2, :],
                                                    sps[:, j2, :], mt,
                                                    ALU.add)
                        blk.append((kb, q_lo))
                    # exp: one ACTIVATE per clean group; per-region on the
                    # diagonal (unwritten PSUM slivers stay out of the AP)
                    if not diag:
                        nc.scalar.activation(es, sps, ACTF.Exp,
                                             scale=inv_sqrt_d)
                    else:
                        for j2, (kb, q_lo) in enumerate(blk):
                            if q_lo > 0:
                                nc.vector.memset(es[:, j2, 0:q_lo], 0.0)
                            qs = slice(q_lo, 512)
                            nc.scalar.activation(es[:, j2, qs],
                                                 sps[:, j2, qs],
                                                 ACTF.Exp, scale=inv_sqrt_d)
                    # incremental tree: acc += es (DVE), frees es early
                    if gi == 0:
                        acc = es
                    else:
                        if gi == 1:
                            t = esump.tile([128, 2, 512], BF16, tag="e2")
                            nc.vector.tensor_tensor(t, acc, es, ALU.add)
                            acc = t
                        else:
                            nc.vector.tensor_tensor(acc, acc, es, ALU.add)
                    for j2, (kb, q_lo) in enumerate(blk):
                        pend_av.append((es, j2, kb, q_lo))
                    # av matmuls trail their exp by two groups
                    emit_avs(4)
                    if gi == 0:
                        emit_den()
                    if state["pending"] is not None and gi == min(1, ng - 1):
                        finalize(*state["pending"])
                        state["pending"] = None
                emit_avs(0)
                # fold the two k-block columns -> [128, 512]
                fold = smallp.tile([128, 512], BF16, tag="fold")
                nc.vector.tensor_tensor(fold, acc[:, 0, :], acc[:, 1, :],
                                        ALU.add)
                # den replicated across all 128 partitions via all-ones lhsT;
                # the matmul itself is deferred into the next unit
                den = pmisc.tile([128, 512], F32, tag="misc", name="den")
                state["pend_den"] = (fold, den)
                state["pending"] = (av, den, hh, qc)

            def flush():
                emit_den()
                if state["pending"] is not None:
                    finalize(*state["pending"])
                    state["pending"] = None

            return emit_unit, flush, pmisc

        # ---------------- Phase C: output projection -------------------------
        def make_c_emitter(phc, pop):
            atp = phc.enter_context(tc.tile_pool(name="atl", bufs=2))
            otp = phc.enter_context(tc.tile_pool(name="ot", bufs=4))
            wop = phc.enter_context(tc.tile_pool(name="wo", bufs=2))
            state = {"at_g": {}, "ot": {}, "wo_sl": None}

            def load_stg(stg):
                at_g = atp.tile([128, 8, 512], BF16, tag="at_g",
                                name=f"at_g{stg}")
                nc.sync.dma_start(
                    out=at_g,
                    in_=at_s.ap()[:, stg * 512:(stg + 1) * 512].rearrange(
                        "(hc p) s -> p hc s", p=128))
                state["at_g"][stg] = at_g

            def emit_unit(stg, st_l, o8):
                # one PSUM bank: out[st, o8] = sum_hc at^T wo
                # o8-major within a stage: wo slice loads once per (stg, o8),
                # the four ot tiles of the stage stay live until its end.
                at_g = state["at_g"][stg]
                st = stg * 4 + st_l
                sl = slice(st_l * 128, (st_l + 1) * 128)
                if st_l == 0:
                    wo_sl = wop.tile([128, 8, 512], BF16, tag="wo_sl",
                                     name=f"wo{stg}_{o8}")
                    nc.sync.dma_start(
                        out=wo_sl,
                        in_=wo_t.ap()[:, :, o8 * 512:(o8 + 1) * 512])
                    state["wo_sl"] = wo_sl
                wo_sl = state["wo_sl"]
                if o8 == 0:
                    state["ot"][st] = otp.tile([128, 8, 512], BF16, tag="ot",
                                               name=f"ot{st}")
                ot = state["ot"][st]
                po = pop.tile([128, 512], F32, tag="misc", name="po")
                for hc in range(8):
                    nc.tensor.matmul(
                        po,
                        at_g[:, hc, sl],
                        wo_sl[:, hc, :],
                        start=(hc == 0), stop=(hc == 7))
                nc.scalar.activation(ot[:, o8, :], po, ACTF.Copy)
                if o8 == 7:
                    nc.sync.dma_start(
                        out=out_p.ap()[st * 128:(st + 1) * 128, :],
                        in_=ot.rearrange("p a b -> p (a b)"))
                    del state["ot"][st]

            return emit_unit, load_stg

        # ================= schedule =================
        with ExitStack() as ph0:
            emit_half(0, ph0)

        if mask_mode == "causal":
            # B01: heads x (qc0, qc1); x0 freed, at_s rows for stg 0,1 written
            with ExitStack() as phb:
                emit_unit_b, flush_b, _ = make_attn_emitter(phb)
                for qc in (0, 1):
                    for hh in range(HEADS_PER_CORE):
                        emit_unit_b(qc, hh)
                flush_b()

            with ExitStack() as ph1:
                emit_half(1, ph1)

            # B23 with C(stg0, stg1) interleaved
            with ExitStack() as phbc:
                emit_unit_b, flush_b, pmisc = make_attn_emitter(phbc)
                emit_unit_c, load_stg = make_c_emitter(phbc, pmisc)
                load_stg(0)
                load_stg(1)
                c_units = [(stg, st_l, o8)
                           for stg in (0, 1)
                           for o8 in range(8)
                           for st_l in range(4)]
                for qc in (2, 3):
                    for hh in range(HEADS_PER_CORE):
                        emit_unit_b(qc, hh)
                        if qc == 3 and hh == 0:
                            # at_s rows for stg2 are complete once the
                            # (qc2, hh7) finalize ran inside this unit
                            load_stg(2)
                        # ~4 C units per B unit balances the two streams
                        for _ in range(4):
                            if c_units:
                                emit_unit_c(*c_units.pop(0))
                flush_b()
                load_stg(3)
                while c_units:
                    emit_unit_c(*c_units.pop(0))

                # C(stg2, stg3) dense
                for stg in (2, 3):
                    for o8 in range(8):
                        for st_l in range(4):
                            emit_unit_c(stg, st_l, o8)
        else:
            with ExitStack() as ph1:
                emit_half(1, ph1)
            with ExitStack() as phbc:
                emit_unit_b, flush_b, pmisc = make_attn_emitter(phbc)
                emit_unit_c, load_stg = make_c_emitter(phbc, pmisc)
                for qc in range(4):
                    for hh in range(HEADS_PER_CORE):
                        emit_unit_b(qc, hh)
                # all units emitted; flush and drain C for every stg in order
                flush_b()
                for stg in range(4):
                    load_stg(stg)
                    for o8 in range(8):
                        for st_l in range(4):
                            emit_unit_c(stg, st_l, o8)

    nc.compile()
    return nc


_PROGRAM_CACHE = {}


def _get_program(mask_mode):
    if mask_mode not in _PROGRAM_CACHE:
        _PROGRAM_CACHE[mask_mode] = _build_program(mask_mode)
    return _PROGRAM_CACHE[mask_mode]


def _classify_mask(attention_mask):
    m = np.asarray(attention_mask)
    if not np.any(m):
        return "none"
    neg = np.float32(np.finfo(np.float32).min)
    causal = np.triu(np.full((S, S), neg, dtype=np.float32), k=1)
    for b in range(m.shape[0]):
        if not np.array_equal(m[b, 0], causal):
            return "full"
    return "causal"


def _prep_core_inputs(hidden_states, attention_mask, position_ids, W_pack, W_o,
                      mask_mode):
    from ml_dtypes import bfloat16

    hidden_states = np.asarray(hidden_states, dtype=np.float32)
    W_pack = np.asarray(W_pack, dtype=np.float32)
    W_o = np.asarray(W_o, dtype=np.float32)
    pos = np.asarray(position_ids).astype(np.int64)

    cos_t, sin_t = _rope_tables(int(pos.max()) + 1)
    # per-batch gathered + transposed + row-permuted (+ sign folded into sin)
    cosT_b, sinT_b = [], []
    for b in range(B):
        c = cos_t[pos[b]][:, PERM].T
        s = (sin_t[pos[b]][:, PERM] * SIGN[None, :]).T
        cosT_b.append(np.ascontiguousarray(c.astype(bfloat16)))
        sinT_b.append(np.ascontiguousarray(s.astype(bfloat16)))

    # x_t[p, c, s] = hidden[b, s, c*128+p]
    x_b = [np.ascontiguousarray(
        hidden_states[b].T.reshape(32, 128, S).transpose(1, 0, 2)
        .astype(bfloat16)) for b in range(B)]

    tri_m = None
    maskT_b = None
    if mask_mode == "causal":
        kk = np.arange(128)[:, None]
        qq = np.arange(128)[None, :]
        tri_m = np.where(kk <= qq, 0.0, NEG_BIG).astype(np.float32)
    elif mask_mode == "full":
        m = np.asarray(attention_mask, dtype=np.float32)
        maskT_b = [np.ascontiguousarray(m[b, 0].T) for b in range(B)]

    in_maps = []
    for cidx in range(8):
        b, g = cidx // 4, cidx % 4
        # per-head d-permuted q/k weight rows, head-major columns in wqk
        qrows = np.concatenate(
            [g * HG + hh * 128 + PERM for hh in range(HEADS_PER_CORE)])
        krows = HIDDEN + qrows
        vrows = 2 * HIDDEN + g * HG + np.arange(HG)
        wqk = np.concatenate([W_pack[qrows], W_pack[krows]], axis=0)  # [2048,4096]
        # wqk_t[oc, p, c, o] = wqk[oc*128+o, c*128+p]
        wqk_t = np.ascontiguousarray(
            wqk.reshape(16, 128, 32, 128).transpose(0, 3, 2, 1)
            .astype(bfloat16))
        wv = W_pack[vrows]                                            # [1024,4096]
        # wv_t[ov, c, p, o] = wv[ov*512+o, c*128+p]
        wv_t = np.ascontiguousarray(
            wv.reshape(2, 512, 32, 128).transpose(0, 2, 3, 1).astype(bfloat16))
        # wo_t[p, hc, o] = W_o[o, g*HG + hc*128 + p]
        wo_t = np.ascontiguousarray(
            W_o[:, g * HG:(g + 1) * HG].reshape(HIDDEN, 8, 128)
            .transpose(2, 1, 0).astype(bfloat16))
        im = {"x_t": x_b[b], "wqk_t": wqk_t, "wv_t": wv_t, "wo_t": wo_t,
              "cosT": cosT_b[b], "sinT": sinT_b[b]}
        if mask_mode == "causal":
            im["tri"] = tri_m
        elif mask_mode == "full":
            im["maskT"] = maskT_b[b]
        in_maps.append(im)
    return in_maps


def _run(hidden_states, attention_mask, position_ids, W_pack, W_o,
         trace=False, trace_kwargs=None):
    from concourse.bass_utils import run_bass_kernel_spmd

    mask_mode = _classify_mask(attention_mask)
    nc = _get_program(mask_mode)
    in_maps = _prep_core_inputs(hidden_states, attention_mask, position_ids,
                                W_pack, W_o, mask_mode)
    try:
        res = run_bass_kernel_spmd(nc, in_maps, list(range(8)), trace=trace,
                                   **(trace_kwargs or {}))
    except Exception:
        # transient NRT_EXEC_UNIT_UNRECOVERABLE wedges recover on retry
        import time as _time
        _time.sleep(15)
        res = run_bass_kernel_spmd(nc, in_maps, list(range(8)), trace=trace,
                                   **(trace_kwargs or {}))
    out = np.zeros((B, S, HIDDEN), dtype=np.float32)
    for c in range(8):
        out[c // 4] += np.asarray(res.results[c]["out_p"], dtype=np.float32)
    return out, res


def kernel(hidden_states, attention_mask, position_ids, W_pack, W_o):
    out, _ = _run(hidden_states, attention_mask, position_ids, W_pack, W_o)
    return out


# revision 40
# speedup vs baseline: 1.0803x; 1.0181x over previous
"""Trainium2 Bass kernel for a single attention layer (Baichuan-style W_pack
attention with rotary embeddings), sharded over 8 NeuronCores:
tensor-parallel over 4 head groups x data-parallel over 2 batches.

v3: scheduling overhaul toward the bf16 PE stream floor (~1.01 ms):
 - V-proj runs before QK-proj in each half so the x DMA stream is consumed
   at arrival rate (kills the ~20 us x-wait stall at each half start).
 - softmax denominator: single all-ones [128,128] stationary matmul whose
   PSUM output is den replicated across all partitions (no [1,512] recip
   chain, no separate broadcast matmul), fed by an incremental DVE tree +
   fold so there is one den matmul per (head, q-chunk).
 - exp batched per 2 score blocks (one ACTIVATE over [128,1024]) off the
   diagonal; av matmuls trail their exp by two groups so the PE never
   waits on the ACT exp.
 - O-proj work for seq groups 0,1 is interleaved into the qc2/qc3
   attention emission to fill residual exp-chain bubbles.

Contract: kernel(**inputs) takes the FULL unsharded inputs and returns the
FULL output [2, 2048, 4096] float32. All sharding / gathering happens here.
"""

import math
import sys

import numpy as np

for _p in ("/opt/trn_rl_repo", "/root/.axon_site/_ro/trn_rl_repo"):
    if _p not in sys.path:
        sys.path.insert(0, _p)

HIDDEN = 4096
N_HEADS = 32
HEAD_DIM = 128
BASE = 10000.0
B = 2
S = 2048
HEADS_PER_CORE = 8          # 32 heads / 4 groups
HG = 1024                   # head-group width = 8 heads * 128
NEG_BIG = -1.0e9

# RoPE partner permutation: quadrant q holds [lo_d 16q..16q+15, hi_d 64+16q..]
# so the rotate-half partner of new-row i is i+-16 inside its 32-row quadrant,
# reachable by DVE stream_shuffle.
PERM = np.zeros(128, dtype=np.int64)
for _q in range(4):
    PERM[32 * _q: 32 * _q + 16] = np.arange(16 * _q, 16 * _q + 16)
    PERM[32 * _q + 16: 32 * _q + 32] = 64 + np.arange(16 * _q, 16 * _q + 16)
SHUF_MASK = [(i + 16) % 32 for i in range(32)]
# sign of the sin term per (new) row: -1 where original d < 64
SIGN = np.where(PERM < 64, -1.0, 1.0).astype(np.float32)


def _rope_tables(max_pos):
    inv_freq = 1.0 / (BASE ** (np.arange(0, HEAD_DIM, 2, dtype=np.float32) / HEAD_DIM))
    t = np.arange(max_pos, dtype=np.float32)
    freqs = np.outer(t, inv_freq)                      # [P, 64]
    emb = np.concatenate((freqs, freqs), axis=-1)      # [P, 128]
    return np.cos(emb).astype(np.float32), np.sin(emb).astype(np.float32)


def _build_program(mask_mode):
    """mask_mode: 'causal' (block-skip + shared triangle mask),
    'none' (dense, no mask), 'full' (dense, stream mask tiles)."""
    import concourse.bacc as bacc
    import concourse.mybir as mybir
    import concourse.tile as tile
    from contextlib import ExitStack

    F32 = mybir.dt.float32
    BF16 = mybir.dt.bfloat16
    ALU = mybir.AluOpType
    ACTF = mybir.ActivationFunctionType

    nc = bacc.Bacc("TRN2", target_bir_lowering=False, debug=False)

    # pre-tiled inputs (host side does all layout work)
    x_t = nc.declare_dram_parameter("x_t", [128, 32, S], BF16, isOutput=False)
    wqk_t = nc.declare_dram_parameter("wqk_t", [16, 128, 32, 128], BF16,
                                      isOutput=False)
    wv_t = nc.declare_dram_parameter("wv_t", [2, 32, 128, 512], BF16,
                                     isOutput=False)
    wo_t = nc.declare_dram_parameter("wo_t", [128, 8, HIDDEN], BF16,
                                     isOutput=False)
    cosT = nc.declare_dram_parameter("cosT", [128, S], BF16, isOutput=False)
    sinT = nc.declare_dram_parameter("sinT", [128, S], BF16, isOutput=False)
    if mask_mode == "causal":
        tri = nc.declare_dram_parameter("tri", [128, 128], F32, isOutput=False)
        triT = nc.declare_dram_parameter("triT", [128, 128], BF16,
                                         isOutput=False)
        idm = nc.declare_dram_parameter("idm", [128, 128], BF16,
                                        isOutput=False)
    elif mask_mode == "full":
        maskT = nc.declare_dram_parameter("maskT", [S, S], F32, isOutput=False)
    out_p = nc.declare_dram_parameter("out_p", [S, HIDDEN], BF16, isOutput=True)

    at_s = nc.dram_tensor("at_scratch", [HG, S], BF16)

    inv_sqrt_d = 1.0 / math.sqrt(HEAD_DIM)

    with tile.TileContext(nc, pool_alloc_mode="queue") as tc, ExitStack() as top:
        const_pool = top.enter_context(tc.tile_pool(name="consts", bufs=1))
        ones_f32 = const_pool.tile([128, 128], F32)
        nc.vector.memset(ones_f32, 1.0)
        ones_sq = const_pool.tile([128, 128], BF16)
        nc.vector.tensor_copy(ones_sq, ones_f32)
        if mask_mode == "causal":
            tri_sb = const_pool.tile([128, 128], F32)
            nc.sync.dma_start(out=tri_sb, in_=tri.ap())
            triT_sb = const_pool.tile([128, 128], BF16)
            nc.sync.dma_start(out=triT_sb, in_=triT.ap())
            id_sb = const_pool.tile([128, 128], BF16)
            nc.sync.dma_start(out=id_sb, in_=idm.ap())

        # SBUF-resident q/k/v; q/k split per head so attention units only
        # wait on the head they read (dep tracking is per-tile)
        res_pool = top.enter_context(tc.tile_pool(name="resident", bufs=1))
        qT_sb = [[res_pool.tile([128, 1024], BF16, name=f"qT{h}_{hh}")
                  for hh in range(HEADS_PER_CORE)] for h in range(2)]
        kT_sb = [[res_pool.tile([128, 1024], BF16, name=f"kT{h}_{hh}")
                  for hh in range(HEADS_PER_CORE)] for h in range(2)]
        v_sb = [res_pool.tile([128, 8, HG], BF16, name=f"v{h}")
                for h in range(2)]

        # ---------------- Phase A: V then QK projection per half -------------
        def emit_half(hs, pha, attn_setup=None):
            s0 = hs * 1024
            # h-chunk sweep order: B-block (16..31) first (its DMA is issued
            # first), A-block (0..15) last.
            H_ORDER = list(range(16, 32)) + list(range(16))
            xpoolA = pha.enter_context(tc.tile_pool(name="xhalfA", bufs=1))
            xpoolB = pha.enter_context(tc.tile_pool(name="xhalfB", bufs=1))
            xtA = xpoolA.tile([128, 16, 1024], BF16, name=f"xtA{hs}")
            xtB = xpoolB.tile([128, 16, 1024], BF16, name=f"xtB{hs}")
            xin = x_t.ap()[:, :, s0:s0 + 1024]

            def emit_x_dma(c):
                dst = xtB[:, c - 16, :] if c >= 16 else xtA[:, c, :]
                nc.sync.dma_start(out=dst, in_=xin[:, c, :])

            def xt_slice(c, sl):
                return xtB[:, c - 16, sl] if c >= 16 else xtA[:, c, sl]

            # --- v projection first, x-stationary, out = v [s, o] ------------
            # x-chunk DMAs are interleaved with the wv tile loads in
            # consumption order so the first pass streams at DMA rate
            with ExitStack() as vv:
                wvp = vv.enter_context(tc.tile_pool(name="wvt", bufs=4))
                pv = vv.enter_context(
                    tc.tile_pool(name="pv", bufs=8, space="PSUM"))
                for ov in range(2):          # v-dim chunks of 512
                    vb = [pv.tile([128, 512], F32, tag="vb", name=f"vb{i}")
                          for i in range(8)]
                    for hi, h in enumerate(H_ORDER):
                        wv_tile = wvp.tile([128, 512], BF16, tag="wv_tile")
                        nc.sync.dma_start(out=wv_tile, in_=wv_t.ap()[ov, h])
                        if ov == 0:
                            emit_x_dma(h)
                        for sc in range(8):
                            nc.tensor.matmul(
                                vb[sc],
                                xt_slice(h, slice(sc * 128, (sc + 1) * 128)),
                                wv_tile,
                                start=(hi == 0), stop=(hi == 31))
                    for sc in range(8):
                        dst = v_sb[hs][:, sc, ov * 512:(ov + 1) * 512]
                        if sc % 2 == 0:
                            nc.scalar.activation(dst, vb[sc], ACTF.Copy)
                        else:
                            nc.vector.tensor_copy(dst, vb[sc])

            # --- q,k projection, weight-stationary, out = projT [o, s] -------
            with ExitStack() as qk:
                cspool = qk.enter_context(tc.tile_pool(name="cossin", bufs=1))
                cos_sb = cspool.tile([128, 1024], BF16)
                nc.sync.dma_start(out=cos_sb, in_=cosT.ap()[:, s0:s0 + 1024])
                sin_sb = cspool.tile([128, 1024], BF16)
                nc.sync.dma_start(out=sin_sb, in_=sinT.ap()[:, s0:s0 + 1024])

                wpool = qk.enter_context(tc.tile_pool(name="wqk", bufs=2))
                pqk = qk.enter_context(
                    tc.tile_pool(name="pqk", bufs=2, space="PSUM"))
                rpool = qk.enter_context(tc.tile_pool(name="rope", bufs=3))
                hook = post_pair = finish = None
                if attn_setup is not None:
                    hook, post_pair, finish = attn_setup(qk)
                # q/k oc pairs back-to-back so head p's attention can weave
                # into the remaining projection stream
                ocs = ([x for p in range(8) for x in (p, p + 8)]
                       if attn_setup else list(range(16)))
                for oci, oc in enumerate(ocs):  # o chunks of 128 (head tiles)
                    w_oc = wpool.tile([128, 32, 128], BF16, tag="w_oc")
                    nc.sync.dma_start(out=w_oc, in_=wqk_t.ap()[oc])
                    pk = pqk.tile([128, 2, 512], F32, tag="pk")
                    for hi, h in enumerate(H_ORDER):
                        for sc in range(2):
                            nc.tensor.matmul(
                                pk[:, sc, :], w_oc[:, h, :],
                                xt_slice(h, slice(sc * 512, (sc + 1) * 512)),
                                start=(hi == 0), stop=(hi == 31))
                        if hook is not None and hi % 4 == 3:
                            hook()
                    # RoPE: q' = q*cos + shuffle16(q)*sin_signed, -> bf16
                    dst = (qT_sb[hs] if oc < 8 else kT_sb[hs])[oc % 8]
                    for sc in range(2):
                        pks = pk[:, sc, :]
                        cs = cos_sb[:, sc * 512:(sc + 1) * 512]
                        sn = sin_sb[:, sc * 512:(sc + 1) * 512]
                        qrot = rpool.tile([128, 512], F32, tag="r", name="qrot")
                        nc.vector.stream_shuffle(qrot, pks, SHUF_MASK)
                        t1 = rpool.tile([128, 512], F32, tag="r", name="t1")
                        nc.vector.tensor_tensor(t1, pks, cs, ALU.mult)
                        t2 = rpool.tile([128, 512], F32, tag="r", name="t2")
                        nc.gpsimd.tensor_tensor(t2, qrot, sn, ALU.mult)
                        nc.vector.tensor_tensor(
                            dst[:, sc * 512:(sc + 1) * 512],
                            t1, t2, ALU.add)
                    if post_pair is not None and oci % 2 == 1:
                        post_pair(oci // 2)
                if finish is not None:
                    finish()

        # ---------------- Phase B: attention, scores kept as S^T [k, q] ------
        # Scores go into [128, 2, 512] PSUM tiles (2 k-blocks per group),
        # exp'd in one ACTIVATE per off-diagonal group, accumulated into a
        # running DVE sum, folded to [128, 512], and reduced+broadcast by a
        # single all-ones stationary matmul (den lands replicated on every
        # PSUM partition). av matmuls trail their exp by two groups; the
        # finalize (recip + av*recip + DMA out) is deferred by one unit.
        def make_attn_emitter(phb, tri_pe=False):
            # tri_pe mode is used when woven into a projection phase: the
            # triangle mask is applied by a tiny PE matmul (identity moving,
            # tri^T stationary, accumulate) so the score->exp chain never
            # hops through the RoPE-congested DVE queue, and PSUM shrinks to
            # ps(2 banks) + shared av/den(2 banks) beside pqk's 4.
            qp_es = phb.enter_context(
                tc.tile_pool(name="es", bufs=2 if tri_pe else 4))
            esump = phb.enter_context(tc.tile_pool(name="esum", bufs=2))
            smallp = phb.enter_context(tc.tile_pool(name="small", bufs=2))
            ps = phb.enter_context(
                tc.tile_pool(name="ps", bufs=1 if tri_pe else 2,
                             space="PSUM"))
            if tri_pe:
                pav = pmisc = phb.enter_context(
                    tc.tile_pool(name="pavd", bufs=2, space="PSUM"))
                av_tag = den_tag = "avd"
            else:
                pav = phb.enter_context(
                    tc.tile_pool(name="pav", bufs=2, space="PSUM"))
                pmisc = phb.enter_context(
                    tc.tile_pool(name="pmisc", bufs=2, space="PSUM"))
                av_tag, den_tag = "av", "misc"
            mp = None
            if mask_mode == "full":
                mp = phb.enter_context(tc.tile_pool(name="msk", bufs=3))

            state = {"pending": None, "pend_den": None}

            def emit_den():
                # den matmul for the previous unit, deferred so the next
                # unit's score matmuls cover the exp->tree->fold latency
                if state["pend_den"] is None:
                    return
                fold, den = state["pend_den"]
                nc.tensor.matmul(den, ones_sq, fold, start=True, stop=True)
                state["pend_den"] = None

            def finalize(av, den, hh, qc):
                recip = smallp.tile([128, 512], F32, tag="recip")
                nc.vector.reciprocal_approx_fast(recip, den)
                at_t = smallp.tile([128, 512], BF16, tag="at_t")
                nc.vector.tensor_tensor(at_t, av, recip, ALU.mult)
                nc.sync.dma_start(
                    out=at_s.ap()[hh * 128:(hh + 1) * 128,
                                  qc * 512:(qc + 1) * 512],
                    in_=at_t)

            def unit_slices(qc, hh):
                # generator: yields once per score group so the caller can
                # interleave foreign PE work between the slices
                emit_den()
                if state["pending"] is not None:
                    finalize(*state["pending"])
                    state["pending"] = None
                nblk = 4 * qc + 4 if mask_mode == "causal" else 16
                ng = nblk // 2
                av = pav.tile([128, 512], F32, tag=av_tag, name="av")
                acc = None
                navs = [0]
                pend_av = []          # (es_tile, j2, kb, q_lo) awaiting av mm

                def emit_avs(upto):
                    while pend_av and len(pend_av) > upto:
                        es, j2, kb, q_lo = pend_av.pop(0)
                        qs = slice(q_lo, 512)
                        khalf, kloc = kb // 8, kb % 8
                        nc.tensor.matmul(
                            av[:, qs],
                            v_sb[khalf][:, kloc, hh * 128:(hh + 1) * 128],
                            es[:, j2, qs],
                            start=(navs[0] == 0), stop=(navs[0] == nblk - 1))
                        navs[0] += 1

                # diagonal groups first: their serial per-region exps overlap
                # the off-diagonal work that follows. The vd=0 block is the
                # first av emitted and covers the full [0:512] PSUM range, so
                # the accumulation start flag is sound.
                if mask_mode == "causal":
                    order = [2 * qc, 2 * qc + 1] + list(range(2 * qc))
                else:
                    order = list(range(ng))
                for gi, g in enumerate(order):
                    es = qp_es.tile([128, 2, 512], BF16, tag="es", name="es")
                    sps = ps.tile([128, 2, 512], F32, tag="sps")
                    diag = False
                    blk = []
                    for j2 in range(2):
                        kb = 2 * g + j2
                        vd = kb - 4 * qc   # diagonal block index
                        q_lo = (128 * vd
                                if (mask_mode == "causal" and vd > 0)
                                else 0)
                        qs = slice(q_lo, 512)
                        khalf, kloc = kb // 8, kb % 8
                        is_d = mask_mode == "causal" and vd >= 0
                        nc.tensor.matmul(
                            sps[:, j2, qs],
                            kT_sb[khalf][hh][:, kloc * 128:(kloc + 1) * 128],
                            qT_sb[qc // 2][hh][:, (qc % 2) * 512 + q_lo:
                                               (qc % 2) * 512 + 512],
                            start=True, stop=not (is_d and tri_pe))
                        if is_d:
                            diag = True
                            mq = slice(128 * vd, 128 * vd + 128)
                            if tri_pe:
                                nc.tensor.matmul(
                                    sps[:, j2, mq], triT_sb, id_sb,
                                    start=False, stop=True)
                            else:
                                nc.vector.tensor_tensor(
                                    sps[:, j2, mq], sps[:, j2, mq], tri_sb,
                                    ALU.add)
                        elif mask_mode == "full":
                            mt = mp.tile([128, 512], F32, tag="mt")
                            nc.sync.dma_start(
                                out=mt,
                                in_=maskT.ap()[kb * 128:(kb + 1) * 128,
                                               qc * 512:(qc + 1) * 512])
                            nc.vector.tensor_tensor(sps[:, j2, :],
                                                    sps[:, j2, :], mt,
                                                    ALU.add)
                        blk.append((kb, q_lo))
                    # exp: one ACTIVATE per clean group; per-region on the
                    # diagonal (unwritten PSUM slivers stay out of the AP)
                    if not diag:
                        nc.scalar.activation(es, sps, ACTF.Exp,
                                             scale=inv_sqrt_d)
                    else:
                        for j2, (kb, q_lo) in enumerate(blk):
                            if q_lo > 0:
                                nc.vector.memset(es[:, j2, 0:q_lo], 0.0)
                            qs = slice(q_lo, 512)
                            nc.scalar.activation(es[:, j2, qs],
                                                 sps[:, j2, qs],
                                                 ACTF.Exp, scale=inv_sqrt_d)
                    # incremental tree: acc += es (DVE), frees es early
                    if gi == 0:
                        acc = es
                    else:
                        if gi == 1:
                            t = esump.tile([128, 2, 512], BF16, tag="e2")
                            nc.vector.tensor_tensor(t, acc, es, ALU.add)
                            acc = t
                        else:
                            nc.vector.tensor_tensor(acc, acc, es, ALU.add)
                    for j2, (kb, q_lo) in enumerate(blk):
                        pend_av.append((es, j2, kb, q_lo))
                    # av matmuls trail their exp by one/two groups
                    emit_avs(2 if tri_pe else 4)
                    yield
                emit_avs(0)
                # fold the two k-block columns -> [128, 512]
                fold = smallp.tile([128, 512], BF16, tag="fold")
                nc.vector.tensor_tensor(fold, acc[:, 0, :], acc[:, 1, :],
                                        ALU.add)
                # den replicated across all 128 partitions via all-ones lhsT;
                # the matmul itself is deferred into the next unit
                den = pmisc.tile([128, 512], F32, tag=den_tag, name="den")
                state["pend_den"] = (fold, den)
                state["pending"] = (av, den, hh, qc)

            def emit_unit(qc, hh):
                for _ in unit_slices(qc, hh):
                    pass

            def flush():
                emit_den()
                if state["pending"] is not None:
                    finalize(*state["pending"])
                    state["pending"] = None

            return emit_unit, flush, pmisc, unit_slices

        # ---------------- Phase C: output projection -------------------------
        def make_c_emitter(phc, pop):
            atp = phc.enter_context(tc.tile_pool(name="atl", bufs=2))
            otp = phc.enter_context(tc.tile_pool(name="ot", bufs=4))
            wop = phc.enter_context(tc.tile_pool(name="wo", bufs=2))
            state = {"at_g": {}, "ot": {}, "wo_sl": None}

            def load_stg(stg):
                at_g = atp.tile([128, 8, 512], BF16, tag="at_g",
                                name=f"at_g{stg}")
                nc.sync.dma_start(
                    out=at_g,
                    in_=at_s.ap()[:, stg * 512:(stg + 1) * 512].rearrange(
                        "(hc p) s -> p hc s", p=128))
                state["at_g"][stg] = at_g

            def emit_unit(stg, st_l, o8):
                # one PSUM bank: out[st, o8] = sum_hc at^T wo
                # o8-major within a stage: wo slice loads once per (stg, o8),
                # the four ot tiles of the stage stay live until its end.
                at_g = state["at_g"][stg]
                st = stg * 4 + st_l
                sl = slice(st_l * 128, (st_l + 1) * 128)
                if st_l == 0:
                    wo_sl = wop.tile([128, 8, 512], BF16, tag="wo_sl",
                                     name=f"wo{stg}_{o8}")
                    nc.sync.dma_start(
                        out=wo_sl,
                        in_=wo_t.ap()[:, :, o8 * 512:(o8 + 1) * 512])
                    state["wo_sl"] = wo_sl
                wo_sl = state["wo_sl"]
                if o8 == 0:
                    state["ot"][st] = otp.tile([128, 8, 512], BF16, tag="ot",
                                               name=f"ot{st}")
                ot = state["ot"][st]
                po = pop.tile([128, 512], F32, tag="misc", name="po")
                for hc in range(8):
                    nc.tensor.matmul(
                        po,
                        at_g[:, hc, sl],
                        wo_sl[:, hc, :],
                        start=(hc == 0), stop=(hc == 7))
                nc.scalar.activation(ot[:, o8, :], po, ACTF.Copy)
                if o8 == 7:
                    nc.sync.dma_start(
                        out=out_p.ap()[st * 128:(st + 1) * 128, :],
                        in_=ot.rearrange("p a b -> p (a b)"))
                    del state["ot"][st]

            return emit_unit, load_stg

        # ================= schedule =================
        from collections import deque

        if mask_mode == "causal":
            # half 0 with B01 (qc0, qc1) woven into the QK oc-pair stream
            def attn_setup0(qk_stack):
                _, flush_b0, _, gen = make_attn_emitter(qk_stack, tri_pe=True)
                pending = deque()

                def hook():
                    while pending:
                        try:
                            next(pending[0])
                            return
                        except StopIteration:
                            pending.popleft()

                def post_pair(p):
                    pending.append(gen(0, p))
                    pending.append(gen(1, p))

                def finish():
                    while pending:
                        try:
                            next(pending[0])
                        except StopIteration:
                            pending.popleft()
                    flush_b0()

                return hook, post_pair, finish

            with ExitStack() as ph0:
                emit_half(0, ph0, attn_setup0)

            with ExitStack() as ph1:
                emit_half(1, ph1)

            # B23 with C(stg0, stg1) interleaved
            with ExitStack() as phbc:
                emit_unit_b, flush_b, pmisc, _ = make_attn_emitter(phbc)
                emit_unit_c, load_stg = make_c_emitter(phbc, pmisc)
                load_stg(0)
                load_stg(1)
                c_units = [(stg, st_l, o8)
                           for stg in (0, 1)
                           for o8 in range(8)
                           for st_l in range(4)]
                for qc in (2, 3):
                    for hh in range(HEADS_PER_CORE):
                        emit_unit_b(qc, hh)
                        if qc == 3 and hh == 0:
                            # at_s rows for stg2 are complete once the
                            # (qc2, hh7) finalize ran inside this unit
                            load_stg(2)
                        # ~4 C units per B unit balances the two streams
                        for _ in range(4):
                            if c_units:
                                emit_unit_c(*c_units.pop(0))
                flush_b()
                load_stg(3)
                while c_units:
                    emit_unit_c(*c_units.pop(0))

                # C(stg2, stg3) dense
                for stg in (2, 3):
                    for o8 in range(8):
                        for st_l in range(4):
                            emit_unit_c(stg, st_l, o8)
        else:
            with ExitStack() as ph0:
                emit_half(0, ph0)
            with ExitStack() as ph1:
                emit_half(1, ph1)
            with ExitStack() as phbc:
                emit_unit_b, flush_b, pmisc, _ = make_attn_emitter(phbc)
                emit_unit_c, load_stg = make_c_emitter(phbc, pmisc)
                for qc in range(4):
                    for hh in range(HEADS_PER_CORE):
                        emit_unit_b(qc, hh)
                # all units emitted; flush and drain C for every stg in order
                flush_b()
                for stg in range(4):
                    load_stg(stg)
                    for o8 in range(8):
                        for st_l in range(4):
                            emit_unit_c(stg, st_l, o8)

    nc.compile()
    return nc


_PROGRAM_CACHE = {}


def _get_program(mask_mode):
    if mask_mode not in _PROGRAM_CACHE:
        _PROGRAM_CACHE[mask_mode] = _build_program(mask_mode)
    return _PROGRAM_CACHE[mask_mode]


def _classify_mask(attention_mask):
    m = np.asarray(attention_mask)
    if not np.any(m):
        return "none"
    neg = np.float32(np.finfo(np.float32).min)
    causal = np.triu(np.full((S, S), neg, dtype=np.float32), k=1)
    for b in range(m.shape[0]):
        if not np.array_equal(m[b, 0], causal):
            return "full"
    return "causal"


def _prep_core_inputs(hidden_states, attention_mask, position_ids, W_pack, W_o,
                      mask_mode):
    from ml_dtypes import bfloat16

    hidden_states = np.asarray(hidden_states, dtype=np.float32)
    W_pack = np.asarray(W_pack, dtype=np.float32)
    W_o = np.asarray(W_o, dtype=np.float32)
    pos = np.asarray(position_ids).astype(np.int64)

    cos_t, sin_t = _rope_tables(int(pos.max()) + 1)
    # per-batch gathered + transposed + row-permuted (+ sign folded into sin)
    cosT_b, sinT_b = [], []
    for b in range(B):
        c = cos_t[pos[b]][:, PERM].T
        s = (sin_t[pos[b]][:, PERM] * SIGN[None, :]).T
        cosT_b.append(np.ascontiguousarray(c.astype(bfloat16)))
        sinT_b.append(np.ascontiguousarray(s.astype(bfloat16)))

    # x_t[p, c, s] = hidden[b, s, c*128+p]
    x_b = [np.ascontiguousarray(
        hidden_states[b].T.reshape(32, 128, S).transpose(1, 0, 2)
        .astype(bfloat16)) for b in range(B)]

    tri_m = None
    triT_m = idm_m = None
    maskT_b = None
    if mask_mode == "causal":
        kk = np.arange(128)[:, None]
        qq = np.arange(128)[None, :]
        tri_m = np.where(kk <= qq, 0.0, NEG_BIG).astype(np.float32)
        triT_m = np.ascontiguousarray(tri_m.T.astype(bfloat16))
        idm_m = np.ascontiguousarray(np.eye(128, dtype=np.float32)
                                     .astype(bfloat16))
    elif mask_mode == "full":
        m = np.asarray(attention_mask, dtype=np.float32)
        maskT_b = [np.ascontiguousarray(m[b, 0].T) for b in range(B)]

    in_maps = []
    for cidx in range(8):
        b, g = cidx // 4, cidx % 4
        # per-head d-permuted q/k weight rows, head-major columns in wqk
        qrows = np.concatenate(
            [g * HG + hh * 128 + PERM for hh in range(HEADS_PER_CORE)])
        krows = HIDDEN + qrows
        vrows = 2 * HIDDEN + g * HG + np.arange(HG)
        wqk = np.concatenate([W_pack[qrows], W_pack[krows]], axis=0)  # [2048,4096]
        # wqk_t[oc, p, c, o] = wqk[oc*128+o, c*128+p]
        wqk_t = np.ascontiguousarray(
            wqk.reshape(16, 128, 32, 128).transpose(0, 3, 2, 1)
            .astype(bfloat16))
        wv = W_pack[vrows]                                            # [1024,4096]
        # wv_t[ov, c, p, o] = wv[ov*512+o, c*128+p]
        wv_t = np.ascontiguousarray(
            wv.reshape(2, 512, 32, 128).transpose(0, 2, 3, 1).astype(bfloat16))
        # wo_t[p, hc, o] = W_o[o, g*HG + hc*128 + p]
        wo_t = np.ascontiguousarray(
            W_o[:, g * HG:(g + 1) * HG].reshape(HIDDEN, 8, 128)
            .transpose(2, 1, 0).astype(bfloat16))
        im = {"x_t": x_b[b], "wqk_t": wqk_t, "wv_t": wv_t, "wo_t": wo_t,
              "cosT": cosT_b[b], "sinT": sinT_b[b]}
        if mask_mode == "causal":
            im["tri"] = tri_m
            im["triT"] = triT_m
            im["idm"] = idm_m
        elif mask_mode == "full":
            im["maskT"] = maskT_b[b]
        in_maps.append(im)
    return in_maps


def _run(hidden_states, attention_mask, position_ids, W_pack, W_o,
         trace=False, trace_kwargs=None):
    from concourse.bass_utils import run_bass_kernel_spmd

    mask_mode = _classify_mask(attention_mask)
    nc = _get_program(mask_mode)
    in_maps = _prep_core_inputs(hidden_states, attention_mask, position_ids,
                                W_pack, W_o, mask_mode)
    try:
        res = run_bass_kernel_spmd(nc, in_maps, list(range(8)), trace=trace,
                                   **(trace_kwargs or {}))
    except Exception:
        # transient NRT_EXEC_UNIT_UNRECOVERABLE wedges recover on retry
        import time as _time
        _time.sleep(15)
        res = run_bass_kernel_spmd(nc, in_maps, list(range(8)), trace=trace,
                                   **(trace_kwargs or {}))
    out = np.zeros((B, S, HIDDEN), dtype=np.float32)
    for c in range(8):
        out[c // 4] += np.asarray(res.results[c]["out_p"], dtype=np.float32)
    return out, res


def kernel(hidden_states, attention_mask, position_ids, W_pack, W_o):
    out, _ = _run(hidden_states, attention_mask, position_ids, W_pack, W_o)
    return out


# revision 41
# speedup vs baseline: 1.0822x; 1.0017x over previous
"""Trainium2 Bass kernel for a single attention layer (Baichuan-style W_pack
attention with rotary embeddings), sharded over 8 NeuronCores:
tensor-parallel over 4 head groups x data-parallel over 2 batches.

v3: scheduling overhaul toward the bf16 PE stream floor (~1.01 ms):
 - V-proj runs before QK-proj in each half so the x DMA stream is consumed
   at arrival rate (kills the ~20 us x-wait stall at each half start).
 - softmax denominator: single all-ones [128,128] stationary matmul whose
   PSUM output is den replicated across all partitions (no [1,512] recip
   chain, no separate broadcast matmul), fed by an incremental DVE tree +
   fold so there is one den matmul per (head, q-chunk).
 - exp batched per 2 score blocks (one ACTIVATE over [128,1024]) off the
   diagonal; av matmuls trail their exp by two groups so the PE never
   waits on the ACT exp.
 - O-proj work for seq groups 0,1 is interleaved into the qc2/qc3
   attention emission to fill residual exp-chain bubbles.

Contract: kernel(**inputs) takes the FULL unsharded inputs and returns the
FULL output [2, 2048, 4096] float32. All sharding / gathering happens here.
"""

import math
import sys

import numpy as np

for _p in ("/opt/trn_rl_repo", "/root/.axon_site/_ro/trn_rl_repo"):
    if _p not in sys.path:
        sys.path.insert(0, _p)

HIDDEN = 4096
N_HEADS = 32
HEAD_DIM = 128
BASE = 10000.0
B = 2
S = 2048
HEADS_PER_CORE = 8          # 32 heads / 4 groups
HG = 1024                   # head-group width = 8 heads * 128
NEG_BIG = -1.0e9

# RoPE partner permutation: quadrant q holds [lo_d 16q..16q+15, hi_d 64+16q..]
# so the rotate-half partner of new-row i is i+-16 inside its 32-row quadrant,
# reachable by DVE stream_shuffle.
PERM = np.zeros(128, dtype=np.int64)
for _q in range(4):
    PERM[32 * _q: 32 * _q + 16] = np.arange(16 * _q, 16 * _q + 16)
    PERM[32 * _q + 16: 32 * _q + 32] = 64 + np.arange(16 * _q, 16 * _q + 16)
SHUF_MASK = [(i + 16) % 32 for i in range(32)]
# sign of the sin term per (new) row: -1 where original d < 64
SIGN = np.where(PERM < 64, -1.0, 1.0).astype(np.float32)


def _rope_tables(max_pos):
    inv_freq = 1.0 / (BASE ** (np.arange(0, HEAD_DIM, 2, dtype=np.float32) / HEAD_DIM))
    t = np.arange(max_pos, dtype=np.float32)
    freqs = np.outer(t, inv_freq)                      # [P, 64]
    emb = np.concatenate((freqs, freqs), axis=-1)      # [P, 128]
    return np.cos(emb).astype(np.float32), np.sin(emb).astype(np.float32)


def _build_program(mask_mode):
    """mask_mode: 'causal' (block-skip + shared triangle mask),
    'none' (dense, no mask), 'full' (dense, stream mask tiles)."""
    import concourse.bacc as bacc
    import concourse.mybir as mybir
    import concourse.tile as tile
    from contextlib import ExitStack

    F32 = mybir.dt.float32
    BF16 = mybir.dt.bfloat16
    ALU = mybir.AluOpType
    ACTF = mybir.ActivationFunctionType

    nc = bacc.Bacc("TRN2", target_bir_lowering=False, debug=False)

    # pre-tiled inputs (host side does all layout work)
    x_t = nc.declare_dram_parameter("x_t", [128, 32, S], BF16, isOutput=False)
    wqk_t = nc.declare_dram_parameter("wqk_t", [16, 128, 32, 128], BF16,
                                      isOutput=False)
    wv_t = nc.declare_dram_parameter("wv_t", [2, 32, 128, 512], BF16,
                                     isOutput=False)
    wo_t = nc.declare_dram_parameter("wo_t", [128, 8, HIDDEN], BF16,
                                     isOutput=False)
    cosT = nc.declare_dram_parameter("cosT", [128, S], BF16, isOutput=False)
    sinT = nc.declare_dram_parameter("sinT", [128, S], BF16, isOutput=False)
    if mask_mode == "causal":
        tri = nc.declare_dram_parameter("tri", [128, 128], F32, isOutput=False)
        triT = nc.declare_dram_parameter("triT", [128, 128], BF16,
                                         isOutput=False)
        idm = nc.declare_dram_parameter("idm", [128, 128], BF16,
                                        isOutput=False)
    elif mask_mode == "full":
        maskT = nc.declare_dram_parameter("maskT", [S, S], F32, isOutput=False)
    out_p = nc.declare_dram_parameter("out_p", [S, HIDDEN], BF16, isOutput=True)

    at_s = nc.dram_tensor("at_scratch", [HG, S], BF16)

    inv_sqrt_d = 1.0 / math.sqrt(HEAD_DIM)

    with tile.TileContext(nc, pool_alloc_mode="queue") as tc, ExitStack() as top:
        const_pool = top.enter_context(tc.tile_pool(name="consts", bufs=1))
        ones_f32 = const_pool.tile([128, 128], F32)
        nc.vector.memset(ones_f32, 1.0)
        ones_sq = const_pool.tile([128, 128], BF16)
        nc.vector.tensor_copy(ones_sq, ones_f32)
        if mask_mode == "causal":
            tri_sb = const_pool.tile([128, 128], F32)
            nc.sync.dma_start(out=tri_sb, in_=tri.ap())
            triT_sb = const_pool.tile([128, 128], BF16)
            nc.sync.dma_start(out=triT_sb, in_=triT.ap())
            id_sb = const_pool.tile([128, 128], BF16)
            nc.sync.dma_start(out=id_sb, in_=idm.ap())

        # SBUF-resident q/k/v; q/k split per head so attention units only
        # wait on the head they read (dep tracking is per-tile)
        res_pool = top.enter_context(tc.tile_pool(name="resident", bufs=1))
        qT_sb = [[res_pool.tile([128, 1024], BF16, name=f"qT{h}_{hh}")
                  for hh in range(HEADS_PER_CORE)] for h in range(2)]
        kT_sb = [[res_pool.tile([128, 1024], BF16, name=f"kT{h}_{hh}")
                  for hh in range(HEADS_PER_CORE)] for h in range(2)]
        v_sb = [res_pool.tile([128, 8, HG], BF16, name=f"v{h}")
                for h in range(2)]

        # ---------------- Phase A: V then QK projection per half -------------
        def emit_half(hs, pha, attn_setup=None):
            s0 = hs * 1024
            # h-chunk sweep order: B-block (16..31) first (its DMA is issued
            # first), A-block (0..15) last.
            H_ORDER = list(range(16, 32)) + list(range(16))
            xpoolA = pha.enter_context(tc.tile_pool(name="xhalfA", bufs=1))
            xpoolB = pha.enter_context(tc.tile_pool(name="xhalfB", bufs=1))
            xtA = xpoolA.tile([128, 16, 1024], BF16, name=f"xtA{hs}")
            xtB = xpoolB.tile([128, 16, 1024], BF16, name=f"xtB{hs}")
            xin = x_t.ap()[:, :, s0:s0 + 1024]

            def emit_x_dma(c):
                dst = xtB[:, c - 16, :] if c >= 16 else xtA[:, c, :]
                nc.sync.dma_start(out=dst, in_=xin[:, c, :])

            def xt_slice(c, sl):
                return xtB[:, c - 16, sl] if c >= 16 else xtA[:, c, sl]

            # --- v projection first, x-stationary, out = v [s, o] ------------
            # x-chunk DMAs are interleaved with the wv tile loads in
            # consumption order so the first pass streams at DMA rate
            with ExitStack() as vv:
                wvp = vv.enter_context(tc.tile_pool(name="wvt", bufs=4))
                pv = vv.enter_context(
                    tc.tile_pool(name="pv", bufs=8, space="PSUM"))
                for ov in range(2):          # v-dim chunks of 512
                    vb = [pv.tile([128, 512], F32, tag="vb", name=f"vb{i}")
                          for i in range(8)]
                    for hi, h in enumerate(H_ORDER):
                        wv_tile = wvp.tile([128, 512], BF16, tag="wv_tile")
                        nc.sync.dma_start(out=wv_tile, in_=wv_t.ap()[ov, h])
                        if ov == 0:
                            emit_x_dma(h)
                        for sc in range(8):
                            nc.tensor.matmul(
                                vb[sc],
                                xt_slice(h, slice(sc * 128, (sc + 1) * 128)),
                                wv_tile,
                                start=(hi == 0), stop=(hi == 31))
                    for sc in range(8):
                        dst = v_sb[hs][:, sc, ov * 512:(ov + 1) * 512]
                        if sc % 2 == 0:
                            nc.scalar.activation(dst, vb[sc], ACTF.Copy)
                        else:
                            nc.vector.tensor_copy(dst, vb[sc])

            # --- q,k projection, weight-stationary, out = projT [o, s] -------
            with ExitStack() as qk:
                cspool = qk.enter_context(tc.tile_pool(name="cossin", bufs=1))
                cos_sb = cspool.tile([128, 1024], BF16)
                nc.sync.dma_start(out=cos_sb, in_=cosT.ap()[:, s0:s0 + 1024])
                sin_sb = cspool.tile([128, 1024], BF16)
                nc.sync.dma_start(out=sin_sb, in_=sinT.ap()[:, s0:s0 + 1024])

                wpool = qk.enter_context(tc.tile_pool(name="wqk", bufs=2))
                pqk = qk.enter_context(
                    tc.tile_pool(name="pqk", bufs=2, space="PSUM"))
                rpool = qk.enter_context(tc.tile_pool(name="rope", bufs=3))
                hook = post_pair = finish = None
                if attn_setup is not None:
                    hook, post_pair, finish = attn_setup(qk)
                # q/k oc pairs back-to-back so head p's attention can weave
                # into the remaining projection stream
                ocs = ([x for p in range(8) for x in (p, p + 8)]
                       if attn_setup else list(range(16)))
                for oci, oc in enumerate(ocs):  # o chunks of 128 (head tiles)
                    w_oc = wpool.tile([128, 32, 128], BF16, tag="w_oc")
                    nc.sync.dma_start(out=w_oc, in_=wqk_t.ap()[oc])
                    pk = pqk.tile([128, 2, 512], F32, tag="pk")
                    for hi, h in enumerate(H_ORDER):
                        for sc in range(2):
                            nc.tensor.matmul(
                                pk[:, sc, :], w_oc[:, h, :],
                                xt_slice(h, slice(sc * 512, (sc + 1) * 512)),
                                start=(hi == 0), stop=(hi == 31))
                        if hook is not None and hi % 4 == 3:
                            hook()
                    # RoPE: q' = q*cos + shuffle16(q)*sin_signed, -> bf16
                    dst = (qT_sb[hs] if oc < 8 else kT_sb[hs])[oc % 8]
                    for sc in range(2):
                        pks = pk[:, sc, :]
                        cs = cos_sb[:, sc * 512:(sc + 1) * 512]
                        sn = sin_sb[:, sc * 512:(sc + 1) * 512]
                        qrot = rpool.tile([128, 512], F32, tag="r", name="qrot")
                        nc.vector.stream_shuffle(qrot, pks, SHUF_MASK)
                        t1 = rpool.tile([128, 512], F32, tag="r", name="t1")
                        nc.vector.tensor_tensor(t1, pks, cs, ALU.mult)
                        t2 = rpool.tile([128, 512], F32, tag="r", name="t2")
                        nc.gpsimd.tensor_tensor(t2, qrot, sn, ALU.mult)
                        nc.vector.tensor_tensor(
                            dst[:, sc * 512:(sc + 1) * 512],
                            t1, t2, ALU.add)
                    if post_pair is not None and oci % 2 == 1:
                        post_pair(oci // 2)
                if finish is not None:
                    finish()

        # ---------------- Phase B: attention, scores kept as S^T [k, q] ------
        # Scores go into [128, 2, 512] PSUM tiles (2 k-blocks per group),
        # exp'd in one ACTIVATE per off-diagonal group, accumulated into a
        # running DVE sum, folded to [128, 512], and reduced+broadcast by a
        # single all-ones stationary matmul (den lands replicated on every
        # PSUM partition). av matmuls trail their exp by two groups; the
        # finalize (recip + av*recip + DMA out) is deferred by one unit.
        def make_attn_emitter(phb, tri_pe=False):
            # tri_pe mode is used when woven into a projection phase: the
            # triangle mask is applied by a tiny PE matmul (identity moving,
            # tri^T stationary, accumulate) so the score->exp chain never
            # hops through the RoPE-congested DVE queue, and PSUM shrinks to
            # ps(2 banks) + shared av/den(2 banks) beside pqk's 4.
            qp_es = phb.enter_context(
                tc.tile_pool(name="es", bufs=2 if tri_pe else 4))
            esump = phb.enter_context(tc.tile_pool(name="esum", bufs=2))
            smallp = phb.enter_context(tc.tile_pool(name="small", bufs=2))
            ps = phb.enter_context(
                tc.tile_pool(name="ps", bufs=1 if tri_pe else 2,
                             space="PSUM"))
            if tri_pe:
                pav = pmisc = phb.enter_context(
                    tc.tile_pool(name="pavd", bufs=2, space="PSUM"))
                av_tag = den_tag = "avd"
            else:
                pav = phb.enter_context(
                    tc.tile_pool(name="pav", bufs=2, space="PSUM"))
                pmisc = phb.enter_context(
                    tc.tile_pool(name="pmisc", bufs=2, space="PSUM"))
                av_tag, den_tag = "av", "misc"
            mp = None
            if mask_mode == "full":
                mp = phb.enter_context(tc.tile_pool(name="msk", bufs=3))

            state = {"pending": None, "pend_den": None}

            def emit_den():
                # den matmul for the previous unit, deferred so the next
                # unit's score matmuls cover the exp->tree->fold latency
                if state["pend_den"] is None:
                    return
                fold, den = state["pend_den"]
                nc.tensor.matmul(den, ones_sq, fold, start=True, stop=True)
                state["pend_den"] = None

            def finalize(av, den, hh, qc):
                recip = smallp.tile([128, 512], F32, tag="recip")
                nc.vector.reciprocal_approx_fast(recip, den)
                at_t = smallp.tile([128, 512], BF16, tag="at_t")
                nc.vector.tensor_tensor(at_t, av, recip, ALU.mult)
                nc.sync.dma_start(
                    out=at_s.ap()[hh * 128:(hh + 1) * 128,
                                  qc * 512:(qc + 1) * 512],
                    in_=at_t)

            def unit_slices(qc, hh):
                # generator: yields once per score group so the caller can
                # interleave foreign PE work between the slices
                emit_den()
                if state["pending"] is not None:
                    finalize(*state["pending"])
                    state["pending"] = None
                nblk = 4 * qc + 4 if mask_mode == "causal" else 16
                ng = nblk // 2
                av = pav.tile([128, 512], F32, tag=av_tag, name="av")
                acc = None
                navs = [0]
                pend_av = []          # (es_tile, j2, kb, q_lo) awaiting av mm

                def emit_avs(upto):
                    while pend_av and len(pend_av) > upto:
                        es, j2, kb, q_lo = pend_av.pop(0)
                        qs = slice(q_lo, 512)
                        khalf, kloc = kb // 8, kb % 8
                        nc.tensor.matmul(
                            av[:, qs],
                            v_sb[khalf][:, kloc, hh * 128:(hh + 1) * 128],
                            es[:, j2, qs],
                            start=(navs[0] == 0), stop=(navs[0] == nblk - 1))
                        navs[0] += 1

                # diagonal groups first: their serial per-region exps overlap
                # the off-diagonal work that follows. The vd=0 block is the
                # first av emitted and covers the full [0:512] PSUM range, so
                # the accumulation start flag is sound.
                if mask_mode == "causal":
                    order = [2 * qc, 2 * qc + 1] + list(range(2 * qc))
                else:
                    order = list(range(ng))
                for gi, g in enumerate(order):
                    es = qp_es.tile([128, 2, 512], BF16, tag="es", name="es")
                    sps = ps.tile([128, 2, 512], F32, tag="sps")
                    diag = False
                    blk = []
                    for j2 in range(2):
                        kb = 2 * g + j2
                        vd = kb - 4 * qc   # diagonal block index
                        q_lo = (128 * vd
                                if (mask_mode == "causal" and vd > 0)
                                else 0)
                        qs = slice(q_lo, 512)
                        khalf, kloc = kb // 8, kb % 8
                        is_d = mask_mode == "causal" and vd >= 0
                        nc.tensor.matmul(
                            sps[:, j2, qs],
                            kT_sb[khalf][hh][:, kloc * 128:(kloc + 1) * 128],
                            qT_sb[qc // 2][hh][:, (qc % 2) * 512 + q_lo:
                                               (qc % 2) * 512 + 512],
                            start=True, stop=not (is_d and tri_pe))
                        if is_d:
                            diag = True
                            mq = slice(128 * vd, 128 * vd + 128)
                            if tri_pe:
                                nc.tensor.matmul(
                                    sps[:, j2, mq], triT_sb, id_sb,
                                    start=False, stop=True)
                            else:
                                nc.vector.tensor_tensor(
                                    sps[:, j2, mq], sps[:, j2, mq], tri_sb,
                                    ALU.add)
                        elif mask_mode == "full":
                            mt = mp.tile([128, 512], F32, tag="mt")
                            nc.sync.dma_start(
                                out=mt,
                                in_=maskT.ap()[kb * 128:(kb + 1) * 128,
                                               qc * 512:(qc + 1) * 512])
                            nc.vector.tensor_tensor(sps[:, j2, :],
                                                    sps[:, j2, :], mt,
                                                    ALU.add)
                        blk.append((kb, q_lo))
                    # exp: one ACTIVATE per clean group; per-region on the
                    # diagonal (unwritten PSUM slivers stay out of the AP)
                    if not diag:
                        nc.scalar.activation(es, sps, ACTF.Exp,
                                             scale=inv_sqrt_d)
                    else:
                        for j2, (kb, q_lo) in enumerate(blk):
                            if q_lo > 0:
                                nc.vector.memset(es[:, j2, 0:q_lo], 0.0)
                            qs = slice(q_lo, 512)
                            nc.scalar.activation(es[:, j2, qs],
                                                 sps[:, j2, qs],
                                                 ACTF.Exp, scale=inv_sqrt_d)
                    # incremental tree: acc += es (DVE), frees es early
                    if gi == 0:
                        acc = es
                    else:
                        if gi == 1:
                            t = esump.tile([128, 2, 512], BF16, tag="e2")
                            nc.vector.tensor_tensor(t, acc, es, ALU.add)
                            acc = t
                        else:
                            nc.vector.tensor_tensor(acc, acc, es, ALU.add)
                    for j2, (kb, q_lo) in enumerate(blk):
                        pend_av.append((es, j2, kb, q_lo))
                    # av matmuls trail their exp by one/two groups
                    emit_avs(2 if tri_pe else 4)
                    yield
                emit_avs(0)
                # fold the two k-block columns -> [128, 512]
                fold = smallp.tile([128, 512], BF16, tag="fold")
                nc.vector.tensor_tensor(fold, acc[:, 0, :], acc[:, 1, :],
                                        ALU.add)
                # den replicated across all 128 partitions via all-ones lhsT;
                # the matmul itself is deferred into the next unit
                den = pmisc.tile([128, 512], F32, tag=den_tag, name="den")
                state["pend_den"] = (fold, den)
                state["pending"] = (av, den, hh, qc)

            def emit_unit(qc, hh):
                for _ in unit_slices(qc, hh):
                    pass

            def flush():
                emit_den()
                if state["pending"] is not None:
                    finalize(*state["pending"])
                    state["pending"] = None

            return emit_unit, flush, pmisc, unit_slices

        # ---------------- Phase C: output projection -------------------------
        def make_c_emitter(phc, pop):
            atp = phc.enter_context(tc.tile_pool(name="atl", bufs=3))
            otp = phc.enter_context(tc.tile_pool(name="ot", bufs=4))
            wop = phc.enter_context(tc.tile_pool(name="wo", bufs=3))
            state = {"at_g": {}, "ot": {}, "wo_sl": None}

            def load_stg(stg):
                at_g = atp.tile([128, 8, 512], BF16, tag="at_g",
                                name=f"at_g{stg}")
                nc.sync.dma_start(
                    out=at_g,
                    in_=at_s.ap()[:, stg * 512:(stg + 1) * 512].rearrange(
                        "(hc p) s -> p hc s", p=128))
                state["at_g"][stg] = at_g

            def emit_unit(stg, st_l, o8):
                # one PSUM bank: out[st, o8] = sum_hc at^T wo
                # o8-major within a stage: wo slice loads once per (stg, o8),
                # the four ot tiles of the stage stay live until its end.
                at_g = state["at_g"][stg]
                st = stg * 4 + st_l
                sl = slice(st_l * 128, (st_l + 1) * 128)
                if st_l == 0:
                    wo_sl = wop.tile([128, 8, 512], BF16, tag="wo_sl",
                                     name=f"wo{stg}_{o8}")
                    nc.sync.dma_start(
                        out=wo_sl,
                        in_=wo_t.ap()[:, :, o8 * 512:(o8 + 1) * 512])
                    state["wo_sl"] = wo_sl
                wo_sl = state["wo_sl"]
                if o8 == 0:
                    state["ot"][st] = otp.tile([128, 8, 512], BF16, tag="ot",
                                               name=f"ot{st}")
                ot = state["ot"][st]
                po = pop.tile([128, 512], F32, tag="misc", name="po")
                for hc in range(8):
                    nc.tensor.matmul(
                        po,
                        at_g[:, hc, sl],
                        wo_sl[:, hc, :],
                        start=(hc == 0), stop=(hc == 7))
                nc.scalar.activation(ot[:, o8, :], po, ACTF.Copy)
                if o8 == 7:
                    nc.sync.dma_start(
                        out=out_p.ap()[st * 128:(st + 1) * 128, :],
                        in_=ot.rearrange("p a b -> p (a b)"))
                    del state["ot"][st]

            return emit_unit, load_stg

        # ================= schedule =================
        from collections import deque

        if mask_mode == "causal":
            # half 0 with B01 (qc0, qc1) woven into the QK oc-pair stream
            def attn_setup0(qk_stack):
                _, flush_b0, _, gen = make_attn_emitter(qk_stack, tri_pe=True)
                pending = deque()

                def hook():
                    while pending:
                        try:
                            next(pending[0])
                            return
                        except StopIteration:
                            pending.popleft()

                def post_pair(p):
                    pending.append(gen(0, p))
                    pending.append(gen(1, p))

                def finish():
                    while pending:
                        try:
                            next(pending[0])
                        except StopIteration:
                            pending.popleft()
                    flush_b0()

                return hook, post_pair, finish

            with ExitStack() as ph0:
                emit_half(0, ph0, attn_setup0)

            with ExitStack() as ph1:
                emit_half(1, ph1)

            # B23 with C(stg0, stg1) interleaved
            with ExitStack() as phbc:
                emit_unit_b, flush_b, pmisc, _ = make_attn_emitter(phbc)
                emit_unit_c, load_stg = make_c_emitter(phbc, pmisc)
                load_stg(0)
                load_stg(1)
                c_units = [(stg, st_l, o8)
                           for stg in (0, 1)
                           for o8 in range(8)
                           for st_l in range(4)]
                for _ in range(8):
                    if c_units:
                        emit_unit_c(*c_units.pop(0))
                for qc in (2, 3):
                    for hh in range(HEADS_PER_CORE):
                        emit_unit_b(qc, hh)
                        if qc == 3 and hh == 0:
                            # at_s rows for stg2 are complete once the
                            # (qc2, hh7) finalize ran inside this unit
                            load_stg(2)
                        # ~4 C units per B unit balances the two streams
                        for _ in range(4):
                            if c_units:
                                emit_unit_c(*c_units.pop(0))
                flush_b()
                load_stg(3)
                while c_units:
                    emit_unit_c(*c_units.pop(0))

                # C(stg2, stg3) dense
                for stg in (2, 3):
                    for o8 in range(8):
                        for st_l in range(4):
                            emit_unit_c(stg, st_l, o8)
        else:
            with ExitStack() as ph0:
                emit_half(0, ph0)
            with ExitStack() as ph1:
                emit_half(1, ph1)
            with ExitStack() as phbc:
                emit_unit_b, flush_b, pmisc, _ = make_attn_emitter(phbc)
                emit_unit_c, load_stg = make_c_emitter(phbc, pmisc)
                for qc in range(4):
                    for hh in range(HEADS_PER_CORE):
                        emit_unit_b(qc, hh)
                # all units emitted; flush and drain C for every stg in order
                flush_b()
                for stg in range(4):
                    load_stg(stg)
                    for o8 in range(8):
                        for st_l in range(4):
                            emit_unit_c(stg, st_l, o8)

    nc.compile()
    return nc


_PROGRAM_CACHE = {}


def _get_program(mask_mode):
    if mask_mode not in _PROGRAM_CACHE:
        _PROGRAM_CACHE[mask_mode] = _build_program(mask_mode)
    return _PROGRAM_CACHE[mask_mode]


def _classify_mask(attention_mask):
    m = np.asarray(attention_mask)
    if not np.any(m):
        return "none"
    neg = np.float32(np.finfo(np.float32).min)
    causal = np.triu(np.full((S, S), neg, dtype=np.float32), k=1)
    for b in range(m.shape[0]):
        if not np.array_equal(m[b, 0], causal):
            return "full"
    return "causal"


def _prep_core_inputs(hidden_states, attention_mask, position_ids, W_pack, W_o,
                      mask_mode):
    from ml_dtypes import bfloat16

    hidden_states = np.asarray(hidden_states, dtype=np.float32)
    W_pack = np.asarray(W_pack, dtype=np.float32)
    W_o = np.asarray(W_o, dtype=np.float32)
    pos = np.asarray(position_ids).astype(np.int64)

    cos_t, sin_t = _rope_tables(int(pos.max()) + 1)
    # per-batch gathered + transposed + row-permuted (+ sign folded into sin)
    cosT_b, sinT_b = [], []
    for b in range(B):
        c = cos_t[pos[b]][:, PERM].T
        s = (sin_t[pos[b]][:, PERM] * SIGN[None, :]).T
        cosT_b.append(np.ascontiguousarray(c.astype(bfloat16)))
        sinT_b.append(np.ascontiguousarray(s.astype(bfloat16)))

    # x_t[p, c, s] = hidden[b, s, c*128+p]
    x_b = [np.ascontiguousarray(
        hidden_states[b].T.reshape(32, 128, S).transpose(1, 0, 2)
        .astype(bfloat16)) for b in range(B)]

    tri_m = None
    triT_m = idm_m = None
    maskT_b = None
    if mask_mode == "causal":
        kk = np.arange(128)[:, None]
        qq = np.arange(128)[None, :]
        tri_m = np.where(kk <= qq, 0.0, NEG_BIG).astype(np.float32)
        triT_m = np.ascontiguousarray(tri_m.T.astype(bfloat16))
        idm_m = np.ascontiguousarray(np.eye(128, dtype=np.float32)
                                     .astype(bfloat16))
    elif mask_mode == "full":
        m = np.asarray(attention_mask, dtype=np.float32)
        maskT_b = [np.ascontiguousarray(m[b, 0].T) for b in range(B)]

    in_maps = []
    for cidx in range(8):
        b, g = cidx // 4, cidx % 4
        # per-head d-permuted q/k weight rows, head-major columns in wqk
        qrows = np.concatenate(
            [g * HG + hh * 128 + PERM for hh in range(HEADS_PER_CORE)])
        krows = HIDDEN + qrows
        vrows = 2 * HIDDEN + g * HG + np.arange(HG)
        wqk = np.concatenate([W_pack[qrows], W_pack[krows]], axis=0)  # [2048,4096]
        # wqk_t[oc, p, c, o] = wqk[oc*128+o, c*128+p]
        wqk_t = np.ascontiguousarray(
            wqk.reshape(16, 128, 32, 128).transpose(0, 3, 2, 1)
            .astype(bfloat16))
        wv = W_pack[vrows]                                            # [1024,4096]
        # wv_t[ov, c, p, o] = wv[ov*512+o, c*128+p]
        wv_t = np.ascontiguousarray(
            wv.reshape(2, 512, 32, 128).transpose(0, 2, 3, 1).astype(bfloat16))
        # wo_t[p, hc, o] = W_o[o, g*HG + hc*128 + p]
        wo_t = np.ascontiguousarray(
            W_o[:, g * HG:(g + 1) * HG].reshape(HIDDEN, 8, 128)
            .transpose(2, 1, 0).astype(bfloat16))
        im = {"x_t": x_b[b], "wqk_t": wqk_t, "wv_t": wv_t, "wo_t": wo_t,
              "cosT": cosT_b[b], "sinT": sinT_b[b]}
        if mask_mode == "causal":
            im["tri"] = tri_m
            im["triT"] = triT_m
            im["idm"] = idm_m
        elif mask_mode == "full":
            im["maskT"] = maskT_b[b]
        in_maps.append(im)
    return in_maps


def _run(hidden_states, attention_mask, position_ids, W_pack, W_o,
         trace=False, trace_kwargs=None):
    from concourse.bass_utils import run_bass_kernel_spmd

    mask_mode = _classify_mask(attention_mask)
    nc = _get_program(mask_mode)
    in_maps = _prep_core_inputs(hidden_states, attention_mask, position_ids,
                                W_pack, W_o, mask_mode)
    try:
        res = run_bass_kernel_spmd(nc, in_maps, list(range(8)), trace=trace,
                                   **(trace_kwargs or {}))
    except Exception:
        # transient NRT_EXEC_UNIT_UNRECOVERABLE wedges recover on retry
        import time as _time
        _time.sleep(15)
        res = run_bass_kernel_spmd(nc, in_maps, list(range(8)), trace=trace,
                                   **(trace_kwargs or {}))
    out = np.zeros((B, S, HIDDEN), dtype=np.float32)
    for c in range(8):
        out[c // 4] += np.asarray(res.results[c]["out_p"], dtype=np.float32)
    return out, res


def kernel(hidden_states, attention_mask, position_ids, W_pack, W_o):
    out, _ = _run(hidden_states, attention_mask, position_ids, W_pack, W_o)
    return out


# revision 42
# speedup vs baseline: 1.0978x; 1.0145x over previous
"""Trainium2 Bass kernel for a single attention layer (Baichuan-style W_pack
attention with rotary embeddings), sharded over 8 NeuronCores:
tensor-parallel over 4 head groups x data-parallel over 2 batches.

v3: scheduling overhaul toward the bf16 PE stream floor (~1.01 ms):
 - V-proj runs before QK-proj in each half so the x DMA stream is consumed
   at arrival rate (kills the ~20 us x-wait stall at each half start).
 - softmax denominator: single all-ones [128,128] stationary matmul whose
   PSUM output is den replicated across all partitions (no [1,512] recip
   chain, no separate broadcast matmul), fed by an incremental DVE tree +
   fold so there is one den matmul per (head, q-chunk).
 - exp batched per 2 score blocks (one ACTIVATE over [128,1024]) off the
   diagonal; av matmuls trail their exp by two groups so the PE never
   waits on the ACT exp.
 - O-proj work for seq groups 0,1 is interleaved into the qc2/qc3
   attention emission to fill residual exp-chain bubbles.

Contract: kernel(**inputs) takes the FULL unsharded inputs and returns the
FULL output [2, 2048, 4096] float32. All sharding / gathering happens here.
"""

import math
import sys

import numpy as np

for _p in ("/opt/trn_rl_repo", "/root/.axon_site/_ro/trn_rl_repo"):
    if _p not in sys.path:
        sys.path.insert(0, _p)

HIDDEN = 4096
N_HEADS = 32
HEAD_DIM = 128
BASE = 10000.0
B = 2
S = 2048
HEADS_PER_CORE = 8          # 32 heads / 4 groups
HG = 1024                   # head-group width = 8 heads * 128
NEG_BIG = -1.0e9

# RoPE partner permutation: quadrant q holds [lo_d 16q..16q+15, hi_d 64+16q..]
# so the rotate-half partner of new-row i is i+-16 inside its 32-row quadrant,
# reachable by DVE stream_shuffle.
PERM = np.zeros(128, dtype=np.int64)
for _q in range(4):
    PERM[32 * _q: 32 * _q + 16] = np.arange(16 * _q, 16 * _q + 16)
    PERM[32 * _q + 16: 32 * _q + 32] = 64 + np.arange(16 * _q, 16 * _q + 16)
SHUF_MASK = [(i + 16) % 32 for i in range(32)]
# sign of the sin term per (new) row: -1 where original d < 64
SIGN = np.where(PERM < 64, -1.0, 1.0).astype(np.float32)


def _rope_tables(max_pos):
    inv_freq = 1.0 / (BASE ** (np.arange(0, HEAD_DIM, 2, dtype=np.float32) / HEAD_DIM))
    t = np.arange(max_pos, dtype=np.float32)
    freqs = np.outer(t, inv_freq)                      # [P, 64]
    emb = np.concatenate((freqs, freqs), axis=-1)      # [P, 128]
    return np.cos(emb).astype(np.float32), np.sin(emb).astype(np.float32)


def _build_program(mask_mode):
    """mask_mode: 'causal' (block-skip + shared triangle mask),
    'none' (dense, no mask), 'full' (dense, stream mask tiles)."""
    import concourse.bacc as bacc
    import concourse.mybir as mybir
    import concourse.tile as tile
    from contextlib import ExitStack

    F32 = mybir.dt.float32
    BF16 = mybir.dt.bfloat16
    ALU = mybir.AluOpType
    ACTF = mybir.ActivationFunctionType

    nc = bacc.Bacc("TRN2", target_bir_lowering=False, debug=False)

    # pre-tiled inputs (host side does all layout work)
    x_t = nc.declare_dram_parameter("x_t", [128, 32, S], BF16, isOutput=False)
    wqk_t = nc.declare_dram_parameter("wqk_t", [16, 128, 32, 128], BF16,
                                      isOutput=False)
    wv_t = nc.declare_dram_parameter("wv_t", [2, 32, 128, 512], BF16,
                                     isOutput=False)
    wo_t = nc.declare_dram_parameter("wo_t", [128, 8, HIDDEN], BF16,
                                     isOutput=False)
    cosT = nc.declare_dram_parameter("cosT", [128, S], BF16, isOutput=False)
    sinT = nc.declare_dram_parameter("sinT", [128, S], BF16, isOutput=False)
    if mask_mode == "causal":
        tri = nc.declare_dram_parameter("tri", [128, 128], F32, isOutput=False)
        triT = nc.declare_dram_parameter("triT", [128, 128], BF16,
                                         isOutput=False)
        idm = nc.declare_dram_parameter("idm", [128, 128], BF16,
                                        isOutput=False)
    elif mask_mode == "full":
        maskT = nc.declare_dram_parameter("maskT", [S, S], F32, isOutput=False)
    out_p = nc.declare_dram_parameter("out_p", [S, HIDDEN], BF16, isOutput=True)

    at_s = nc.dram_tensor("at_scratch", [HG, S], BF16)

    inv_sqrt_d = 1.0 / math.sqrt(HEAD_DIM)

    with tile.TileContext(nc, pool_alloc_mode="queue") as tc, ExitStack() as top:
        const_pool = top.enter_context(tc.tile_pool(name="consts", bufs=1))
        ones_f32 = const_pool.tile([128, 128], F32)
        nc.vector.memset(ones_f32, 1.0)
        ones_sq = const_pool.tile([128, 128], BF16)
        nc.vector.tensor_copy(ones_sq, ones_f32)
        if mask_mode == "causal":
            tri_sb = const_pool.tile([128, 128], F32)
            nc.sync.dma_start(out=tri_sb, in_=tri.ap())
            triT_sb = const_pool.tile([128, 128], BF16)
            nc.sync.dma_start(out=triT_sb, in_=triT.ap())
            id_sb = const_pool.tile([128, 128], BF16)
            nc.sync.dma_start(out=id_sb, in_=idm.ap())

        # SBUF-resident q/k/v; q/k split per head so attention units only
        # wait on the head they read (dep tracking is per-tile)
        res_pool = top.enter_context(tc.tile_pool(name="resident", bufs=1))
        qT_sb = [[res_pool.tile([128, 1024], BF16, name=f"qT{h}_{hh}")
                  for hh in range(HEADS_PER_CORE)] for h in range(2)]
        kT_sb = [[res_pool.tile([128, 1024], BF16, name=f"kT{h}_{hh}")
                  for hh in range(HEADS_PER_CORE)] for h in range(2)]
        v_sb = [res_pool.tile([128, 8, HG], BF16, name=f"v{h}")
                for h in range(2)]

        # ---------------- Phase A: V then QK projection per half -------------
        def emit_half(hs, pha, attn_setup=None):
            s0 = hs * 1024
            # h-chunk sweep order: B-block (16..31) first (its DMA is issued
            # first), A-block (0..15) last.
            H_ORDER = list(range(16, 32)) + list(range(16))
            xpoolA = pha.enter_context(tc.tile_pool(name="xhalfA", bufs=1))
            xpoolB = pha.enter_context(tc.tile_pool(name="xhalfB", bufs=1))
            xtA = xpoolA.tile([128, 16, 1024], BF16, name=f"xtA{hs}")
            xtB = xpoolB.tile([128, 16, 1024], BF16, name=f"xtB{hs}")
            xin = x_t.ap()[:, :, s0:s0 + 1024]

            def emit_x_dma(c):
                dst = xtB[:, c - 16, :] if c >= 16 else xtA[:, c, :]
                nc.sync.dma_start(out=dst, in_=xin[:, c, :])

            def xt_slice(c, sl):
                return xtB[:, c - 16, sl] if c >= 16 else xtA[:, c, sl]

            # --- v projection first, x-stationary, out = v [s, o] ------------
            # x-chunk DMAs are interleaved with the wv tile loads in
            # consumption order so the first pass streams at DMA rate
            with ExitStack() as vv:
                wvp = vv.enter_context(tc.tile_pool(name="wvt", bufs=4))
                pv = vv.enter_context(
                    tc.tile_pool(name="pv", bufs=8, space="PSUM"))
                for ov in range(2):          # v-dim chunks of 512
                    vb = [pv.tile([128, 512], F32, tag="vb", name=f"vb{i}")
                          for i in range(8)]
                    for hi, h in enumerate(H_ORDER):
                        wv_tile = wvp.tile([128, 512], BF16, tag="wv_tile")
                        nc.sync.dma_start(out=wv_tile, in_=wv_t.ap()[ov, h])
                        if ov == 0:
                            emit_x_dma(h)
                        for sc in range(8):
                            nc.tensor.matmul(
                                vb[sc],
                                xt_slice(h, slice(sc * 128, (sc + 1) * 128)),
                                wv_tile,
                                start=(hi == 0), stop=(hi == 31))
                    for sc in range(8):
                        dst = v_sb[hs][:, sc, ov * 512:(ov + 1) * 512]
                        if sc % 2 == 0:
                            nc.scalar.activation(dst, vb[sc], ACTF.Copy)
                        else:
                            nc.vector.tensor_copy(dst, vb[sc])

            # --- q,k projection, weight-stationary, out = projT [o, s] -------
            with ExitStack() as qk:
                cspool = qk.enter_context(tc.tile_pool(name="cossin", bufs=1))
                cos_sb = cspool.tile([128, 1024], BF16)
                nc.sync.dma_start(out=cos_sb, in_=cosT.ap()[:, s0:s0 + 1024])
                sin_sb = cspool.tile([128, 1024], BF16)
                nc.sync.dma_start(out=sin_sb, in_=sinT.ap()[:, s0:s0 + 1024])

                wpool = qk.enter_context(tc.tile_pool(name="wqk", bufs=2))
                pqk = qk.enter_context(
                    tc.tile_pool(name="pqk", bufs=2, space="PSUM"))
                rpool = qk.enter_context(tc.tile_pool(name="rope", bufs=3))
                hook = post_pair = finish = None
                if attn_setup is not None:
                    hook, post_pair, finish = attn_setup(qk)
                # q/k oc pairs back-to-back so head p's attention can weave
                # into the remaining projection stream
                ocs = ([x for p in range(8) for x in (p, p + 8)]
                       if attn_setup else list(range(16)))
                for oci, oc in enumerate(ocs):  # o chunks of 128 (head tiles)
                    w_oc = wpool.tile([128, 32, 128], BF16, tag="w_oc")
                    nc.sync.dma_start(out=w_oc, in_=wqk_t.ap()[oc])
                    pk = pqk.tile([128, 2, 512], F32, tag="pk")
                    for hi, h in enumerate(H_ORDER):
                        for sc in range(2):
                            nc.tensor.matmul(
                                pk[:, sc, :], w_oc[:, h, :],
                                xt_slice(h, slice(sc * 512, (sc + 1) * 512)),
                                start=(hi == 0), stop=(hi == 31))
                        if hook is not None and hi % 4 == 3:
                            hook()
                    # RoPE: q' = q*cos + shuffle16(q)*sin_signed, -> bf16
                    dst = (qT_sb[hs] if oc < 8 else kT_sb[hs])[oc % 8]
                    for sc in range(2):
                        pks = pk[:, sc, :]
                        cs = cos_sb[:, sc * 512:(sc + 1) * 512]
                        sn = sin_sb[:, sc * 512:(sc + 1) * 512]
                        qrot = rpool.tile([128, 512], F32, tag="r", name="qrot")
                        nc.vector.stream_shuffle(qrot, pks, SHUF_MASK)
                        t1 = rpool.tile([128, 512], F32, tag="r", name="t1")
                        nc.vector.tensor_tensor(t1, pks, cs, ALU.mult)
                        t2 = rpool.tile([128, 512], F32, tag="r", name="t2")
                        nc.gpsimd.tensor_tensor(t2, qrot, sn, ALU.mult)
                        nc.vector.tensor_tensor(
                            dst[:, sc * 512:(sc + 1) * 512],
                            t1, t2, ALU.add)
                    if post_pair is not None and oci % 2 == 1:
                        post_pair(oci // 2)
                if finish is not None:
                    finish()

        # ---------------- Phase B: attention, scores kept as S^T [k, q] ------
        # Scores go into [128, 2, 512] PSUM tiles (2 k-blocks per group),
        # exp'd in one ACTIVATE per off-diagonal group, accumulated into a
        # running DVE sum, folded to [128, 512], and reduced+broadcast by a
        # single all-ones stationary matmul (den lands replicated on every
        # PSUM partition). av matmuls trail their exp by two groups; the
        # finalize (recip + av*recip + DMA out) is deferred by one unit.
        def make_attn_emitter(phb, tri_pe=False, weave=False):
            # tri_pe mode is used when woven into a projection phase: the
            # triangle mask is applied by a tiny PE matmul (identity moving,
            # tri^T stationary, accumulate) so the score->exp chain never
            # hops through the RoPE-congested DVE queue, and PSUM shrinks to
            # ps(2 banks) + shared av/den(2 banks) beside pqk's 4.
            qp_es = phb.enter_context(
                tc.tile_pool(name="es", bufs=2 if weave else 4))
            esump = phb.enter_context(tc.tile_pool(name="esum", bufs=2))
            smallp = phb.enter_context(tc.tile_pool(name="small", bufs=2))
            ps = phb.enter_context(
                tc.tile_pool(name="ps", bufs=1 if weave else 2,
                             space="PSUM"))
            if weave:
                pav = pmisc = phb.enter_context(
                    tc.tile_pool(name="pavd", bufs=2, space="PSUM"))
                av_tag = den_tag = "avd"
            else:
                pav = phb.enter_context(
                    tc.tile_pool(name="pav", bufs=2, space="PSUM"))
                pmisc = phb.enter_context(
                    tc.tile_pool(name="pmisc", bufs=2, space="PSUM"))
                av_tag, den_tag = "av", "misc"
            mp = None
            if mask_mode == "full":
                mp = phb.enter_context(tc.tile_pool(name="msk", bufs=3))

            state = {"pending": None, "pend_den": None}

            def emit_den():
                # den matmul for the previous unit, deferred so the next
                # unit's score matmuls cover the exp->tree->fold latency
                if state["pend_den"] is None:
                    return
                fold, den = state["pend_den"]
                nc.tensor.matmul(den, ones_sq, fold, start=True, stop=True)
                state["pend_den"] = None

            def finalize(av, den, hh, qc):
                recip = smallp.tile([128, 512], F32, tag="recip")
                nc.vector.reciprocal_approx_fast(recip, den)
                at_t = smallp.tile([128, 512], BF16, tag="at_t")
                nc.vector.tensor_tensor(at_t, av, recip, ALU.mult)
                nc.sync.dma_start(
                    out=at_s.ap()[hh * 128:(hh + 1) * 128,
                                  qc * 512:(qc + 1) * 512],
                    in_=at_t)

            def unit_slices(qc, hh):
                # generator: yields once per score group so the caller can
                # interleave foreign PE work between the slices
                emit_den()
                if state["pending"] is not None:
                    finalize(*state["pending"])
                    state["pending"] = None
                nblk = 4 * qc + 4 if mask_mode == "causal" else 16
                ng = nblk // 2
                av = pav.tile([128, 512], F32, tag=av_tag, name="av")
                acc = None
                navs = [0]
                pend_av = []          # (es_tile, j2, kb, q_lo) awaiting av mm

                def emit_avs(upto):
                    while pend_av and len(pend_av) > upto:
                        es, j2, kb, q_lo = pend_av.pop(0)
                        qs = slice(q_lo, 512)
                        khalf, kloc = kb // 8, kb % 8
                        nc.tensor.matmul(
                            av[:, qs],
                            v_sb[khalf][:, kloc, hh * 128:(hh + 1) * 128],
                            es[:, j2, qs],
                            start=(navs[0] == 0), stop=(navs[0] == nblk - 1))
                        navs[0] += 1

                # diagonal groups first: their serial per-region exps overlap
                # the off-diagonal work that follows. The vd=0 block is the
                # first av emitted and covers the full [0:512] PSUM range, so
                # the accumulation start flag is sound.
                if mask_mode == "causal":
                    order = [2 * qc, 2 * qc + 1] + list(range(2 * qc))
                else:
                    order = list(range(ng))
                for gi, g in enumerate(order):
                    es = qp_es.tile([128, 2, 512], BF16, tag="es", name="es")
                    sps = ps.tile([128, 2, 512], F32, tag="sps")
                    diag = False
                    blk = []
                    for j2 in range(2):
                        kb = 2 * g + j2
                        vd = kb - 4 * qc   # diagonal block index
                        q_lo = (128 * vd
                                if (mask_mode == "causal" and vd > 0)
                                else 0)
                        qs = slice(q_lo, 512)
                        khalf, kloc = kb // 8, kb % 8
                        is_d = mask_mode == "causal" and vd >= 0
                        nc.tensor.matmul(
                            sps[:, j2, qs],
                            kT_sb[khalf][hh][:, kloc * 128:(kloc + 1) * 128],
                            qT_sb[qc // 2][hh][:, (qc % 2) * 512 + q_lo:
                                               (qc % 2) * 512 + 512],
                            start=True, stop=not (is_d and tri_pe))
                        if is_d:
                            diag = True
                            mq = slice(128 * vd, 128 * vd + 128)
                            if tri_pe:
                                nc.tensor.matmul(
                                    sps[:, j2, mq], triT_sb, id_sb,
                                    start=False, stop=True)
                            else:
                                nc.vector.tensor_tensor(
                                    sps[:, j2, mq], sps[:, j2, mq], tri_sb,
                                    ALU.add)
                        elif mask_mode == "full":
                            mt = mp.tile([128, 512], F32, tag="mt")
                            nc.sync.dma_start(
                                out=mt,
                                in_=maskT.ap()[kb * 128:(kb + 1) * 128,
                                               qc * 512:(qc + 1) * 512])
                            nc.vector.tensor_tensor(sps[:, j2, :],
                                                    sps[:, j2, :], mt,
                                                    ALU.add)
                        blk.append((kb, q_lo))
                    # exp: one ACTIVATE per clean group; per-region on the
                    # diagonal (unwritten PSUM slivers stay out of the AP)
                    if not diag:
                        nc.scalar.activation(es, sps, ACTF.Exp,
                                             scale=inv_sqrt_d)
                    else:
                        for j2, (kb, q_lo) in enumerate(blk):
                            if q_lo > 0:
                                nc.vector.memset(es[:, j2, 0:q_lo], 0.0)
                            qs = slice(q_lo, 512)
                            nc.scalar.activation(es[:, j2, qs],
                                                 sps[:, j2, qs],
                                                 ACTF.Exp, scale=inv_sqrt_d)
                    # incremental tree: acc += es (DVE), frees es early
                    if gi == 0:
                        acc = es
                    else:
                        if gi == 1:
                            t = esump.tile([128, 2, 512], BF16, tag="e2")
                            nc.vector.tensor_tensor(t, acc, es, ALU.add)
                            acc = t
                        else:
                            nc.vector.tensor_tensor(acc, acc, es, ALU.add)
                    for j2, (kb, q_lo) in enumerate(blk):
                        pend_av.append((es, j2, kb, q_lo))
                    # av matmuls trail their exp by one/two groups
                    emit_avs(2 if weave else 4)
                    yield
                emit_avs(0)
                # fold the two k-block columns -> [128, 512]
                fold = smallp.tile([128, 512], BF16, tag="fold")
                nc.vector.tensor_tensor(fold, acc[:, 0, :], acc[:, 1, :],
                                        ALU.add)
                # den replicated across all 128 partitions via all-ones lhsT;
                # the matmul itself is deferred into the next unit
                den = pmisc.tile([128, 512], F32, tag=den_tag, name="den")
                state["pend_den"] = (fold, den)
                state["pending"] = (av, den, hh, qc)

            def emit_unit(qc, hh):
                for _ in unit_slices(qc, hh):
                    pass

            def flush():
                emit_den()
                if state["pending"] is not None:
                    finalize(*state["pending"])
                    state["pending"] = None

            return emit_unit, flush, pmisc, unit_slices

        # ---------------- Phase C: output projection -------------------------
        def make_c_emitter(phc, pop):
            atp = phc.enter_context(tc.tile_pool(name="atl", bufs=3))
            otp = phc.enter_context(tc.tile_pool(name="ot", bufs=4))
            wop = phc.enter_context(tc.tile_pool(name="wo", bufs=3))
            state = {"at_g": {}, "ot": {}, "wo_sl": None}

            def load_stg(stg):
                at_g = atp.tile([128, 8, 512], BF16, tag="at_g",
                                name=f"at_g{stg}")
                nc.sync.dma_start(
                    out=at_g,
                    in_=at_s.ap()[:, stg * 512:(stg + 1) * 512].rearrange(
                        "(hc p) s -> p hc s", p=128))
                state["at_g"][stg] = at_g

            def emit_unit(stg, st_l, o8):
                # one PSUM bank: out[st, o8] = sum_hc at^T wo
                # o8-major within a stage: wo slice loads once per (stg, o8),
                # the four ot tiles of the stage stay live until its end.
                at_g = state["at_g"][stg]
                st = stg * 4 + st_l
                sl = slice(st_l * 128, (st_l + 1) * 128)
                if st_l == 0:
                    wo_sl = wop.tile([128, 8, 512], BF16, tag="wo_sl",
                                     name=f"wo{stg}_{o8}")
                    nc.sync.dma_start(
                        out=wo_sl,
                        in_=wo_t.ap()[:, :, o8 * 512:(o8 + 1) * 512])
                    state["wo_sl"] = wo_sl
                wo_sl = state["wo_sl"]
                if o8 == 0:
                    state["ot"][st] = otp.tile([128, 8, 512], BF16, tag="ot",
                                               name=f"ot{st}")
                ot = state["ot"][st]
                po = pop.tile([128, 512], F32, tag="misc", name="po")
                for hc in range(8):
                    nc.tensor.matmul(
                        po,
                        at_g[:, hc, sl],
                        wo_sl[:, hc, :],
                        start=(hc == 0), stop=(hc == 7))
                nc.scalar.activation(ot[:, o8, :], po, ACTF.Copy)
                if o8 == 7:
                    nc.sync.dma_start(
                        out=out_p.ap()[st * 128:(st + 1) * 128, :],
                        in_=ot.rearrange("p a b -> p (a b)"))
                    del state["ot"][st]

            return emit_unit, load_stg

        # ================= schedule =================
        from collections import deque

        if mask_mode == "causal":
            # half 0 with B01 (qc0, qc1) woven into the QK oc-pair stream
            def attn_setup0(qk_stack):
                _, flush_b0, _, gen = make_attn_emitter(qk_stack, tri_pe=True,
                                                        weave=True)
                pending = deque()

                def hook():
                    while pending:
                        try:
                            next(pending[0])
                            return
                        except StopIteration:
                            pending.popleft()

                def post_pair(p):
                    pending.append(gen(0, p))
                    pending.append(gen(1, p))

                def finish():
                    while pending:
                        try:
                            next(pending[0])
                        except StopIteration:
                            pending.popleft()
                    flush_b0()

                return hook, post_pair, finish

            with ExitStack() as ph0:
                emit_half(0, ph0, attn_setup0)

            with ExitStack() as ph1:
                emit_half(1, ph1)

            # B23 with C(stg0, stg1) interleaved
            with ExitStack() as phbc:
                emit_unit_b, flush_b, pmisc, _ = make_attn_emitter(
                    phbc, tri_pe=True)
                emit_unit_c, load_stg = make_c_emitter(phbc, pmisc)
                load_stg(0)
                load_stg(1)
                c_units = [(stg, st_l, o8)
                           for stg in (0, 1)
                           for o8 in range(8)
                           for st_l in range(4)]
                for qc in (2, 3):
                    for hh in range(HEADS_PER_CORE):
                        emit_unit_b(qc, hh)
                        if qc == 3 and hh == 0:
                            # at_s rows for stg2 are complete once the
                            # (qc2, hh7) finalize ran inside this unit
                            load_stg(2)
                        # ~4 C units per B unit balances the two streams
                        for _ in range(4):
                            if c_units:
                                emit_unit_c(*c_units.pop(0))
                flush_b()
                load_stg(3)
                while c_units:
                    emit_unit_c(*c_units.pop(0))

                # C(stg2, stg3) dense
                for stg in (2, 3):
                    for o8 in range(8):
                        for st_l in range(4):
                            emit_unit_c(stg, st_l, o8)
        else:
            with ExitStack() as ph0:
                emit_half(0, ph0)
            with ExitStack() as ph1:
                emit_half(1, ph1)
            with ExitStack() as phbc:
                emit_unit_b, flush_b, pmisc, _ = make_attn_emitter(phbc)
                emit_unit_c, load_stg = make_c_emitter(phbc, pmisc)
                for qc in range(4):
                    for hh in range(HEADS_PER_CORE):
                        emit_unit_b(qc, hh)
                # all units emitted; flush and drain C for every stg in order
                flush_b()
                for stg in range(4):
                    load_stg(stg)
                    for o8 in range(8):
                        for st_l in range(4):
                            emit_unit_c(stg, st_l, o8)

    nc.compile()
    return nc


_PROGRAM_CACHE = {}


def _get_program(mask_mode):
    if mask_mode not in _PROGRAM_CACHE:
        _PROGRAM_CACHE[mask_mode] = _build_program(mask_mode)
    return _PROGRAM_CACHE[mask_mode]


def _classify_mask(attention_mask):
    m = np.asarray(attention_mask)
    if not np.any(m):
        return "none"
    neg = np.float32(np.finfo(np.float32).min)
    causal = np.triu(np.full((S, S), neg, dtype=np.float32), k=1)
    for b in range(m.shape[0]):
        if not np.array_equal(m[b, 0], causal):
            return "full"
    return "causal"


def _prep_core_inputs(hidden_states, attention_mask, position_ids, W_pack, W_o,
                      mask_mode):
    from ml_dtypes import bfloat16

    hidden_states = np.asarray(hidden_states, dtype=np.float32)
    W_pack = np.asarray(W_pack, dtype=np.float32)
    W_o = np.asarray(W_o, dtype=np.float32)
    pos = np.asarray(position_ids).astype(np.int64)

    cos_t, sin_t = _rope_tables(int(pos.max()) + 1)
    # per-batch gathered + transposed + row-permuted (+ sign folded into sin)
    cosT_b, sinT_b = [], []
    for b in range(B):
        c = cos_t[pos[b]][:, PERM].T
        s = (sin_t[pos[b]][:, PERM] * SIGN[None, :]).T
        cosT_b.append(np.ascontiguousarray(c.astype(bfloat16)))
        sinT_b.append(np.ascontiguousarray(s.astype(bfloat16)))

    # x_t[p, c, s] = hidden[b, s, c*128+p]
    x_b = [np.ascontiguousarray(
        hidden_states[b].T.reshape(32, 128, S).transpose(1, 0, 2)
        .astype(bfloat16)) for b in range(B)]

    tri_m = None
    triT_m = idm_m = None
    maskT_b = None
    if mask_mode == "causal":
        kk = np.arange(128)[:, None]
        qq = np.arange(128)[None, :]
        tri_m = np.where(kk <= qq, 0.0, NEG_BIG).astype(np.float32)
        triT_m = np.ascontiguousarray(tri_m.T.astype(bfloat16))
        idm_m = np.ascontiguousarray(np.eye(128, dtype=np.float32)
                                     .astype(bfloat16))
    elif mask_mode == "full":
        m = np.asarray(attention_mask, dtype=np.float32)
        maskT_b = [np.ascontiguousarray(m[b, 0].T) for b in range(B)]

    in_maps = []
    for cidx in range(8):
        b, g = cidx // 4, cidx % 4
        # per-head d-permuted q/k weight rows, head-major columns in wqk
        qrows = np.concatenate(
            [g * HG + hh * 128 + PERM for hh in range(HEADS_PER_CORE)])
        krows = HIDDEN + qrows
        vrows = 2 * HIDDEN + g * HG + np.arange(HG)
        wqk = np.concatenate([W_pack[qrows], W_pack[krows]], axis=0)  # [2048,4096]
        # wqk_t[oc, p, c, o] = wqk[oc*128+o, c*128+p]
        wqk_t = np.ascontiguousarray(
            wqk.reshape(16, 128, 32, 128).transpose(0, 3, 2, 1)
            .astype(bfloat16))
        wv = W_pack[vrows]                                            # [1024,4096]
        # wv_t[ov, c, p, o] = wv[ov*512+o, c*128+p]
        wv_t = np.ascontiguousarray(
            wv.reshape(2, 512, 32, 128).transpose(0, 2, 3, 1).astype(bfloat16))
        # wo_t[p, hc, o] = W_o[o, g*HG + hc*128 + p]
        wo_t = np.ascontiguousarray(
            W_o[:, g * HG:(g + 1) * HG].reshape(HIDDEN, 8, 128)
            .transpose(2, 1, 0).astype(bfloat16))
        im = {"x_t": x_b[b], "wqk_t": wqk_t, "wv_t": wv_t, "wo_t": wo_t,
              "cosT": cosT_b[b], "sinT": sinT_b[b]}
        if mask_mode == "causal":
            im["tri"] = tri_m
            im["triT"] = triT_m
            im["idm"] = idm_m
        elif mask_mode == "full":
            im["maskT"] = maskT_b[b]
        in_maps.append(im)
    return in_maps


def _run(hidden_states, attention_mask, position_ids, W_pack, W_o,
         trace=False, trace_kwargs=None):
    from concourse.bass_utils import run_bass_kernel_spmd

    mask_mode = _classify_mask(attention_mask)
    nc = _get_program(mask_mode)
    in_maps = _prep_core_inputs(hidden_states, attention_mask, position_ids,
                                W_pack, W_o, mask_mode)
    try:
        res = run_bass_kernel_spmd(nc, in_maps, list(range(8)), trace=trace,
                                   **(trace_kwargs or {}))
    except Exception:
        # transient NRT_EXEC_UNIT_UNRECOVERABLE wedges recover on retry
        import time as _time
        _time.sleep(15)
        res = run_bass_kernel_spmd(nc, in_maps, list(range(8)), trace=trace,
                                   **(trace_kwargs or {}))
    out = np.zeros((B, S, HIDDEN), dtype=np.float32)
    for c in range(8):
        out[c // 4] += np.asarray(res.results[c]["out_p"], dtype=np.float32)
    return out, res


def kernel(hidden_states, attention_mask, position_ids, W_pack, W_o):
    out, _ = _run(hidden_states, attention_mask, position_ids, W_pack, W_o)
    return out


# revision 43
# speedup vs baseline: 1.0985x; 1.0006x over previous
"""Trainium2 Bass kernel for a single attention layer (Baichuan-style W_pack
attention with rotary embeddings), sharded over 8 NeuronCores:
tensor-parallel over 4 head groups x data-parallel over 2 batches.

v3: scheduling overhaul toward the bf16 PE stream floor (~1.01 ms):
 - V-proj runs before QK-proj in each half so the x DMA stream is consumed
   at arrival rate (kills the ~20 us x-wait stall at each half start).
 - softmax denominator: single all-ones [128,128] stationary matmul whose
   PSUM output is den replicated across all partitions (no [1,512] recip
   chain, no separate broadcast matmul), fed by an incremental DVE tree +
   fold so there is one den matmul per (head, q-chunk).
 - exp batched per 2 score blocks (one ACTIVATE over [128,1024]) off the
   diagonal; av matmuls trail their exp by two groups so the PE never
   waits on the ACT exp.
 - O-proj work for seq groups 0,1 is interleaved into the qc2/qc3
   attention emission to fill residual exp-chain bubbles.

Contract: kernel(**inputs) takes the FULL unsharded inputs and returns the
FULL output [2, 2048, 4096] float32. All sharding / gathering happens here.
"""

import math
import sys

import numpy as np

for _p in ("/opt/trn_rl_repo", "/root/.axon_site/_ro/trn_rl_repo"):
    if _p not in sys.path:
        sys.path.insert(0, _p)

HIDDEN = 4096
N_HEADS = 32
HEAD_DIM = 128
BASE = 10000.0
B = 2
S = 2048
HEADS_PER_CORE = 8          # 32 heads / 4 groups
HG = 1024                   # head-group width = 8 heads * 128
NEG_BIG = -1.0e9

# RoPE partner permutation: quadrant q holds [lo_d 16q..16q+15, hi_d 64+16q..]
# so the rotate-half partner of new-row i is i+-16 inside its 32-row quadrant,
# reachable by DVE stream_shuffle.
PERM = np.zeros(128, dtype=np.int64)
for _q in range(4):
    PERM[32 * _q: 32 * _q + 16] = np.arange(16 * _q, 16 * _q + 16)
    PERM[32 * _q + 16: 32 * _q + 32] = 64 + np.arange(16 * _q, 16 * _q + 16)
SHUF_MASK = [(i + 16) % 32 for i in range(32)]
# sign of the sin term per (new) row: -1 where original d < 64
SIGN = np.where(PERM < 64, -1.0, 1.0).astype(np.float32)


def _rope_tables(max_pos):
    inv_freq = 1.0 / (BASE ** (np.arange(0, HEAD_DIM, 2, dtype=np.float32) / HEAD_DIM))
    t = np.arange(max_pos, dtype=np.float32)
    freqs = np.outer(t, inv_freq)                      # [P, 64]
    emb = np.concatenate((freqs, freqs), axis=-1)      # [P, 128]
    return np.cos(emb).astype(np.float32), np.sin(emb).astype(np.float32)


def _build_program(mask_mode):
    """mask_mode: 'causal' (block-skip + shared triangle mask),
    'none' (dense, no mask), 'full' (dense, stream mask tiles)."""
    import concourse.bacc as bacc
    import concourse.mybir as mybir
    import concourse.tile as tile
    from contextlib import ExitStack

    F32 = mybir.dt.float32
    BF16 = mybir.dt.bfloat16
    ALU = mybir.AluOpType
    ACTF = mybir.ActivationFunctionType

    nc = bacc.Bacc("TRN2", target_bir_lowering=False, debug=False)

    # pre-tiled inputs (host side does all layout work)
    x_t = nc.declare_dram_parameter("x_t", [128, 32, S], BF16, isOutput=False)
    wqk_t = nc.declare_dram_parameter("wqk_t", [16, 128, 32, 128], BF16,
                                      isOutput=False)
    wv_t = nc.declare_dram_parameter("wv_t", [2, 32, 128, 512], BF16,
                                     isOutput=False)
    wo_t = nc.declare_dram_parameter("wo_t", [128, 8, HIDDEN], BF16,
                                     isOutput=False)
    cosT = nc.declare_dram_parameter("cosT", [128, S], BF16, isOutput=False)
    sinT = nc.declare_dram_parameter("sinT", [128, S], BF16, isOutput=False)
    if mask_mode == "causal":
        tri = nc.declare_dram_parameter("tri", [128, 128], F32, isOutput=False)
        triT = nc.declare_dram_parameter("triT", [128, 128], BF16,
                                         isOutput=False)
        idm = nc.declare_dram_parameter("idm", [128, 128], BF16,
                                        isOutput=False)
    elif mask_mode == "full":
        maskT = nc.declare_dram_parameter("maskT", [S, S], F32, isOutput=False)
    out_p = nc.declare_dram_parameter("out_p", [S, HIDDEN], BF16, isOutput=True)

    at_s = nc.dram_tensor("at_scratch", [HG, S], BF16)

    inv_sqrt_d = 1.0 / math.sqrt(HEAD_DIM)

    with tile.TileContext(nc, pool_alloc_mode="queue") as tc, ExitStack() as top:
        const_pool = top.enter_context(tc.tile_pool(name="consts", bufs=1))
        ones_f32 = const_pool.tile([128, 128], F32)
        nc.vector.memset(ones_f32, 1.0)
        ones_sq = const_pool.tile([128, 128], BF16)
        nc.vector.tensor_copy(ones_sq, ones_f32)
        if mask_mode == "causal":
            tri_sb = const_pool.tile([128, 128], F32)
            nc.sync.dma_start(out=tri_sb, in_=tri.ap())
            triT_sb = const_pool.tile([128, 128], BF16)
            nc.sync.dma_start(out=triT_sb, in_=triT.ap())
            id_sb = const_pool.tile([128, 128], BF16)
            nc.sync.dma_start(out=id_sb, in_=idm.ap())

        # SBUF-resident q/k/v; q/k split per head so attention units only
        # wait on the head they read (dep tracking is per-tile)
        res_pool = top.enter_context(tc.tile_pool(name="resident", bufs=1))
        qT_sb = [[res_pool.tile([128, 1024], BF16, name=f"qT{h}_{hh}")
                  for hh in range(HEADS_PER_CORE)] for h in range(2)]
        kT_sb = [[res_pool.tile([128, 1024], BF16, name=f"kT{h}_{hh}")
                  for hh in range(HEADS_PER_CORE)] for h in range(2)]
        v_sb = [res_pool.tile([128, 8, HG], BF16, name=f"v{h}")
                for h in range(2)]

        # ---------------- Phase A: V then QK projection per half -------------
        def emit_half(hs, pha, attn_setup=None):
            s0 = hs * 1024
            # h-chunk sweep order: B-block (16..31) first (its DMA is issued
            # first), A-block (0..15) last.
            H_ORDER = list(range(16, 32)) + list(range(16))
            xpoolA = pha.enter_context(tc.tile_pool(name="xhalfA", bufs=1))
            xpoolB = pha.enter_context(tc.tile_pool(name="xhalfB", bufs=1))
            xtA = xpoolA.tile([128, 16, 1024], BF16, name=f"xtA{hs}")
            xtB = xpoolB.tile([128, 16, 1024], BF16, name=f"xtB{hs}")
            xin = x_t.ap()[:, :, s0:s0 + 1024]

            def emit_x_dma(c):
                dst = xtB[:, c - 16, :] if c >= 16 else xtA[:, c, :]
                nc.sync.dma_start(out=dst, in_=xin[:, c, :])

            def xt_slice(c, sl):
                return xtB[:, c - 16, sl] if c >= 16 else xtA[:, c, sl]

            # --- v projection first, x-stationary, out = v [s, o] ------------
            # x-chunk DMAs are interleaved with the wv tile loads in
            # consumption order so the first pass streams at DMA rate
            with ExitStack() as vv:
                wvp = vv.enter_context(tc.tile_pool(name="wvt", bufs=4))
                pv = vv.enter_context(
                    tc.tile_pool(name="pv", bufs=8, space="PSUM"))
                for ov in range(2):          # v-dim chunks of 512
                    vb = [pv.tile([128, 512], F32, tag="vb", name=f"vb{i}")
                          for i in range(8)]
                    for hi, h in enumerate(H_ORDER):
                        wv_tile = wvp.tile([128, 512], BF16, tag="wv_tile")
                        nc.sync.dma_start(out=wv_tile, in_=wv_t.ap()[ov, h])
                        if ov == 0:
                            emit_x_dma(h)
                        for sc in range(8):
                            nc.tensor.matmul(
                                vb[sc],
                                xt_slice(h, slice(sc * 128, (sc + 1) * 128)),
                                wv_tile,
                                start=(hi == 0), stop=(hi == 31))
                    for sc in range(8):
                        dst = v_sb[hs][:, sc, ov * 512:(ov + 1) * 512]
                        if sc % 2 == 0:
                            nc.scalar.activation(dst, vb[sc], ACTF.Copy)
                        else:
                            nc.vector.tensor_copy(dst, vb[sc])

            # --- q,k projection, weight-stationary, out = projT [o, s] -------
            with ExitStack() as qk:
                cspool = qk.enter_context(tc.tile_pool(name="cossin", bufs=1))
                cos_sb = cspool.tile([128, 1024], BF16)
                nc.sync.dma_start(out=cos_sb, in_=cosT.ap()[:, s0:s0 + 1024])
                sin_sb = cspool.tile([128, 1024], BF16)
                nc.sync.dma_start(out=sin_sb, in_=sinT.ap()[:, s0:s0 + 1024])

                wpool = qk.enter_context(tc.tile_pool(name="wqk", bufs=2))
                pqk = qk.enter_context(
                    tc.tile_pool(name="pqk", bufs=2, space="PSUM"))
                rpool = qk.enter_context(tc.tile_pool(name="rope", bufs=3))
                hook = post_pair = finish = None
                if attn_setup is not None:
                    hook, post_pair, finish = attn_setup(qk)
                # q/k oc pairs back-to-back so head p's attention can weave
                # into the remaining projection stream
                ocs = ([x for p in range(8) for x in (p, p + 8)]
                       if attn_setup else list(range(16)))
                for oci, oc in enumerate(ocs):  # o chunks of 128 (head tiles)
                    w_oc = wpool.tile([128, 32, 128], BF16, tag="w_oc")
                    nc.sync.dma_start(out=w_oc, in_=wqk_t.ap()[oc])
                    pk = pqk.tile([128, 2, 512], F32, tag="pk")
                    for hi, h in enumerate(H_ORDER):
                        for sc in range(2):
                            nc.tensor.matmul(
                                pk[:, sc, :], w_oc[:, h, :],
                                xt_slice(h, slice(sc * 512, (sc + 1) * 512)),
                                start=(hi == 0), stop=(hi == 31))
                        if hook is not None and hi % 4 == 3:
                            hook()
                    # RoPE: q' = q*cos + shuffle16(q)*sin_signed, -> bf16
                    dst = (qT_sb[hs] if oc < 8 else kT_sb[hs])[oc % 8]
                    for sc in range(2):
                        pks = pk[:, sc, :]
                        cs = cos_sb[:, sc * 512:(sc + 1) * 512]
                        sn = sin_sb[:, sc * 512:(sc + 1) * 512]
                        qrot = rpool.tile([128, 512], F32, tag="r", name="qrot")
                        nc.vector.stream_shuffle(qrot, pks, SHUF_MASK)
                        t1 = rpool.tile([128, 512], F32, tag="r", name="t1")
                        nc.vector.tensor_tensor(t1, pks, cs, ALU.mult)
                        t2 = rpool.tile([128, 512], F32, tag="r", name="t2")
                        nc.gpsimd.tensor_tensor(t2, qrot, sn, ALU.mult)
                        nc.vector.tensor_tensor(
                            dst[:, sc * 512:(sc + 1) * 512],
                            t1, t2, ALU.add)
                    if post_pair is not None and oci % 2 == 1:
                        post_pair(oci // 2)
                if finish is not None:
                    finish()

        # ---------------- Phase B: attention, scores kept as S^T [k, q] ------
        # Scores go into [128, 2, 512] PSUM tiles (2 k-blocks per group),
        # exp'd in one ACTIVATE per off-diagonal group, accumulated into a
        # running DVE sum, folded to [128, 512], and reduced+broadcast by a
        # single all-ones stationary matmul (den lands replicated on every
        # PSUM partition). av matmuls trail their exp by two groups; the
        # finalize (recip + av*recip + DMA out) is deferred by one unit.
        def make_attn_emitter(phb, tri_pe=False, weave=False):
            # tri_pe mode is used when woven into a projection phase: the
            # triangle mask is applied by a tiny PE matmul (identity moving,
            # tri^T stationary, accumulate) so the score->exp chain never
            # hops through the RoPE-congested DVE queue, and PSUM shrinks to
            # ps(2 banks) + shared av/den(2 banks) beside pqk's 4.
            qp_es = phb.enter_context(
                tc.tile_pool(name="es", bufs=2 if weave else 4))
            esump = phb.enter_context(tc.tile_pool(name="esum", bufs=2))
            smallp = phb.enter_context(tc.tile_pool(name="small", bufs=2))
            # av/den/misc pools are created before ps so their banks land on
            # the predecessor phase's just-freed PSUM (their first writes are
            # late); the score tiles then get fresh banks and never wait on
            # the previous phase's last PSUM readers.
            if weave:
                pav = pmisc = phb.enter_context(
                    tc.tile_pool(name="pavd", bufs=2, space="PSUM"))
                av_tag = den_tag = "avd"
            else:
                pav = phb.enter_context(
                    tc.tile_pool(name="pav", bufs=2, space="PSUM"))
                pmisc = phb.enter_context(
                    tc.tile_pool(name="pmisc", bufs=2, space="PSUM"))
                av_tag, den_tag = "av", "misc"
            ps = phb.enter_context(
                tc.tile_pool(name="ps", bufs=1 if weave else 2,
                             space="PSUM"))
            mp = None
            if mask_mode == "full":
                mp = phb.enter_context(tc.tile_pool(name="msk", bufs=3))

            state = {"pending": None, "pend_den": None}

            def emit_den():
                # den matmul for the previous unit, deferred so the next
                # unit's score matmuls cover the exp->tree->fold latency
                if state["pend_den"] is None:
                    return
                fold, den = state["pend_den"]
                nc.tensor.matmul(den, ones_sq, fold, start=True, stop=True)
                state["pend_den"] = None

            def finalize(av, den, hh, qc):
                recip = smallp.tile([128, 512], F32, tag="recip")
                nc.vector.reciprocal_approx_fast(recip, den)
                at_t = smallp.tile([128, 512], BF16, tag="at_t")
                nc.vector.tensor_tensor(at_t, av, recip, ALU.mult)
                nc.sync.dma_start(
                    out=at_s.ap()[hh * 128:(hh + 1) * 128,
                                  qc * 512:(qc + 1) * 512],
                    in_=at_t)

            def unit_slices(qc, hh):
                # generator: yields once per score group so the caller can
                # interleave foreign PE work between the slices
                emit_den()
                if state["pending"] is not None:
                    finalize(*state["pending"])
                    state["pending"] = None
                nblk = 4 * qc + 4 if mask_mode == "causal" else 16
                ng = nblk // 2
                av = pav.tile([128, 512], F32, tag=av_tag, name="av")
                acc = None
                navs = [0]
                pend_av = []          # (es_tile, j2, kb, q_lo) awaiting av mm

                def emit_avs(upto):
                    while pend_av and len(pend_av) > upto:
                        es, j2, kb, q_lo = pend_av.pop(0)
                        qs = slice(q_lo, 512)
                        khalf, kloc = kb // 8, kb % 8
                        nc.tensor.matmul(
                            av[:, qs],
                            v_sb[khalf][:, kloc, hh * 128:(hh + 1) * 128],
                            es[:, j2, qs],
                            start=(navs[0] == 0), stop=(navs[0] == nblk - 1))
                        navs[0] += 1

                # diagonal groups first: their serial per-region exps overlap
                # the off-diagonal work that follows. The vd=0 block is the
                # first av emitted and covers the full [0:512] PSUM range, so
                # the accumulation start flag is sound.
                if mask_mode == "causal":
                    order = [2 * qc, 2 * qc + 1] + list(range(2 * qc))
                else:
                    order = list(range(ng))
                for gi, g in enumerate(order):
                    es = qp_es.tile([128, 2, 512], BF16, tag="es", name="es")
                    sps = ps.tile([128, 2, 512], F32, tag="sps")
                    diag = False
                    blk = []
                    for j2 in range(2):
                        kb = 2 * g + j2
                        vd = kb - 4 * qc   # diagonal block index
                        q_lo = (128 * vd
                                if (mask_mode == "causal" and vd > 0)
                                else 0)
                        qs = slice(q_lo, 512)
                        khalf, kloc = kb // 8, kb % 8
                        is_d = mask_mode == "causal" and vd >= 0
                        nc.tensor.matmul(
                            sps[:, j2, qs],
                            kT_sb[khalf][hh][:, kloc * 128:(kloc + 1) * 128],
                            qT_sb[qc // 2][hh][:, (qc % 2) * 512 + q_lo:
                                               (qc % 2) * 512 + 512],
                            start=True, stop=not (is_d and tri_pe))
                        if is_d:
                            diag = True
                            mq = slice(128 * vd, 128 * vd + 128)
                            if tri_pe:
                                nc.tensor.matmul(
                                    sps[:, j2, mq], triT_sb, id_sb,
                                    start=False, stop=True)
                            else:
                                nc.vector.tensor_tensor(
                                    sps[:, j2, mq], sps[:, j2, mq], tri_sb,
                                    ALU.add)
                        elif mask_mode == "full":
                            mt = mp.tile([128, 512], F32, tag="mt")
                            nc.sync.dma_start(
                                out=mt,
                                in_=maskT.ap()[kb * 128:(kb + 1) * 128,
                                               qc * 512:(qc + 1) * 512])
                            nc.vector.tensor_tensor(sps[:, j2, :],
                                                    sps[:, j2, :], mt,
                                                    ALU.add)
                        blk.append((kb, q_lo))
                    # exp: one ACTIVATE per clean group; per-region on the
                    # diagonal (unwritten PSUM slivers stay out of the AP)
                    if not diag:
                        nc.scalar.activation(es, sps, ACTF.Exp,
                                             scale=inv_sqrt_d)
                    else:
                        for j2, (kb, q_lo) in enumerate(blk):
                            if q_lo > 0:
                                nc.vector.memset(es[:, j2, 0:q_lo], 0.0)
                            qs = slice(q_lo, 512)
                            nc.scalar.activation(es[:, j2, qs],
                                                 sps[:, j2, qs],
                                                 ACTF.Exp, scale=inv_sqrt_d)
                    # incremental tree: acc += es (DVE), frees es early
                    if gi == 0:
                        acc = es
                    else:
                        if gi == 1:
                            t = esump.tile([128, 2, 512], BF16, tag="e2")
                            nc.vector.tensor_tensor(t, acc, es, ALU.add)
                            acc = t
                        else:
                            nc.vector.tensor_tensor(acc, acc, es, ALU.add)
                    for j2, (kb, q_lo) in enumerate(blk):
                        pend_av.append((es, j2, kb, q_lo))
                    # av matmuls trail their exp by one/two groups
                    emit_avs(2 if weave else 4)
                    yield
                emit_avs(0)
                # fold the two k-block columns -> [128, 512]
                fold = smallp.tile([128, 512], BF16, tag="fold")
                nc.vector.tensor_tensor(fold, acc[:, 0, :], acc[:, 1, :],
                                        ALU.add)
                # den replicated across all 128 partitions via all-ones lhsT;
                # the matmul itself is deferred into the next unit
                den = pmisc.tile([128, 512], F32, tag=den_tag, name="den")
                state["pend_den"] = (fold, den)
                state["pending"] = (av, den, hh, qc)

            def emit_unit(qc, hh):
                for _ in unit_slices(qc, hh):
                    pass

            def flush():
                emit_den()
                if state["pending"] is not None:
                    finalize(*state["pending"])
                    state["pending"] = None

            return emit_unit, flush, pmisc, unit_slices

        # ---------------- Phase C: output projection -------------------------
        def make_c_emitter(phc, pop):
            atp = phc.enter_context(tc.tile_pool(name="atl", bufs=3))
            otp = phc.enter_context(tc.tile_pool(name="ot", bufs=4))
            wop = phc.enter_context(tc.tile_pool(name="wo", bufs=3))
            state = {"at_g": {}, "ot": {}, "wo_sl": None}

            def load_stg(stg):
                at_g = atp.tile([128, 8, 512], BF16, tag="at_g",
                                name=f"at_g{stg}")
                nc.sync.dma_start(
                    out=at_g,
                    in_=at_s.ap()[:, stg * 512:(stg + 1) * 512].rearrange(
                        "(hc p) s -> p hc s", p=128))
                state["at_g"][stg] = at_g

            def emit_unit(stg, st_l, o8):
                # one PSUM bank: out[st, o8] = sum_hc at^T wo
                # o8-major within a stage: wo slice loads once per (stg, o8),
                # the four ot tiles of the stage stay live until its end.
                at_g = state["at_g"][stg]
                st = stg * 4 + st_l
                sl = slice(st_l * 128, (st_l + 1) * 128)
                if st_l == 0:
                    wo_sl = wop.tile([128, 8, 512], BF16, tag="wo_sl",
                                     name=f"wo{stg}_{o8}")
                    nc.sync.dma_start(
                        out=wo_sl,
                        in_=wo_t.ap()[:, :, o8 * 512:(o8 + 1) * 512])
                    state["wo_sl"] = wo_sl
                wo_sl = state["wo_sl"]
                if o8 == 0:
                    state["ot"][st] = otp.tile([128, 8, 512], BF16, tag="ot",
                                               name=f"ot{st}")
                ot = state["ot"][st]
                po = pop.tile([128, 512], F32, tag="misc", name="po")
                for hc in range(8):
                    nc.tensor.matmul(
                        po,
                        at_g[:, hc, sl],
                        wo_sl[:, hc, :],
                        start=(hc == 0), stop=(hc == 7))
                nc.scalar.activation(ot[:, o8, :], po, ACTF.Copy)
                if o8 == 7:
                    nc.sync.dma_start(
                        out=out_p.ap()[st * 128:(st + 1) * 128, :],
                        in_=ot.rearrange("p a b -> p (a b)"))
                    del state["ot"][st]

            return emit_unit, load_stg

        # ================= schedule =================
        from collections import deque

        if mask_mode == "causal":
            # half 0 with B01 (qc0, qc1) woven into the QK oc-pair stream
            def attn_setup0(qk_stack):
                _, flush_b0, _, gen = make_attn_emitter(qk_stack, tri_pe=True,
                                                        weave=True)
                pending = deque()

                def hook():
                    while pending:
                        try:
                            next(pending[0])
                            return
                        except StopIteration:
                            pending.popleft()

                def post_pair(p):
                    pending.append(gen(0, p))
                    pending.append(gen(1, p))

                def finish():
                    while pending:
                        try:
                            next(pending[0])
                        except StopIteration:
                            pending.popleft()
                    flush_b0()

                return hook, post_pair, finish

            with ExitStack() as ph0:
                emit_half(0, ph0, attn_setup0)

            with ExitStack() as ph1:
                emit_half(1, ph1)

            # B23 with C(stg0, stg1) interleaved
            with ExitStack() as phbc:
                emit_unit_b, flush_b, pmisc, _ = make_attn_emitter(
                    phbc, tri_pe=True)
                emit_unit_c, load_stg = make_c_emitter(phbc, pmisc)
                load_stg(0)
                load_stg(1)
                c_units = [(stg, st_l, o8)
                           for stg in (0, 1)
                           for o8 in range(8)
                           for st_l in range(4)]
                for qc in (2, 3):
                    for hh in range(HEADS_PER_CORE):
                        emit_unit_b(qc, hh)
                        if qc == 3 and hh == 0:
                            # at_s rows for stg2 are complete once the
                            # (qc2, hh7) finalize ran inside this unit
                            load_stg(2)
                        # ~4 C units per B unit balances the two streams
                        for _ in range(4):
                            if c_units:
                                emit_unit_c(*c_units.pop(0))
                flush_b()
                load_stg(3)
                while c_units:
                    emit_unit_c(*c_units.pop(0))

                # C(stg2, stg3) dense
                for stg in (2, 3):
                    for o8 in range(8):
                        for st_l in range(4):
                            emit_unit_c(stg, st_l, o8)
        else:
            with ExitStack() as ph0:
                emit_half(0, ph0)
            with ExitStack() as ph1:
                emit_half(1, ph1)
            with ExitStack() as phbc:
                emit_unit_b, flush_b, pmisc, _ = make_attn_emitter(phbc)
                emit_unit_c, load_stg = make_c_emitter(phbc, pmisc)
                for qc in range(4):
                    for hh in range(HEADS_PER_CORE):
                        emit_unit_b(qc, hh)
                # all units emitted; flush and drain C for every stg in order
                flush_b()
                for stg in range(4):
                    load_stg(stg)
                    for o8 in range(8):
                        for st_l in range(4):
                            emit_unit_c(stg, st_l, o8)

    nc.compile()
    return nc


_PROGRAM_CACHE = {}


def _get_program(mask_mode):
    if mask_mode not in _PROGRAM_CACHE:
        _PROGRAM_CACHE[mask_mode] = _build_program(mask_mode)
    return _PROGRAM_CACHE[mask_mode]


def _classify_mask(attention_mask):
    m = np.asarray(attention_mask)
    if not np.any(m):
        return "none"
    neg = np.float32(np.finfo(np.float32).min)
    causal = np.triu(np.full((S, S), neg, dtype=np.float32), k=1)
    for b in range(m.shape[0]):
        if not np.array_equal(m[b, 0], causal):
            return "full"
    return "causal"


def _prep_core_inputs(hidden_states, attention_mask, position_ids, W_pack, W_o,
                      mask_mode):
    from ml_dtypes import bfloat16

    hidden_states = np.asarray(hidden_states, dtype=np.float32)
    W_pack = np.asarray(W_pack, dtype=np.float32)
    W_o = np.asarray(W_o, dtype=np.float32)
    pos = np.asarray(position_ids).astype(np.int64)

    cos_t, sin_t = _rope_tables(int(pos.max()) + 1)
    # per-batch gathered + transposed + row-permuted (+ sign folded into sin)
    cosT_b, sinT_b = [], []
    for b in range(B):
        c = cos_t[pos[b]][:, PERM].T
        s = (sin_t[pos[b]][:, PERM] * SIGN[None, :]).T
        cosT_b.append(np.ascontiguousarray(c.astype(bfloat16)))
        sinT_b.append(np.ascontiguousarray(s.astype(bfloat16)))

    # x_t[p, c, s] = hidden[b, s, c*128+p]
    x_b = [np.ascontiguousarray(
        hidden_states[b].T.reshape(32, 128, S).transpose(1, 0, 2)
        .astype(bfloat16)) for b in range(B)]

    tri_m = None
    triT_m = idm_m = None
    maskT_b = None
    if mask_mode == "causal":
        kk = np.arange(128)[:, None]
        qq = np.arange(128)[None, :]
        tri_m = np.where(kk <= qq, 0.0, NEG_BIG).astype(np.float32)
        triT_m = np.ascontiguousarray(tri_m.T.astype(bfloat16))
        idm_m = np.ascontiguousarray(np.eye(128, dtype=np.float32)
                                     .astype(bfloat16))
    elif mask_mode == "full":
        m = np.asarray(attention_mask, dtype=np.float32)
        maskT_b = [np.ascontiguousarray(m[b, 0].T) for b in range(B)]

    in_maps = []
    for cidx in range(8):
        b, g = cidx // 4, cidx % 4
        # per-head d-permuted q/k weight rows, head-major columns in wqk
        qrows = np.concatenate(
            [g * HG + hh * 128 + PERM for hh in range(HEADS_PER_CORE)])
        krows = HIDDEN + qrows
        vrows = 2 * HIDDEN + g * HG + np.arange(HG)
        wqk = np.concatenate([W_pack[qrows], W_pack[krows]], axis=0)  # [2048,4096]
        # wqk_t[oc, p, c, o] = wqk[oc*128+o, c*128+p]
        wqk_t = np.ascontiguousarray(
            wqk.reshape(16, 128, 32, 128).transpose(0, 3, 2, 1)
            .astype(bfloat16))
        wv = W_pack[vrows]                                            # [1024,4096]
        # wv_t[ov, c, p, o] = wv[ov*512+o, c*128+p]
        wv_t = np.ascontiguousarray(
            wv.reshape(2, 512, 32, 128).transpose(0, 2, 3, 1).astype(bfloat16))
        # wo_t[p, hc, o] = W_o[o, g*HG + hc*128 + p]
        wo_t = np.ascontiguousarray(
            W_o[:, g * HG:(g + 1) * HG].reshape(HIDDEN, 8, 128)
            .transpose(2, 1, 0).astype(bfloat16))
        im = {"x_t": x_b[b], "wqk_t": wqk_t, "wv_t": wv_t, "wo_t": wo_t,
              "cosT": cosT_b[b], "sinT": sinT_b[b]}
        if mask_mode == "causal":
            im["tri"] = tri_m
            im["triT"] = triT_m
            im["idm"] = idm_m
        elif mask_mode == "full":
            im["maskT"] = maskT_b[b]
        in_maps.append(im)
    return in_maps


def _run(hidden_states, attention_mask, position_ids, W_pack, W_o,
         trace=False, trace_kwargs=None):
    from concourse.bass_utils import run_bass_kernel_spmd

    mask_mode = _classify_mask(attention_mask)
    nc = _get_program(mask_mode)
    in_maps = _prep_core_inputs(hidden_states, attention_mask, position_ids,
                                W_pack, W_o, mask_mode)
    try:
        res = run_bass_kernel_spmd(nc, in_maps, list(range(8)), trace=trace,
                                   **(trace_kwargs or {}))
    except Exception:
        # transient NRT_EXEC_UNIT_UNRECOVERABLE wedges recover on retry
        import time as _time
        _time.sleep(15)
        res = run_bass_kernel_spmd(nc, in_maps, list(range(8)), trace=trace,
                                   **(trace_kwargs or {}))
    out = np.zeros((B, S, HIDDEN), dtype=np.float32)
    for c in range(8):
        out[c // 4] += np.asarray(res.results[c]["out_p"], dtype=np.float32)
    return out, res


def kernel(hidden_states, attention_mask, position_ids, W_pack, W_o):
    out, _ = _run(hidden_states, attention_mask, position_ids, W_pack, W_o)
    return out


# revision 45
# speedup vs baseline: 1.0988x; 1.0003x over previous
"""Trainium2 Bass kernel for a single attention layer (Baichuan-style W_pack
attention with rotary embeddings), sharded over 8 NeuronCores:
tensor-parallel over 4 head groups x data-parallel over 2 batches.

v3: scheduling overhaul toward the bf16 PE stream floor (~1.01 ms):
 - V-proj runs before QK-proj in each half so the x DMA stream is consumed
   at arrival rate (kills the ~20 us x-wait stall at each half start).
 - softmax denominator: single all-ones [128,128] stationary matmul whose
   PSUM output is den replicated across all partitions (no [1,512] recip
   chain, no separate broadcast matmul), fed by an incremental DVE tree +
   fold so there is one den matmul per (head, q-chunk).
 - exp batched per 2 score blocks (one ACTIVATE over [128,1024]) off the
   diagonal; av matmuls trail their exp by two groups so the PE never
   waits on the ACT exp.
 - O-proj work for seq groups 0,1 is interleaved into the qc2/qc3
   attention emission to fill residual exp-chain bubbles.

Contract: kernel(**inputs) takes the FULL unsharded inputs and returns the
FULL output [2, 2048, 4096] float32. All sharding / gathering happens here.
"""

import math
import sys

import numpy as np

for _p in ("/opt/trn_rl_repo", "/root/.axon_site/_ro/trn_rl_repo"):
    if _p not in sys.path:
        sys.path.insert(0, _p)

HIDDEN = 4096
N_HEADS = 32
HEAD_DIM = 128
BASE = 10000.0
B = 2
S = 2048
HEADS_PER_CORE = 8          # 32 heads / 4 groups
HG = 1024                   # head-group width = 8 heads * 128
NEG_BIG = -1.0e9

# RoPE partner permutation: quadrant q holds [lo_d 16q..16q+15, hi_d 64+16q..]
# so the rotate-half partner of new-row i is i+-16 inside its 32-row quadrant,
# reachable by DVE stream_shuffle.
PERM = np.zeros(128, dtype=np.int64)
for _q in range(4):
    PERM[32 * _q: 32 * _q + 16] = np.arange(16 * _q, 16 * _q + 16)
    PERM[32 * _q + 16: 32 * _q + 32] = 64 + np.arange(16 * _q, 16 * _q + 16)
SHUF_MASK = [(i + 16) % 32 for i in range(32)]
# sign of the sin term per (new) row: -1 where original d < 64
SIGN = np.where(PERM < 64, -1.0, 1.0).astype(np.float32)


def _rope_tables(max_pos):
    inv_freq = 1.0 / (BASE ** (np.arange(0, HEAD_DIM, 2, dtype=np.float32) / HEAD_DIM))
    t = np.arange(max_pos, dtype=np.float32)
    freqs = np.outer(t, inv_freq)                      # [P, 64]
    emb = np.concatenate((freqs, freqs), axis=-1)      # [P, 128]
    return np.cos(emb).astype(np.float32), np.sin(emb).astype(np.float32)


def _build_program(mask_mode):
    """mask_mode: 'causal' (block-skip + shared triangle mask),
    'none' (dense, no mask), 'full' (dense, stream mask tiles)."""
    import concourse.bacc as bacc
    import concourse.mybir as mybir
    import concourse.tile as tile
    from contextlib import ExitStack

    F32 = mybir.dt.float32
    BF16 = mybir.dt.bfloat16
    ALU = mybir.AluOpType
    ACTF = mybir.ActivationFunctionType

    nc = bacc.Bacc("TRN2", target_bir_lowering=False, debug=False)

    # pre-tiled inputs (host side does all layout work)
    x_t = nc.declare_dram_parameter("x_t", [128, 32, S], BF16, isOutput=False)
    wqk_t = nc.declare_dram_parameter("wqk_t", [16, 128, 32, 128], BF16,
                                      isOutput=False)
    wv_t = nc.declare_dram_parameter("wv_t", [2, 32, 128, 512], BF16,
                                     isOutput=False)
    wo_t = nc.declare_dram_parameter("wo_t", [128, 8, HIDDEN], BF16,
                                     isOutput=False)
    cosT = nc.declare_dram_parameter("cosT", [128, S], BF16, isOutput=False)
    sinT = nc.declare_dram_parameter("sinT", [128, S], BF16, isOutput=False)
    if mask_mode == "causal":
        tri = nc.declare_dram_parameter("tri", [128, 128], F32, isOutput=False)
        triT = nc.declare_dram_parameter("triT", [128, 128], BF16,
                                         isOutput=False)
        idm = nc.declare_dram_parameter("idm", [128, 128], BF16,
                                        isOutput=False)
    elif mask_mode == "full":
        maskT = nc.declare_dram_parameter("maskT", [S, S], F32, isOutput=False)
    out_p = nc.declare_dram_parameter("out_p", [S, HIDDEN], BF16, isOutput=True)

    at_s = nc.dram_tensor("at_scratch", [HG, S], BF16)

    inv_sqrt_d = 1.0 / math.sqrt(HEAD_DIM)

    with tile.TileContext(nc, pool_alloc_mode="queue") as tc, ExitStack() as top:
        const_pool = top.enter_context(tc.tile_pool(name="consts", bufs=1))
        ones_f32 = const_pool.tile([128, 128], F32)
        nc.vector.memset(ones_f32, 1.0)
        ones_sq = const_pool.tile([128, 128], BF16)
        nc.vector.tensor_copy(ones_sq, ones_f32)
        if mask_mode == "causal":
            tri_sb = const_pool.tile([128, 128], F32)
            nc.sync.dma_start(out=tri_sb, in_=tri.ap())
            triT_sb = const_pool.tile([128, 128], BF16)
            nc.sync.dma_start(out=triT_sb, in_=triT.ap())
            id_sb = const_pool.tile([128, 128], BF16)
            nc.sync.dma_start(out=id_sb, in_=idm.ap())

        # SBUF-resident q/k/v; q/k split per head so attention units only
        # wait on the head they read (dep tracking is per-tile)
        res_pool = top.enter_context(tc.tile_pool(name="resident", bufs=1))
        qT_sb = [[res_pool.tile([128, 1024], BF16, name=f"qT{h}_{hh}")
                  for hh in range(HEADS_PER_CORE)] for h in range(2)]
        kT_sb = [[res_pool.tile([128, 1024], BF16, name=f"kT{h}_{hh}")
                  for hh in range(HEADS_PER_CORE)] for h in range(2)]
        v_sb = [res_pool.tile([128, 8, HG], BF16, name=f"v{h}")
                for h in range(2)]

        # ---------------- Phase A: V then QK projection per half -------------
        def emit_half(hs, pha, attn_setup=None):
            s0 = hs * 1024
            # h-chunk sweep order: B-block (16..31) first (its DMA is issued
            # first), A-block (0..15) last.
            H_ORDER = list(range(16, 32)) + list(range(16))
            xpoolA = pha.enter_context(tc.tile_pool(name="xhalfA", bufs=1))
            xpoolB = pha.enter_context(tc.tile_pool(name="xhalfB", bufs=1))
            xtA = xpoolA.tile([128, 16, 1024], BF16, name=f"xtA{hs}")
            xtB = xpoolB.tile([128, 16, 1024], BF16, name=f"xtB{hs}")
            xin = x_t.ap()[:, :, s0:s0 + 1024]

            def emit_x_dma(c):
                dst = xtB[:, c - 16, :] if c >= 16 else xtA[:, c, :]
                nc.sync.dma_start(out=dst, in_=xin[:, c, :])

            def xt_slice(c, sl):
                return xtB[:, c - 16, sl] if c >= 16 else xtA[:, c, sl]

            # --- v projection first, x-stationary, out = v [s, o] ------------
            # x-chunk DMAs are interleaved with the wv tile loads in
            # consumption order so the first pass streams at DMA rate
            with ExitStack() as vv:
                wvp = vv.enter_context(tc.tile_pool(name="wvt", bufs=4))
                pv = vv.enter_context(
                    tc.tile_pool(name="pv", bufs=8, space="PSUM"))
                for ov in range(2):          # v-dim chunks of 512
                    vb = [pv.tile([128, 512], F32, tag="vb", name=f"vb{i}")
                          for i in range(8)]
                    for hi, h in enumerate(H_ORDER):
                        wv_tile = wvp.tile([128, 512], BF16, tag="wv_tile")
                        if ov == 0:
                            emit_x_dma(h)
                        nc.sync.dma_start(out=wv_tile, in_=wv_t.ap()[ov, h])
                        for sc in range(8):
                            nc.tensor.matmul(
                                vb[sc],
                                xt_slice(h, slice(sc * 128, (sc + 1) * 128)),
                                wv_tile,
                                start=(hi == 0), stop=(hi == 31))
                    for sc in range(8):
                        dst = v_sb[hs][:, sc, ov * 512:(ov + 1) * 512]
                        if sc % 2 == 0:
                            nc.scalar.activation(dst, vb[sc], ACTF.Copy)
                        else:
                            nc.vector.tensor_copy(dst, vb[sc])

            # --- q,k projection, weight-stationary, out = projT [o, s] -------
            with ExitStack() as qk:
                cspool = qk.enter_context(tc.tile_pool(name="cossin", bufs=1))
                cos_sb = cspool.tile([128, 1024], BF16)
                nc.sync.dma_start(out=cos_sb, in_=cosT.ap()[:, s0:s0 + 1024])
                sin_sb = cspool.tile([128, 1024], BF16)
                nc.sync.dma_start(out=sin_sb, in_=sinT.ap()[:, s0:s0 + 1024])

                wpool = qk.enter_context(tc.tile_pool(name="wqk", bufs=2))
                pqk = qk.enter_context(
                    tc.tile_pool(name="pqk", bufs=2, space="PSUM"))
                rpool = qk.enter_context(tc.tile_pool(name="rope", bufs=3))
                hook = post_pair = finish = None
                if attn_setup is not None:
                    hook, post_pair, finish = attn_setup(qk)
                # q/k oc pairs back-to-back so head p's attention can weave
                # into the remaining projection stream
                ocs = ([x for p in range(8) for x in (p, p + 8)]
                       if attn_setup else list(range(16)))
                for oci, oc in enumerate(ocs):  # o chunks of 128 (head tiles)
                    w_oc = wpool.tile([128, 32, 128], BF16, tag="w_oc")
                    nc.sync.dma_start(out=w_oc, in_=wqk_t.ap()[oc])
                    pk = pqk.tile([128, 2, 512], F32, tag="pk")
                    for hi, h in enumerate(H_ORDER):
                        for sc in range(2):
                            nc.tensor.matmul(
                                pk[:, sc, :], w_oc[:, h, :],
                                xt_slice(h, slice(sc * 512, (sc + 1) * 512)),
                                start=(hi == 0), stop=(hi == 31))
                        if hook is not None and hi % 4 == 3:
                            hook()
                    # RoPE: q' = q*cos + shuffle16(q)*sin_signed, -> bf16
                    dst = (qT_sb[hs] if oc < 8 else kT_sb[hs])[oc % 8]
                    for sc in range(2):
                        pks = pk[:, sc, :]
                        cs = cos_sb[:, sc * 512:(sc + 1) * 512]
                        sn = sin_sb[:, sc * 512:(sc + 1) * 512]
                        qrot = rpool.tile([128, 512], F32, tag="r", name="qrot")
                        nc.vector.stream_shuffle(qrot, pks, SHUF_MASK)
                        t1 = rpool.tile([128, 512], F32, tag="r", name="t1")
                        nc.vector.tensor_tensor(t1, pks, cs, ALU.mult)
                        t2 = rpool.tile([128, 512], F32, tag="r", name="t2")
                        nc.gpsimd.tensor_tensor(t2, qrot, sn, ALU.mult)
                        nc.vector.tensor_tensor(
                            dst[:, sc * 512:(sc + 1) * 512],
                            t1, t2, ALU.add)
                    if post_pair is not None and oci % 2 == 1:
                        post_pair(oci // 2)
                if finish is not None:
                    finish()

        # ---------------- Phase B: attention, scores kept as S^T [k, q] ------
        # Scores go into [128, 2, 512] PSUM tiles (2 k-blocks per group),
        # exp'd in one ACTIVATE per off-diagonal group, accumulated into a
        # running DVE sum, folded to [128, 512], and reduced+broadcast by a
        # single all-ones stationary matmul (den lands replicated on every
        # PSUM partition). av matmuls trail their exp by two groups; the
        # finalize (recip + av*recip + DMA out) is deferred by one unit.
        def make_attn_emitter(phb, tri_pe=False, weave=False):
            # tri_pe mode is used when woven into a projection phase: the
            # triangle mask is applied by a tiny PE matmul (identity moving,
            # tri^T stationary, accumulate) so the score->exp chain never
            # hops through the RoPE-congested DVE queue, and PSUM shrinks to
            # ps(2 banks) + shared av/den(2 banks) beside pqk's 4.
            qp_es = phb.enter_context(
                tc.tile_pool(name="es", bufs=2 if weave else 4))
            esump = phb.enter_context(tc.tile_pool(name="esum", bufs=2))
            smallp = phb.enter_context(tc.tile_pool(name="small", bufs=2))
            # av/den/misc pools are created before ps so their banks land on
            # the predecessor phase's just-freed PSUM (their first writes are
            # late); the score tiles then get fresh banks and never wait on
            # the previous phase's last PSUM readers.
            if weave:
                pav = pmisc = phb.enter_context(
                    tc.tile_pool(name="pavd", bufs=2, space="PSUM"))
                av_tag = den_tag = "avd"
            else:
                pav = phb.enter_context(
                    tc.tile_pool(name="pav", bufs=2, space="PSUM"))
                pmisc = phb.enter_context(
                    tc.tile_pool(name="pmisc", bufs=2, space="PSUM"))
                av_tag, den_tag = "av", "misc"
            ps = phb.enter_context(
                tc.tile_pool(name="ps", bufs=1 if weave else 2,
                             space="PSUM"))
            mp = None
            if mask_mode == "full":
                mp = phb.enter_context(tc.tile_pool(name="msk", bufs=3))

            state = {"pending": None, "pend_den": None}

            def emit_den():
                # den matmul for the previous unit, deferred so the next
                # unit's score matmuls cover the exp->tree->fold latency
                if state["pend_den"] is None:
                    return
                fold, den = state["pend_den"]
                nc.tensor.matmul(den, ones_sq, fold, start=True, stop=True)
                state["pend_den"] = None

            def finalize(av, den, hh, qc):
                recip = smallp.tile([128, 512], F32, tag="recip")
                nc.vector.reciprocal_approx_fast(recip, den)
                at_t = smallp.tile([128, 512], BF16, tag="at_t")
                nc.vector.tensor_tensor(at_t, av, recip, ALU.mult)
                nc.sync.dma_start(
                    out=at_s.ap()[hh * 128:(hh + 1) * 128,
                                  qc * 512:(qc + 1) * 512],
                    in_=at_t)

            def unit_slices(qc, hh):
                # generator: yields once per score group so the caller can
                # interleave foreign PE work between the slices
                emit_den()
                if state["pending"] is not None:
                    finalize(*state["pending"])
                    state["pending"] = None
                nblk = 4 * qc + 4 if mask_mode == "causal" else 16
                ng = nblk // 2
                av = pav.tile([128, 512], F32, tag=av_tag, name="av")
                acc = None
                navs = [0]
                pend_av = []          # (es_tile, j2, kb, q_lo) awaiting av mm

                def emit_avs(upto):
                    while pend_av and len(pend_av) > upto:
                        es, j2, kb, q_lo = pend_av.pop(0)
                        qs = slice(q_lo, 512)
                        khalf, kloc = kb // 8, kb % 8
                        nc.tensor.matmul(
                            av[:, qs],
                            v_sb[khalf][:, kloc, hh * 128:(hh + 1) * 128],
                            es[:, j2, qs],
                            start=(navs[0] == 0), stop=(navs[0] == nblk - 1))
                        navs[0] += 1

                # diagonal groups first: their serial per-region exps overlap
                # the off-diagonal work that follows. The vd=0 block is the
                # first av emitted and covers the full [0:512] PSUM range, so
                # the accumulation start flag is sound.
                if mask_mode == "causal":
                    order = [2 * qc, 2 * qc + 1] + list(range(2 * qc))
                else:
                    order = list(range(ng))
                for gi, g in enumerate(order):
                    es = qp_es.tile([128, 2, 512], BF16, tag="es", name="es")
                    sps = ps.tile([128, 2, 512], F32, tag="sps")
                    diag = False
                    blk = []
                    for j2 in range(2):
                        kb = 2 * g + j2
                        vd = kb - 4 * qc   # diagonal block index
                        q_lo = (128 * vd
                                if (mask_mode == "causal" and vd > 0)
                                else 0)
                        qs = slice(q_lo, 512)
                        khalf, kloc = kb // 8, kb % 8
                        is_d = mask_mode == "causal" and vd >= 0
                        nc.tensor.matmul(
                            sps[:, j2, qs],
                            kT_sb[khalf][hh][:, kloc * 128:(kloc + 1) * 128],
                            qT_sb[qc // 2][hh][:, (qc % 2) * 512 + q_lo:
                                               (qc % 2) * 512 + 512],
                            start=True, stop=not (is_d and tri_pe))
                        if is_d:
                            diag = True
                            mq = slice(128 * vd, 128 * vd + 128)
                            if tri_pe:
                                nc.tensor.matmul(
                                    sps[:, j2, mq], triT_sb, id_sb,
                                    start=False, stop=True)
                            else:
                                nc.vector.tensor_tensor(
                                    sps[:, j2, mq], sps[:, j2, mq], tri_sb,
                                    ALU.add)
                        elif mask_mode == "full":
                            mt = mp.tile([128, 512], F32, tag="mt")
                            nc.sync.dma_start(
                                out=mt,
                                in_=maskT.ap()[kb * 128:(kb + 1) * 128,
                                               qc * 512:(qc + 1) * 512])
                            nc.vector.tensor_tensor(sps[:, j2, :],
                                                    sps[:, j2, :], mt,
                                                    ALU.add)
                        blk.append((kb, q_lo))
                    # exp: one ACTIVATE per clean group; per-region on the
                    # diagonal (unwritten PSUM slivers stay out of the AP)
                    if not diag:
                        nc.scalar.activation(es, sps, ACTF.Exp,
                                             scale=inv_sqrt_d)
                    else:
                        for j2, (kb, q_lo) in enumerate(blk):
                            if q_lo > 0:
                                nc.vector.memset(es[:, j2, 0:q_lo], 0.0)
                            qs = slice(q_lo, 512)
                            nc.scalar.activation(es[:, j2, qs],
                                                 sps[:, j2, qs],
                                                 ACTF.Exp, scale=inv_sqrt_d)
                    # incremental tree: acc += es (DVE), frees es early
                    if gi == 0:
                        acc = es
                    else:
                        if gi == 1:
                            t = esump.tile([128, 2, 512], BF16, tag="e2")
                            nc.vector.tensor_tensor(t, acc, es, ALU.add)
                            acc = t
                        else:
                            nc.vector.tensor_tensor(acc, acc, es, ALU.add)
                    for j2, (kb, q_lo) in enumerate(blk):
                        pend_av.append((es, j2, kb, q_lo))
                    # av matmuls trail their exp by one/two groups
                    emit_avs(2 if weave else 4)
                    yield
                emit_avs(0)
                # fold the two k-block columns -> [128, 512]
                fold = smallp.tile([128, 512], BF16, tag="fold")
                nc.vector.tensor_tensor(fold, acc[:, 0, :], acc[:, 1, :],
                                        ALU.add)
                # den replicated across all 128 partitions via all-ones lhsT;
                # the matmul itself is deferred into the next unit
                den = pmisc.tile([128, 512], F32, tag=den_tag, name="den")
                state["pend_den"] = (fold, den)
                state["pending"] = (av, den, hh, qc)

            def emit_unit(qc, hh):
                for _ in unit_slices(qc, hh):
                    pass

            def flush():
                emit_den()
                if state["pending"] is not None:
                    finalize(*state["pending"])
                    state["pending"] = None

            return emit_unit, flush, pmisc, unit_slices

        # ---------------- Phase C: output projection -------------------------
        def make_c_emitter(phc, pop):
            atp = phc.enter_context(tc.tile_pool(name="atl", bufs=3))
            otp = phc.enter_context(tc.tile_pool(name="ot", bufs=4))
            wop = phc.enter_context(tc.tile_pool(name="wo", bufs=3))
            state = {"at_g": {}, "ot": {}, "wo_sl": None}

            def load_stg(stg):
                at_g = atp.tile([128, 8, 512], BF16, tag="at_g",
                                name=f"at_g{stg}")
                nc.sync.dma_start(
                    out=at_g,
                    in_=at_s.ap()[:, stg * 512:(stg + 1) * 512].rearrange(
                        "(hc p) s -> p hc s", p=128))
                state["at_g"][stg] = at_g

            def emit_unit(stg, st_l, o8):
                # one PSUM bank: out[st, o8] = sum_hc at^T wo
                # o8-major within a stage: wo slice loads once per (stg, o8),
                # the four ot tiles of the stage stay live until its end.
                at_g = state["at_g"][stg]
                st = stg * 4 + st_l
                sl = slice(st_l * 128, (st_l + 1) * 128)
                if st_l == 0:
                    wo_sl = wop.tile([128, 8, 512], BF16, tag="wo_sl",
                                     name=f"wo{stg}_{o8}")
                    nc.sync.dma_start(
                        out=wo_sl,
                        in_=wo_t.ap()[:, :, o8 * 512:(o8 + 1) * 512])
                    state["wo_sl"] = wo_sl
                wo_sl = state["wo_sl"]
                if o8 == 0:
                    state["ot"][st] = otp.tile([128, 8, 512], BF16, tag="ot",
                                               name=f"ot{st}")
                ot = state["ot"][st]
                po = pop.tile([128, 512], F32, tag="misc", name="po")
                for hc in range(8):
                    nc.tensor.matmul(
                        po,
                        at_g[:, hc, sl],
                        wo_sl[:, hc, :],
                        start=(hc == 0), stop=(hc == 7))
                nc.scalar.activation(ot[:, o8, :], po, ACTF.Copy)
                if o8 == 7:
                    nc.sync.dma_start(
                        out=out_p.ap()[st * 128:(st + 1) * 128, :],
                        in_=ot.rearrange("p a b -> p (a b)"))
                    del state["ot"][st]

            return emit_unit, load_stg

        # ================= schedule =================
        from collections import deque

        if mask_mode == "causal":
            # half 0 with B01 (qc0, qc1) woven into the QK oc-pair stream
            def attn_setup0(qk_stack):
                _, flush_b0, _, gen = make_attn_emitter(qk_stack, tri_pe=True,
                                                        weave=True)
                pending = deque()

                def hook():
                    while pending:
                        try:
                            next(pending[0])
                            return
                        except StopIteration:
                            pending.popleft()

                def post_pair(p):
                    pending.append(gen(0, p))
                    pending.append(gen(1, p))

                def finish():
                    while pending:
                        try:
                            next(pending[0])
                        except StopIteration:
                            pending.popleft()
                    flush_b0()

                return hook, post_pair, finish

            with ExitStack() as ph0:
                emit_half(0, ph0, attn_setup0)

            with ExitStack() as ph1:
                emit_half(1, ph1)

            # B23 with C(stg0, stg1) interleaved
            with ExitStack() as phbc:
                emit_unit_b, flush_b, pmisc, _ = make_attn_emitter(
                    phbc, tri_pe=True)
                emit_unit_c, load_stg = make_c_emitter(phbc, pmisc)
                load_stg(0)
                load_stg(1)
                c_units = [(stg, st_l, o8)
                           for stg in (0, 1)
                           for o8 in range(8)
                           for st_l in range(4)]
                for qc in (2, 3):
                    for hh in range(HEADS_PER_CORE):
                        emit_unit_b(qc, hh)
                        if qc == 3 and hh == 0:
                            # at_s rows for stg2 are complete once the
                            # (qc2, hh7) finalize ran inside this unit
                            load_stg(2)
                        # ~4 C units per B unit balances the two streams
                        for _ in range(4):
                            if c_units:
                                emit_unit_c(*c_units.pop(0))
                flush_b()
                load_stg(3)
                while c_units:
                    emit_unit_c(*c_units.pop(0))

                # C(stg2, stg3) dense
                for stg in (2, 3):
                    for o8 in range(8):
                        for st_l in range(4):
                            emit_unit_c(stg, st_l, o8)
        else:
            with ExitStack() as ph0:
                emit_half(0, ph0)
            with ExitStack() as ph1:
                emit_half(1, ph1)
            with ExitStack() as phbc:
                emit_unit_b, flush_b, pmisc, _ = make_attn_emitter(phbc)
                emit_unit_c, load_stg = make_c_emitter(phbc, pmisc)
                for qc in range(4):
                    for hh in range(HEADS_PER_CORE):
                        emit_unit_b(qc, hh)
                # all units emitted; flush and drain C for every stg in order
                flush_b()
                for stg in range(4):
                    load_stg(stg)
                    for o8 in range(8):
                        for st_l in range(4):
                            emit_unit_c(stg, st_l, o8)

    nc.compile()
    return nc


_PROGRAM_CACHE = {}


def _get_program(mask_mode):
    if mask_mode not in _PROGRAM_CACHE:
        _PROGRAM_CACHE[mask_mode] = _build_program(mask_mode)
    return _PROGRAM_CACHE[mask_mode]


def _classify_mask(attention_mask):
    m = np.asarray(attention_mask)
    if not np.any(m):
        return "none"
    neg = np.float32(np.finfo(np.float32).min)
    causal = np.triu(np.full((S, S), neg, dtype=np.float32), k=1)
    for b in range(m.shape[0]):
        if not np.array_equal(m[b, 0], causal):
            return "full"
    return "causal"


def _prep_core_inputs(hidden_states, attention_mask, position_ids, W_pack, W_o,
                      mask_mode):
    from ml_dtypes import bfloat16

    hidden_states = np.asarray(hidden_states, dtype=np.float32)
    W_pack = np.asarray(W_pack, dtype=np.float32)
    W_o = np.asarray(W_o, dtype=np.float32)
    pos = np.asarray(position_ids).astype(np.int64)

    cos_t, sin_t = _rope_tables(int(pos.max()) + 1)
    # per-batch gathered + transposed + row-permuted (+ sign folded into sin)
    cosT_b, sinT_b = [], []
    for b in range(B):
        c = cos_t[pos[b]][:, PERM].T
        s = (sin_t[pos[b]][:, PERM] * SIGN[None, :]).T
        cosT_b.append(np.ascontiguousarray(c.astype(bfloat16)))
        sinT_b.append(np.ascontiguousarray(s.astype(bfloat16)))

    # x_t[p, c, s] = hidden[b, s, c*128+p]
    x_b = [np.ascontiguousarray(
        hidden_states[b].T.reshape(32, 128, S).transpose(1, 0, 2)
        .astype(bfloat16)) for b in range(B)]

    tri_m = None
    triT_m = idm_m = None
    maskT_b = None
    if mask_mode == "causal":
        kk = np.arange(128)[:, None]
        qq = np.arange(128)[None, :]
        tri_m = np.where(kk <= qq, 0.0, NEG_BIG).astype(np.float32)
        triT_m = np.ascontiguousarray(tri_m.T.astype(bfloat16))
        idm_m = np.ascontiguousarray(np.eye(128, dtype=np.float32)
                                     .astype(bfloat16))
    elif mask_mode == "full":
        m = np.asarray(attention_mask, dtype=np.float32)
        maskT_b = [np.ascontiguousarray(m[b, 0].T) for b in range(B)]

    in_maps = []
    for cidx in range(8):
        b, g = cidx // 4, cidx % 4
        # per-head d-permuted q/k weight rows, head-major columns in wqk
        qrows = np.concatenate(
            [g * HG + hh * 128 + PERM for hh in range(HEADS_PER_CORE)])
        krows = HIDDEN + qrows
        vrows = 2 * HIDDEN + g * HG + np.arange(HG)
        wqk = np.concatenate([W_pack[qrows], W_pack[krows]], axis=0)  # [2048,4096]
        # wqk_t[oc, p, c, o] = wqk[oc*128+o, c*128+p]
        wqk_t = np.ascontiguousarray(
            wqk.reshape(16, 128, 32, 128).transpose(0, 3, 2, 1)
            .astype(bfloat16))
        wv = W_pack[vrows]                                            # [1024,4096]
        # wv_t[ov, c, p, o] = wv[ov*512+o, c*128+p]
        wv_t = np.ascontiguousarray(
            wv.reshape(2, 512, 32, 128).transpose(0, 2, 3, 1).astype(bfloat16))
        # wo_t[p, hc, o] = W_o[o, g*HG + hc*128 + p]
        wo_t = np.ascontiguousarray(
            W_o[:, g * HG:(g + 1) * HG].reshape(HIDDEN, 8, 128)
            .transpose(2, 1, 0).astype(bfloat16))
        im = {"x_t": x_b[b], "wqk_t": wqk_t, "wv_t": wv_t, "wo_t": wo_t,
              "cosT": cosT_b[b], "sinT": sinT_b[b]}
        if mask_mode == "causal":
            im["tri"] = tri_m
            im["triT"] = triT_m
            im["idm"] = idm_m
        elif mask_mode == "full":
            im["maskT"] = maskT_b[b]
        in_maps.append(im)
    return in_maps


def _run(hidden_states, attention_mask, position_ids, W_pack, W_o,
         trace=False, trace_kwargs=None):
    from concourse.bass_utils import run_bass_kernel_spmd

    mask_mode = _classify_mask(attention_mask)
    nc = _get_program(mask_mode)
    in_maps = _prep_core_inputs(hidden_states, attention_mask, position_ids,
                                W_pack, W_o, mask_mode)
    try:
        res = run_bass_kernel_spmd(nc, in_maps, list(range(8)), trace=trace,
                                   **(trace_kwargs or {}))
    except Exception:
        # transient NRT_EXEC_UNIT_UNRECOVERABLE wedges recover on retry
        import time as _time
        _time.sleep(15)
        res = run_bass_kernel_spmd(nc, in_maps, list(range(8)), trace=trace,
                                   **(trace_kwargs or {}))
    out = np.zeros((B, S, HIDDEN), dtype=np.float32)
    for c in range(8):
        out[c // 4] += np.asarray(res.results[c]["out_p"], dtype=np.float32)
    return out, res


def kernel(hidden_states, attention_mask, position_ids, W_pack, W_o):
    out, _ = _run(hidden_states, attention_mask, position_ids, W_pack, W_o)
    return out


# revision 46
# speedup vs baseline: 1.1047x; 1.0053x over previous
"""Trainium2 Bass kernel for a single attention layer (Baichuan-style W_pack
attention with rotary embeddings), sharded over 8 NeuronCores:
tensor-parallel over 4 head groups x data-parallel over 2 batches.

v5 (~1.088 ms, from the 1.167 ms v2 baseline; bf16 PE stream floor is
~1.01 ms and the PE runs at the 216 ns/matmul issue floor nearly
everywhere):
 - V-proj runs before QK-proj in each half so the x DMA stream is consumed
   at arrival rate (x-chunk DMAs interleaved with wv tiles in consumption
   order).
 - qc0/qc1 attention is WOVEN into QK-proj of half 0: the oc loop runs in
   (q-head h, k-head h) pairs and a generator emits attention slices
   between h-chunks, so the exp/DVE chains hide entirely under the
   projection stream (measured +0.8 us over the pure stream floor).
   In woven mode the triangle mask is applied by a tiny PE matmul
   (tri^T stationary x identity moving, PSUM accumulate) so the
   score->exp chain never hops through the RoPE-congested DVE queue.
 - q/k SBUF residents are split per head so attention only waits on the
   head it reads (tile-granular dependency tracking).
 - softmax denominator: single all-ones [128,128] stationary matmul whose
   PSUM output is den replicated across all partitions (no [1,512] recip
   chain, no broadcast matmul), fed by an incremental DVE tree + fold;
   the den matmul and the finalize are deferred into the next unit.
 - exp batched per 2 score blocks (one ACTIVATE over [128,1024]) off the
   diagonal; av matmuls trail their exp; es slivers zeroed on DVE (the
   gpsimd memset queue was a 2.3 us/op serializer).
 - O-proj for seq stages 0,1 is interleaved into the qc2/qc3 attention
   emission; stages 2,3 run dense with at_g/wo prefetch; wo is streamed
   per (stage, o8) slice.
 - PSUM pools are created so score tiles land on fresh banks and never
   wait on the previous phase's last PSUM readers.

Contract: kernel(**inputs) takes the FULL unsharded inputs and returns the
FULL output [2, 2048, 4096] float32. All sharding / gathering happens here.
"""

import math
import sys

import numpy as np

for _p in ("/opt/trn_rl_repo", "/root/.axon_site/_ro/trn_rl_repo"):
    if _p not in sys.path:
        sys.path.insert(0, _p)

HIDDEN = 4096
N_HEADS = 32
HEAD_DIM = 128
BASE = 10000.0
B = 2
S = 2048
HEADS_PER_CORE = 8          # 32 heads / 4 groups
HG = 1024                   # head-group width = 8 heads * 128
NEG_BIG = -1.0e9

# RoPE partner permutation: quadrant q holds [lo_d 16q..16q+15, hi_d 64+16q..]
# so the rotate-half partner of new-row i is i+-16 inside its 32-row quadrant,
# reachable by DVE stream_shuffle.
PERM = np.zeros(128, dtype=np.int64)
for _q in range(4):
    PERM[32 * _q: 32 * _q + 16] = np.arange(16 * _q, 16 * _q + 16)
    PERM[32 * _q + 16: 32 * _q + 32] = 64 + np.arange(16 * _q, 16 * _q + 16)
SHUF_MASK = [(i + 16) % 32 for i in range(32)]
# sign of the sin term per (new) row: -1 where original d < 64
SIGN = np.where(PERM < 64, -1.0, 1.0).astype(np.float32)


def _rope_tables(max_pos):
    inv_freq = 1.0 / (BASE ** (np.arange(0, HEAD_DIM, 2, dtype=np.float32) / HEAD_DIM))
    t = np.arange(max_pos, dtype=np.float32)
    freqs = np.outer(t, inv_freq)                      # [P, 64]
    emb = np.concatenate((freqs, freqs), axis=-1)      # [P, 128]
    return np.cos(emb).astype(np.float32), np.sin(emb).astype(np.float32)


def _build_program(mask_mode):
    """mask_mode: 'causal' (block-skip + shared triangle mask),
    'none' (dense, no mask), 'full' (dense, stream mask tiles)."""
    import concourse.bacc as bacc
    import concourse.mybir as mybir
    import concourse.tile as tile
    from contextlib import ExitStack

    F32 = mybir.dt.float32
    BF16 = mybir.dt.bfloat16
    ALU = mybir.AluOpType
    ACTF = mybir.ActivationFunctionType

    nc = bacc.Bacc("TRN2", target_bir_lowering=False, debug=False)

    # pre-tiled inputs (host side does all layout work)
    x_t = nc.declare_dram_parameter("x_t", [128, 32, S], BF16, isOutput=False)
    wqk_t = nc.declare_dram_parameter("wqk_t", [16, 128, 32, 128], BF16,
                                      isOutput=False)
    wv_t = nc.declare_dram_parameter("wv_t", [2, 32, 128, 512], BF16,
                                     isOutput=False)
    wo_t = nc.declare_dram_parameter("wo_t", [128, 8, HIDDEN], BF16,
                                     isOutput=False)
    cosT = nc.declare_dram_parameter("cosT", [128, S], BF16, isOutput=False)
    sinT = nc.declare_dram_parameter("sinT", [128, S], BF16, isOutput=False)
    if mask_mode == "causal":
        tri = nc.declare_dram_parameter("tri", [128, 128], F32, isOutput=False)
        triT = nc.declare_dram_parameter("triT", [128, 128], BF16,
                                         isOutput=False)
        idm = nc.declare_dram_parameter("idm", [128, 128], BF16,
                                        isOutput=False)
    elif mask_mode == "full":
        maskT = nc.declare_dram_parameter("maskT", [S, S], F32, isOutput=False)
    out_p = nc.declare_dram_parameter("out_p", [S, HIDDEN], BF16, isOutput=True)

    at_s = nc.dram_tensor("at_scratch", [HG, S], BF16)

    inv_sqrt_d = 1.0 / math.sqrt(HEAD_DIM)

    with tile.TileContext(nc, pool_alloc_mode="queue") as tc, ExitStack() as top:
        const_pool = top.enter_context(tc.tile_pool(name="consts", bufs=1))
        ones_f32 = const_pool.tile([128, 128], F32)
        nc.vector.memset(ones_f32, 1.0)
        ones_sq = const_pool.tile([128, 128], BF16)
        nc.vector.tensor_copy(ones_sq, ones_f32)
        if mask_mode == "causal":
            tri_sb = const_pool.tile([128, 128], F32)
            nc.sync.dma_start(out=tri_sb, in_=tri.ap())
            triT_sb = const_pool.tile([128, 128], BF16)
            nc.sync.dma_start(out=triT_sb, in_=triT.ap())
            id_sb = const_pool.tile([128, 128], BF16)
            nc.sync.dma_start(out=id_sb, in_=idm.ap())

        # SBUF-resident q/k/v; q/k split per head so attention units only
        # wait on the head they read (dep tracking is per-tile)
        res_pool = top.enter_context(tc.tile_pool(name="resident", bufs=1))
        qT_sb = [[res_pool.tile([128, 1024], BF16, name=f"qT{h}_{hh}")
                  for hh in range(HEADS_PER_CORE)] for h in range(2)]
        kT_sb = [[res_pool.tile([128, 1024], BF16, name=f"kT{h}_{hh}")
                  for hh in range(HEADS_PER_CORE)] for h in range(2)]
        v_sb = [res_pool.tile([128, 8, HG], BF16, name=f"v{h}")
                for h in range(2)]

        # ---------------- Phase A: V then QK projection per half -------------
        def emit_half(hs, pha, attn_setup=None):
            s0 = hs * 1024
            # h-chunk sweep order: B-block (16..31) first (its DMA is issued
            # first), A-block (0..15) last.
            H_ORDER = list(range(16, 32)) + list(range(16))
            xpoolA = pha.enter_context(tc.tile_pool(name="xhalfA", bufs=1))
            xpoolB = pha.enter_context(tc.tile_pool(name="xhalfB", bufs=1))
            xtA = xpoolA.tile([128, 16, 1024], BF16, name=f"xtA{hs}")
            xtB = xpoolB.tile([128, 16, 1024], BF16, name=f"xtB{hs}")
            xin = x_t.ap()[:, :, s0:s0 + 1024]

            def emit_x_dma(c):
                dst = xtB[:, c - 16, :] if c >= 16 else xtA[:, c, :]
                nc.sync.dma_start(out=dst, in_=xin[:, c, :])

            def xt_slice(c, sl):
                return xtB[:, c - 16, sl] if c >= 16 else xtA[:, c, sl]

            # --- v projection first, x-stationary, out = v [s, o] ------------
            # x-chunk DMAs are interleaved with the wv tile loads in
            # consumption order so the first pass streams at DMA rate
            with ExitStack() as vv:
                wvp = vv.enter_context(tc.tile_pool(name="wvt", bufs=4))
                pv = vv.enter_context(
                    tc.tile_pool(name="pv", bufs=8, space="PSUM"))
                for ov in range(2):          # v-dim chunks of 512
                    vb = [pv.tile([128, 512], F32, tag="vb", name=f"vb{i}")
                          for i in range(8)]
                    for hi, h in enumerate(H_ORDER):
                        wv_tile = wvp.tile([128, 512], BF16, tag="wv_tile")
                        if ov == 0:
                            emit_x_dma(h)
                        nc.sync.dma_start(out=wv_tile, in_=wv_t.ap()[ov, h])
                        for sc in range(8):
                            nc.tensor.matmul(
                                vb[sc],
                                xt_slice(h, slice(sc * 128, (sc + 1) * 128)),
                                wv_tile,
                                start=(hi == 0), stop=(hi == 31))
                    for sc in range(8):
                        dst = v_sb[hs][:, sc, ov * 512:(ov + 1) * 512]
                        if sc % 2 == 0:
                            nc.scalar.activation(dst, vb[sc], ACTF.Copy)
                        else:
                            nc.vector.tensor_copy(dst, vb[sc])

            # --- q,k projection, weight-stationary, out = projT [o, s] -------
            with ExitStack() as qk:
                cspool = qk.enter_context(tc.tile_pool(name="cossin", bufs=1))
                cos_sb = cspool.tile([128, 1024], BF16)
                nc.sync.dma_start(out=cos_sb, in_=cosT.ap()[:, s0:s0 + 1024])
                sin_sb = cspool.tile([128, 1024], BF16)
                nc.sync.dma_start(out=sin_sb, in_=sinT.ap()[:, s0:s0 + 1024])

                wpool = qk.enter_context(tc.tile_pool(name="wqk", bufs=2))
                pqk = qk.enter_context(
                    tc.tile_pool(name="pqk", bufs=2, space="PSUM"))
                rpool = qk.enter_context(tc.tile_pool(name="rope", bufs=3))
                hook = post_pair = finish = None
                if attn_setup is not None:
                    hook, post_pair, finish = attn_setup(qk)
                # q/k oc pairs back-to-back so head p's attention can weave
                # into the remaining projection stream
                ocs = ([x for p in range(8) for x in (p, p + 8)]
                       if attn_setup else list(range(16)))
                for oci, oc in enumerate(ocs):  # o chunks of 128 (head tiles)
                    w_oc = wpool.tile([128, 32, 128], BF16, tag="w_oc")
                    nc.sync.dma_start(out=w_oc, in_=wqk_t.ap()[oc])
                    pk = pqk.tile([128, 2, 512], F32, tag="pk")
                    for hi, h in enumerate(H_ORDER):
                        for sc in range(2):
                            nc.tensor.matmul(
                                pk[:, sc, :], w_oc[:, h, :],
                                xt_slice(h, slice(sc * 512, (sc + 1) * 512)),
                                start=(hi == 0), stop=(hi == 31))
                        if hook is not None and hi % 4 == 3:
                            hook()
                    # RoPE: q' = q*cos + shuffle16(q)*sin_signed, -> bf16
                    dst = (qT_sb[hs] if oc < 8 else kT_sb[hs])[oc % 8]
                    for sc in range(2):
                        pks = pk[:, sc, :]
                        cs = cos_sb[:, sc * 512:(sc + 1) * 512]
                        sn = sin_sb[:, sc * 512:(sc + 1) * 512]
                        qrot = rpool.tile([128, 512], F32, tag="r", name="qrot")
                        nc.vector.stream_shuffle(qrot, pks, SHUF_MASK)
                        t1 = rpool.tile([128, 512], F32, tag="r", name="t1")
                        nc.vector.tensor_tensor(t1, pks, cs, ALU.mult)
                        t2 = rpool.tile([128, 512], F32, tag="r", name="t2")
                        nc.gpsimd.tensor_tensor(t2, qrot, sn, ALU.mult)
                        nc.vector.tensor_tensor(
                            dst[:, sc * 512:(sc + 1) * 512],
                            t1, t2, ALU.add)
                    if post_pair is not None and oci % 2 == 1:
                        post_pair(oci // 2)
                if finish is not None:
                    finish()

        # ---------------- Phase B: attention, scores kept as S^T [k, q] ------
        # Scores go into [128, 2, 512] PSUM tiles (2 k-blocks per group),
        # exp'd in one ACTIVATE per off-diagonal group, accumulated into a
        # running DVE sum, folded to [128, 512], and reduced+broadcast by a
        # single all-ones stationary matmul (den lands replicated on every
        # PSUM partition). av matmuls trail their exp by two groups; the
        # finalize (recip + av*recip + DMA out) is deferred by one unit.
        def make_attn_emitter(phb, tri_pe=False, weave=False):
            # tri_pe mode is used when woven into a projection phase: the
            # triangle mask is applied by a tiny PE matmul (identity moving,
            # tri^T stationary, accumulate) so the score->exp chain never
            # hops through the RoPE-congested DVE queue, and PSUM shrinks to
            # ps(2 banks) + shared av/den(2 banks) beside pqk's 4.
            qp_es = phb.enter_context(
                tc.tile_pool(name="es", bufs=2 if weave else 4))
            esump = phb.enter_context(tc.tile_pool(name="esum", bufs=2))
            smallp = phb.enter_context(tc.tile_pool(name="small", bufs=2))
            # av/den/misc pools are created before ps so their banks land on
            # the predecessor phase's just-freed PSUM (their first writes are
            # late); the score tiles then get fresh banks and never wait on
            # the previous phase's last PSUM readers.
            if weave:
                pav = pmisc = phb.enter_context(
                    tc.tile_pool(name="pavd", bufs=2, space="PSUM"))
                av_tag = den_tag = "avd"
            else:
                pav = phb.enter_context(
                    tc.tile_pool(name="pav", bufs=2, space="PSUM"))
                pmisc = phb.enter_context(
                    tc.tile_pool(name="pmisc", bufs=2, space="PSUM"))
                av_tag, den_tag = "av", "misc"
            ps = phb.enter_context(
                tc.tile_pool(name="ps", bufs=1 if weave else 2,
                             space="PSUM"))
            mp = None
            if mask_mode == "full":
                mp = phb.enter_context(tc.tile_pool(name="msk", bufs=3))

            state = {"pending": None, "pend_den": None}

            def emit_den():
                # den matmul for the previous unit, deferred so the next
                # unit's score matmuls cover the exp->tree->fold latency
                if state["pend_den"] is None:
                    return
                fold, den = state["pend_den"]
                nc.tensor.matmul(den, ones_sq, fold, start=True, stop=True)
                state["pend_den"] = None

            def finalize(av, den, hh, qc):
                recip = smallp.tile([128, 512], F32, tag="recip")
                nc.vector.reciprocal_approx_fast(recip, den)
                at_t = smallp.tile([128, 512], BF16, tag="at_t")
                nc.vector.tensor_tensor(at_t, av, recip, ALU.mult)
                nc.sync.dma_start(
                    out=at_s.ap()[hh * 128:(hh + 1) * 128,
                                  qc * 512:(qc + 1) * 512],
                    in_=at_t)

            def unit_slices(qc, hh):
                # generator: yields once per score group so the caller can
                # interleave foreign PE work between the slices
                emit_den()
                if state["pending"] is not None:
                    finalize(*state["pending"])
                    state["pending"] = None
                nblk = 4 * qc + 4 if mask_mode == "causal" else 16
                ng = nblk // 2
                av = pav.tile([128, 512], F32, tag=av_tag, name="av")
                acc = None
                navs = [0]
                pend_av = []          # (es_tile, j2, kb, q_lo) awaiting av mm

                def emit_avs(upto):
                    while pend_av and len(pend_av) > upto:
                        es, j2, kb, q_lo = pend_av.pop(0)
                        qs = slice(q_lo, 512)
                        khalf, kloc = kb // 8, kb % 8
                        nc.tensor.matmul(
                            av[:, qs],
                            v_sb[khalf][:, kloc, hh * 128:(hh + 1) * 128],
                            es[:, j2, qs],
                            start=(navs[0] == 0), stop=(navs[0] == nblk - 1))
                        navs[0] += 1

                # diagonal groups first: their serial per-region exps overlap
                # the off-diagonal work that follows. The vd=0 block is the
                # first av emitted and covers the full [0:512] PSUM range, so
                # the accumulation start flag is sound.
                if mask_mode == "causal":
                    order = [2 * qc, 2 * qc + 1] + list(range(2 * qc))
                else:
                    order = list(range(ng))
                for gi, g in enumerate(order):
                    es = qp_es.tile([128, 2, 512], BF16, tag="es", name="es")
                    sps = ps.tile([128, 2, 512], F32, tag="sps")
                    diag = False
                    blk = []
                    for j2 in range(2):
                        kb = 2 * g + j2
                        vd = kb - 4 * qc   # diagonal block index
                        q_lo = (128 * vd
                                if (mask_mode == "causal" and vd > 0)
                                else 0)
                        qs = slice(q_lo, 512)
                        khalf, kloc = kb // 8, kb % 8
                        is_d = mask_mode == "causal" and vd >= 0
                        nc.tensor.matmul(
                            sps[:, j2, qs],
                            kT_sb[khalf][hh][:, kloc * 128:(kloc + 1) * 128],
                            qT_sb[qc // 2][hh][:, (qc % 2) * 512 + q_lo:
                                               (qc % 2) * 512 + 512],
                            start=True, stop=not (is_d and tri_pe))
                        if is_d:
                            diag = True
                            mq = slice(128 * vd, 128 * vd + 128)
                            if tri_pe:
                                nc.tensor.matmul(
                                    sps[:, j2, mq], triT_sb, id_sb,
                                    start=False, stop=True)
                            else:
                                nc.vector.tensor_tensor(
                                    sps[:, j2, mq], sps[:, j2, mq], tri_sb,
                                    ALU.add)
                        elif mask_mode == "full":
                            mt = mp.tile([128, 512], F32, tag="mt")
                            nc.sync.dma_start(
                                out=mt,
                                in_=maskT.ap()[kb * 128:(kb + 1) * 128,
                                               qc * 512:(qc + 1) * 512])
                            nc.vector.tensor_tensor(sps[:, j2, :],
                                                    sps[:, j2, :], mt,
                                                    ALU.add)
                        blk.append((kb, q_lo))
                    # exp: one ACTIVATE per clean group; per-region on the
                    # diagonal (unwritten PSUM slivers stay out of the AP)
                    if not diag:
                        nc.scalar.activation(es, sps, ACTF.Exp,
                                             scale=inv_sqrt_d)
                    else:
                        for j2, (kb, q_lo) in enumerate(blk):
                            if q_lo > 0:
                                nc.vector.memset(es[:, j2, 0:q_lo], 0.0)
                            qs = slice(q_lo, 512)
                            nc.scalar.activation(es[:, j2, qs],
                                                 sps[:, j2, qs],
                                                 ACTF.Exp, scale=inv_sqrt_d)
                    # incremental tree: acc += es (DVE), frees es early
                    if gi == 0:
                        acc = es
                    else:
                        if gi == 1:
                            t = esump.tile([128, 2, 512], BF16, tag="e2")
                            nc.vector.tensor_tensor(t, acc, es, ALU.add)
                            acc = t
                        else:
                            nc.vector.tensor_tensor(acc, acc, es, ALU.add)
                    for j2, (kb, q_lo) in enumerate(blk):
                        pend_av.append((es, j2, kb, q_lo))
                    # av matmuls trail their exp by one/two groups
                    emit_avs(2 if weave else 4)
                    yield
                emit_avs(0)
                # fold the two k-block columns -> [128, 512]
                fold = smallp.tile([128, 512], BF16, tag="fold")
                nc.vector.tensor_tensor(fold, acc[:, 0, :], acc[:, 1, :],
                                        ALU.add)
                # den replicated across all 128 partitions via all-ones lhsT;
                # the matmul itself is deferred into the next unit
                den = pmisc.tile([128, 512], F32, tag=den_tag, name="den")
                state["pend_den"] = (fold, den)
                state["pending"] = (av, den, hh, qc)

            def emit_unit(qc, hh):
                for _ in unit_slices(qc, hh):
                    pass

            def flush():
                emit_den()
                if state["pending"] is not None:
                    finalize(*state["pending"])
                    state["pending"] = None

            return emit_unit, flush, pmisc, unit_slices

        # ---------------- Phase C: output projection -------------------------
        def make_c_emitter(phc, pop):
            atp = phc.enter_context(tc.tile_pool(name="atl", bufs=3))
            otp = phc.enter_context(tc.tile_pool(name="ot", bufs=4))
            wop = phc.enter_context(tc.tile_pool(name="wo", bufs=3))
            state = {"at_g": {}, "ot": {}, "wo_sl": None}

            def load_stg(stg):
                at_g = atp.tile([128, 8, 512], BF16, tag="at_g",
                                name=f"at_g{stg}")
                nc.sync.dma_start(
                    out=at_g,
                    in_=at_s.ap()[:, stg * 512:(stg + 1) * 512].rearrange(
                        "(hc p) s -> p hc s", p=128))
                state["at_g"][stg] = at_g

            def emit_unit(stg, st_l, o8):
                # one PSUM bank: out[st, o8] = sum_hc at^T wo
                # o8-major within a stage: wo slice loads once per (stg, o8),
                # the four ot tiles of the stage stay live until its end.
                at_g = state["at_g"][stg]
                st = stg * 4 + st_l
                sl = slice(st_l * 128, (st_l + 1) * 128)
                if st_l == 0:
                    wo_sl = wop.tile([128, 8, 512], BF16, tag="wo_sl",
                                     name=f"wo{stg}_{o8}")
                    nc.sync.dma_start(
                        out=wo_sl,
                        in_=wo_t.ap()[:, :, o8 * 512:(o8 + 1) * 512])
                    state["wo_sl"] = wo_sl
                wo_sl = state["wo_sl"]
                if o8 == 0:
                    state["ot"][st] = otp.tile([128, 8, 512], BF16, tag="ot",
                                               name=f"ot{st}")
                ot = state["ot"][st]
                po = pop.tile([128, 512], F32, tag="misc", name="po")
                for hc in range(8):
                    nc.tensor.matmul(
                        po,
                        at_g[:, hc, sl],
                        wo_sl[:, hc, :],
                        start=(hc == 0), stop=(hc == 7))
                nc.scalar.activation(ot[:, o8, :], po, ACTF.Copy)
                if o8 == 7:
                    nc.sync.dma_start(
                        out=out_p.ap()[st * 128:(st + 1) * 128, :],
                        in_=ot.rearrange("p a b -> p (a b)"))
                    del state["ot"][st]

            return emit_unit, load_stg

        # ================= schedule =================
        from collections import deque

        if mask_mode == "causal":
            # half 0 with B01 (qc0, qc1) woven into the QK oc-pair stream
            def attn_setup0(qk_stack):
                _, flush_b0, _, gen = make_attn_emitter(qk_stack, tri_pe=True,
                                                        weave=True)
                pending = deque()

                def hook():
                    while pending:
                        try:
                            next(pending[0])
                            return
                        except StopIteration:
                            pending.popleft()

                def post_pair(p):
                    pending.append(gen(0, p))
                    pending.append(gen(1, p))

                def finish():
                    while pending:
                        try:
                            next(pending[0])
                        except StopIteration:
                            pending.popleft()
                    flush_b0()

                return hook, post_pair, finish

            with ExitStack() as ph0:
                emit_half(0, ph0, attn_setup0)

            with ExitStack() as ph1:
                emit_half(1, ph1)

            # B23 with C(stg0, stg1) interleaved
            with ExitStack() as phbc:
                emit_unit_b, flush_b, pmisc, _ = make_attn_emitter(
                    phbc, tri_pe=True)
                emit_unit_c, load_stg = make_c_emitter(phbc, pmisc)
                load_stg(0)
                load_stg(1)
                c_units = [(stg, st_l, o8)
                           for stg in (0, 1)
                           for o8 in range(8)
                           for st_l in range(4)]
                for qc in (2, 3):
                    for hh in range(HEADS_PER_CORE):
                        emit_unit_b(qc, hh)
                        if qc == 3 and hh == 0:
                            # at_s rows for stg2 are complete once the
                            # (qc2, hh7) finalize ran inside this unit
                            load_stg(2)
                        # ~4 C units per B unit balances the two streams
                        for _ in range(4):
                            if c_units:
                                emit_unit_c(*c_units.pop(0))
                flush_b()
                load_stg(3)
                while c_units:
                    emit_unit_c(*c_units.pop(0))

                # C(stg2, stg3) dense
                for stg in (2, 3):
                    for o8 in range(8):
                        for st_l in range(4):
                            emit_unit_c(stg, st_l, o8)
        else:
            with ExitStack() as ph0:
                emit_half(0, ph0)
            with ExitStack() as ph1:
                emit_half(1, ph1)
            with ExitStack() as phbc:
                emit_unit_b, flush_b, pmisc, _ = make_attn_emitter(phbc)
                emit_unit_c, load_stg = make_c_emitter(phbc, pmisc)
                for qc in range(4):
                    for hh in range(HEADS_PER_CORE):
                        emit_unit_b(qc, hh)
                # all units emitted; flush and drain C for every stg in order
                flush_b()
                for stg in range(4):
                    load_stg(stg)
                    for o8 in range(8):
                        for st_l in range(4):
                            emit_unit_c(stg, st_l, o8)

    nc.compile()
    return nc


_PROGRAM_CACHE = {}


def _get_program(mask_mode):
    if mask_mode not in _PROGRAM_CACHE:
        _PROGRAM_CACHE[mask_mode] = _build_program(mask_mode)
    return _PROGRAM_CACHE[mask_mode]


def _classify_mask(attention_mask):
    m = np.asarray(attention_mask)
    if not np.any(m):
        return "none"
    neg = np.float32(np.finfo(np.float32).min)
    causal = np.triu(np.full((S, S), neg, dtype=np.float32), k=1)
    for b in range(m.shape[0]):
        if not np.array_equal(m[b, 0], causal):
            return "full"
    return "causal"


def _prep_core_inputs(hidden_states, attention_mask, position_ids, W_pack, W_o,
                      mask_mode):
    from ml_dtypes import bfloat16

    hidden_states = np.asarray(hidden_states, dtype=np.float32)
    W_pack = np.asarray(W_pack, dtype=np.float32)
    W_o = np.asarray(W_o, dtype=np.float32)
    pos = np.asarray(position_ids).astype(np.int64)

    cos_t, sin_t = _rope_tables(int(pos.max()) + 1)
    # per-batch gathered + transposed + row-permuted (+ sign folded into sin)
    cosT_b, sinT_b = [], []
    for b in range(B):
        c = cos_t[pos[b]][:, PERM].T
        s = (sin_t[pos[b]][:, PERM] * SIGN[None, :]).T
        cosT_b.append(np.ascontiguousarray(c.astype(bfloat16)))
        sinT_b.append(np.ascontiguousarray(s.astype(bfloat16)))

    # x_t[p, c, s] = hidden[b, s, c*128+p]
    x_b = [np.ascontiguousarray(
        hidden_states[b].T.reshape(32, 128, S).transpose(1, 0, 2)
        .astype(bfloat16)) for b in range(B)]

    tri_m = None
    triT_m = idm_m = None
    maskT_b = None
    if mask_mode == "causal":
        kk = np.arange(128)[:, None]
        qq = np.arange(128)[None, :]
        tri_m = np.where(kk <= qq, 0.0, NEG_BIG).astype(np.float32)
        triT_m = np.ascontiguousarray(tri_m.T.astype(bfloat16))
        idm_m = np.ascontiguousarray(np.eye(128, dtype=np.float32)
                                     .astype(bfloat16))
    elif mask_mode == "full":
        m = np.asarray(attention_mask, dtype=np.float32)
        maskT_b = [np.ascontiguousarray(m[b, 0].T) for b in range(B)]

    in_maps = []
    for cidx in range(8):
        b, g = cidx // 4, cidx % 4
        # per-head d-permuted q/k weight rows, head-major columns in wqk
        qrows = np.concatenate(
            [g * HG + hh * 128 + PERM for hh in range(HEADS_PER_CORE)])
        krows = HIDDEN + qrows
        vrows = 2 * HIDDEN + g * HG + np.arange(HG)
        wqk = np.concatenate([W_pack[qrows], W_pack[krows]], axis=0)  # [2048,4096]
        # wqk_t[oc, p, c, o] = wqk[oc*128+o, c*128+p]
        wqk_t = np.ascontiguousarray(
            wqk.reshape(16, 128, 32, 128).transpose(0, 3, 2, 1)
            .astype(bfloat16))
        wv = W_pack[vrows]                                            # [1024,4096]
        # wv_t[ov, c, p, o] = wv[ov*512+o, c*128+p]
        wv_t = np.ascontiguousarray(
            wv.reshape(2, 512, 32, 128).transpose(0, 2, 3, 1).astype(bfloat16))
        # wo_t[p, hc, o] = W_o[o, g*HG + hc*128 + p]
        wo_t = np.ascontiguousarray(
            W_o[:, g * HG:(g + 1) * HG].reshape(HIDDEN, 8, 128)
            .transpose(2, 1, 0).astype(bfloat16))
        im = {"x_t": x_b[b], "wqk_t": wqk_t, "wv_t": wv_t, "wo_t": wo_t,
              "cosT": cosT_b[b], "sinT": sinT_b[b]}
        if mask_mode == "causal":
            im["tri"] = tri_m
            im["triT"] = triT_m
            im["idm"] = idm_m
        elif mask_mode == "full":
            im["maskT"] = maskT_b[b]
        in_maps.append(im)
    return in_maps


def _run(hidden_states, attention_mask, position_ids, W_pack, W_o,
         trace=False, trace_kwargs=None):
    from concourse.bass_utils import run_bass_kernel_spmd

    mask_mode = _classify_mask(attention_mask)
    nc = _get_program(mask_mode)
    in_maps = _prep_core_inputs(hidden_states, attention_mask, position_ids,
                                W_pack, W_o, mask_mode)
    try:
        res = run_bass_kernel_spmd(nc, in_maps, list(range(8)), trace=trace,
                                   **(trace_kwargs or {}))
    except Exception:
        # transient NRT_EXEC_UNIT_UNRECOVERABLE wedges recover on retry
        import time as _time
        _time.sleep(15)
        res = run_bass_kernel_spmd(nc, in_maps, list(range(8)), trace=trace,
                                   **(trace_kwargs or {}))
    out = np.zeros((B, S, HIDDEN), dtype=np.float32)
    for c in range(8):
        out[c // 4] += np.asarray(res.results[c]["out_p"], dtype=np.float32)
    return out, res


def kernel(hidden_states, attention_mask, position_ids, W_pack, W_o):
    out, _ = _run(hidden_states, attention_mask, position_ids, W_pack, W_o)
    return out
